# revision 25
# baseline (speedup 1.0000x reference)
"""PathDiscovery Trainium2 Bass kernel.

Full pipeline on-device, data-parallel over batch (1 batch element per
NeuronCore, 8 cores):
  - fchange: mean |nf[t+1]-nf[t]| over (t, F)
  - source MLP -> z = logits * fchange -> top-8 (max8/max_index)
  - guided random walk with restart, 8 walks x 4 steps, vectorized over
    walks on partitions; the categorical sample is reproduced exactly as
    argmax_j probs_j * exp(gumbel_j) with host-precomputed (input
    independent) gumbel noise; adj/feature rows fetched by indirect DMA
  - bidirectional GRU encoding of gathered path features + path scorer

Host side: shard inputs over cores, precompute exp(gumbel)/uniform draws
from jax CPU threefry (deterministic, input-independent), pre-transpose
small weights, assemble outputs, softmax over K for weights.
"""

import os
import subprocess
import sys
import tempfile

import numpy as np

import concourse.bass as bass
import concourse.bacc as bacc
import concourse.mybir as mybir
from concourse import tile
from concourse.bass_utils import run_bass_kernel_spmd

B, T, N, F = 8, 12, 4096, 64
H = 64
K = 8
L = 5
ALPHA = 0.15
NT = N // 128          # 32 node tiles of 128
NCHUNK = N // 512      # 8 matmul chunks of 512
NHALF = N // 2
FD = mybir.dt.float32
I32 = mybir.dt.int32
U32 = mybir.dt.uint32
ALU = mybir.AluOpType
ACT = mybir.ActivationFunctionType
AX = mybir.AxisListType


# --------------------------------------------------------------------------
# Host-side RNG precompute (input independent; must match jax CPU threefry)
# --------------------------------------------------------------------------

_RNG_CACHE = None

_RNG_SCRIPT = r"""
import numpy as np
import jax, jax.numpy as jnp
B, K, L, N = 8, 8, 5, 4096
wkeys = jax.random.split(jax.random.key(42), B * K).reshape(B, K)
GUM = np.zeros((B, K, L - 1, N), np.float32)
UNI = np.zeros((B, K, L - 1), np.float32)
for b in range(B):
    for k in range(K):
        step_keys = jax.random.split(wkeys[b, k], L - 1)
        for i in range(L - 1):
            ku, kc = jax.random.split(step_keys[i])
            GUM[b, k, i] = np.asarray(jax.random.gumbel(kc, (N,), jnp.float32))
            UNI[b, k, i] = float(jax.random.uniform(ku))
np.savez(OUT_PATH, gum=GUM, uni=UNI)
"""


def _host_rng():
    """exp(gumbel) (B,K,4,N) f32 and uniforms (B,K,4) f32, via jax CPU."""
    global _RNG_CACHE
    if _RNG_CACHE is not None:
        return _RNG_CACHE
    # jax in this process may be bound to the axon backend; compute in a
    # clean subprocess pinned to the CPU backend so the threefry bits match
    # the reference implementation exactly.
    import jax  # just to locate the installed jax for the subprocess

    site_dir = os.path.dirname(os.path.dirname(jax.__file__))
    env = dict(os.environ)
    env.pop("TRN_TERMINAL_POOL_IPS", None)  # disables axon sitecustomize boot
    env["JAX_PLATFORMS"] = "cpu"
    env["PYTHONPATH"] = site_dir + os.pathsep + env.get("PYTHONPATH", "")
    with tempfile.TemporaryDirectory() as td:
        out_path = os.path.join(td, "rng.npz")
        script = f"OUT_PATH = {out_path!r}\n" + _RNG_SCRIPT
        subprocess.run(
            [sys.executable, "-c", script], env=env, check=True,
            capture_output=True,
        )
        dat = np.load(out_path)
        gum, uni = dat["gum"], dat["uni"]
    eg = np.exp(gum.astype(np.float64)).astype(np.float32)
    _RNG_CACHE = (eg, uni)
    return _RNG_CACHE


# --------------------------------------------------------------------------
# Bass kernel builder
# --------------------------------------------------------------------------

def build_nc(num_devices=1, debug=False):
    nc = bacc.Bacc(
        "TRN2",
        target_bir_lowering=False,
        debug=debug,
        num_devices=num_devices,
    )

    d_nf = nc.dram_tensor("nf", [T, N, F], FD, kind="ExternalInput")
    d_feats = nc.dram_tensor("feats", [N, F], FD, kind="ExternalInput")
    d_adj = nc.dram_tensor("adj", [N, N], FD, kind="ExternalInput")
    d_w1t = nc.dram_tensor("w1t", [F, H], FD, kind="ExternalInput")
    d_b1 = nc.dram_tensor("b1", [H, 1], FD, kind="ExternalInput")
    d_w2t = nc.dram_tensor("w2t", [H, 1], FD, kind="ExternalInput")
    d_b2 = nc.dram_tensor("b2", [1, 1], FD, kind="ExternalInput")
    gru_dram = {}
    for pre in ("f", "b"):
        for nm, shape in (
            ("ih_rzT", [F, 2 * H]), ("ih_nT", [F, H]),
            ("hh_rzT", [H, 2 * H]), ("hh_nT", [H, H]),
            ("bih_rz", [2 * H, 1]), ("bih_n", [H, 1]),
            ("bhh_rz", [2 * H, 1]), ("bhh_n", [H, 1]),
        ):
            gru_dram[f"{pre}_{nm}"] = nc.dram_tensor(
                f"{pre}_{nm}", shape, FD, kind="ExternalInput")
    d_wps1t = nc.dram_tensor("wps1t", [2 * H, H], FD, kind="ExternalInput")
    d_bps1 = nc.dram_tensor("bps1", [H, 1], FD, kind="ExternalInput")
    d_wps2t = nc.dram_tensor("wps2t", [H, 1], FD, kind="ExternalInput")
    d_bps2 = nc.dram_tensor("bps2", [1, 1], FD, kind="ExternalInput")
    d_eg = nc.dram_tensor("eg", [L - 1, 64, 512], FD, kind="ExternalInput")
    d_uni = nc.dram_tensor("uni", [L - 1, 1, K], FD, kind="ExternalInput")
    d_ident = nc.dram_tensor("ident", [128, 128], FD, kind="ExternalInput")
    d_selmat = nc.dram_tensor("selmat", [8, 64], FD, kind="ExternalInput")
    d_repmat = nc.dram_tensor("repmat", [8, 64], FD, kind="ExternalInput")
    d_base512 = nc.dram_tensor("base512", [64, 1], FD, kind="ExternalInput")
    d_schunk = nc.dram_tensor("schunk", [64, 1], FD, kind="ExternalInput")

    d_paths = nc.dram_tensor("paths_o", [K, L], I32, kind="ExternalOutput")
    d_pfT = nc.dram_tensor("pfT_o", [2 * H, K], FD, kind="ExternalOutput")
    d_scores = nc.dram_tensor("scores_o", [1, K], FD, kind="ExternalOutput")
    d_fch = nc.dram_tensor("fch_o", [1, N], FD, kind="ExternalOutput")
    d_z = nc.dram_tensor("z_o", [1, N], FD, kind="ExternalOutput")

    with tile.TileContext(nc) as tc:
        with (
            tc.tile_pool(name="const", bufs=1) as constp,
            tc.tile_pool(name="big", bufs=1) as bigp,
            tc.tile_pool(name="row", bufs=2) as rowp,
            tc.tile_pool(name="small", bufs=1) as smp,
            tc.tile_pool(name="ps", bufs=3, space="PSUM") as psp,
            tc.tile_pool(name="psh", bufs=1, space="PSUM") as psh,
        ):
            # ---------------- constants ----------------
            ident = constp.tile([128, 128], FD, tag="ident")
            nc.sync.dma_start(ident[:], d_ident[:])
            w1t = constp.tile([F, H], FD, tag="w1t")
            nc.sync.dma_start(w1t[:], d_w1t[:])
            b1 = constp.tile([H, 1], FD, tag="b1")
            nc.sync.dma_start(b1[:], d_b1[:])
            w2t = constp.tile([H, 1], FD, tag="w2t")
            nc.sync.dma_start(w2t[:], d_w2t[:])
            b2 = constp.tile([1, 1], FD, tag="b2")
            nc.sync.dma_start(b2[:], d_b2[:])
            gru = {}
            for nm, dtr in gru_dram.items():
                t_ = constp.tile(list(dtr.shape), FD, tag=nm)
                nc.sync.dma_start(t_[:], dtr[:])
                gru[nm] = t_
            wps1t = constp.tile([2 * H, H], FD, tag="wps1t")
            nc.sync.dma_start(wps1t[:], d_wps1t[:])
            bps1 = constp.tile([H, 1], FD, tag="bps1")
            nc.sync.dma_start(bps1[:], d_bps1[:])
            wps2t = constp.tile([H, 1], FD, tag="wps2t")
            nc.sync.dma_start(wps2t[:], d_wps2t[:])
            bps2 = constp.tile([1, 1], FD, tag="bps2")
            nc.sync.dma_start(bps2[:], d_bps2[:])

            ones_f8 = constp.tile([F, K], FD, tag="ones_f8")
            nc.vector.memset(ones_f8[:], 1.0)
            selmat = constp.tile([8, 64], FD, tag="selmat")
            nc.sync.dma_start(selmat[:], d_selmat[:])
            repmat = constp.tile([8, 64], FD, tag="repmat")
            nc.sync.dma_start(repmat[:], d_repmat[:])
            base512 = constp.tile([64, 1], FD, tag="base512")
            nc.sync.dma_start(base512[:], d_base512[:])
            schunk = constp.tile([64, 1], FD, tag="schunk")
            nc.sync.dma_start(schunk[:], d_schunk[:])

            iota512 = constp.tile([64, 512], FD, tag="iota512")
            nc.gpsimd.iota(
                iota512[:], pattern=[[1, 512]], base=0, channel_multiplier=0,
                allow_small_or_imprecise_dtypes=True,
            )

            uni_t = []
            for s in range(L - 1):
                t_ = constp.tile([1, K], FD, tag=f"uni{s}")
                nc.sync.dma_start(t_[:], d_uni[s])
                uni_t.append(t_)

            # persistent big tiles
            featsT = bigp.tile([F, N], FD, tag="featsT")
            sq64 = bigp.tile([64, 512], FD, tag="sq64")
            vis01 = bigp.tile([64, 512], FD, tag="vis01")
            src_col = bigp.tile([K, 1], FD, tag="src_col")
            path_f = bigp.tile([K, L], FD, tag="path_f")
            pfT = bigp.tile([2 * H, K], FD, tag="pfT")

            fch_row = rowp.tile([1, N], FD, tag="row")

            # ---------------- phase 1: fchange + featsT (scoped pools) ----
            with (
                tc.tile_pool(name="nfs", bufs=3) as nfp,
                tc.tile_pool(name="dts", bufs=2) as dtp,
            ):
                acc = nfp.tile([128, NT], FD, tag="acc")
                nc.vector.memset(acc[:], 0.0)
                nf_view = d_nf.ap().rearrange("t (a p) f -> t p a f", p=128)
                prev = nfp.tile([128, NT * F], FD, tag="nft")
                nc.gpsimd.dma_start(prev[:], nf_view[0])
                for t in range(1, T):
                    curt = nfp.tile([128, NT * F], FD, tag="nft")
                    nc.gpsimd.dma_start(curt[:], nf_view[t])
                    dtile = dtp.tile([128, NT * F], FD, tag="dtile")
                    nc.vector.tensor_tensor(
                        dtile[:], curt[:], prev[:], op=ALU.subtract)
                    r = smp.tile([128, NT], FD, tag="red")
                    nc.vector.tensor_reduce(
                        r[:], dtile[:].rearrange("p (a f) -> p a f", f=F),
                        axis=AX.X, op=ALU.add, apply_absolute_value=True,
                    )
                    nc.vector.tensor_tensor(acc[:], acc[:], r[:], op=ALU.add)
                    prev = curt
                # prev holds nf[T-1] = feats in (p, a, f) layout
                # fchange = acc / 704 -> transpose -> row
                fchm = smp.tile([128, NT], FD, tag="fchm")
                nc.vector.tensor_scalar(
                    fchm[:], acc[:],
                    float(np.float32(1.0) / np.float32((T - 1) * F)), None,
                    op0=ALU.mult)
                fch_ps = psp.tile([NT, 128], FD, tag="mm")
                nc.tensor.transpose(fch_ps[:], fchm[:], ident[:])
                fch_t = smp.tile([NT, 128], FD, tag="fch_t")
                nc.scalar.copy(fch_t[:], fch_ps[:])
                nc.sync.dma_start(fch_row[:], fch_t[:])
                nc.sync.dma_start(d_fch[:], fch_row[:])

                # featsT from transposed 128x64 tiles of feats
                for a in range(NT):
                    tp = psp.tile([F, 128], FD, tag="mm")
                    nc.tensor.transpose(
                        tp[:], prev[:, a * F:(a + 1) * F], ident[:])
                    nc.scalar.copy(featsT[:, a * 128:(a + 1) * 128], tp[:])

            # ---------------- sq -> sq64 (64, 512) ----------------
            sq_row = rowp.tile([1, N], FD, tag="row")
            for c in range(NCHUNK):
                sl = slice(c * 512, (c + 1) * 512)
                sqc = smp.tile([F, 512], FD, tag="sqc")
                nc.scalar.square(sqc[:], featsT[:, sl])
                sq_ps = psp.tile([1, 512], FD, tag="mm")
                nc.tensor.matmul(sq_ps[:], ones_f8[:, 0:1], sqc[:],
                                 start=True, stop=True)
                nc.scalar.copy(sq_row[:, sl], sq_ps[:])
            sq_rs = smp.tile([8, 512], FD, tag="rs8")
            nc.sync.dma_start(sq_rs[:], sq_row[:])
            sq64_ps = psp.tile([64, 512], FD, tag="mm")
            nc.tensor.matmul(sq64_ps[:], selmat[:], sq_rs[:],
                             start=True, stop=True)
            nc.scalar.copy(sq64[:], sq64_ps[:])

            # ---------------- acc_bias -> vis01 = (1 + accb) ----------
            S = smp.tile([1, 1], FD, tag="S")
            nc.vector.tensor_reduce(
                S[:], fch_row[:], axis=AX.X, op=ALU.add)
            Sp = smp.tile([1, 1], FD, tag="Sp")
            nc.vector.tensor_scalar(Sp[:], S[:], 1e-8, None, op0=ALU.add)
            rSp = smp.tile([1, 1], FD, tag="rSp")
            nc.vector.reciprocal(rSp[:], Sp[:])
            accb_row = rowp.tile([1, N], FD, tag="row")
            nc.vector.tensor_scalar(
                accb_row[:], fch_row[:], rSp[:, 0:1], None, op0=ALU.mult)
            ac_rs = smp.tile([8, 512], FD, tag="rs8")
            nc.sync.dma_start(ac_rs[:], accb_row[:])
            ac64_ps = psp.tile([64, 512], FD, tag="mm")
            nc.tensor.matmul(ac64_ps[:], selmat[:], ac_rs[:],
                             start=True, stop=True)
            # vis01 starts as (1 + acc_bias); visited nodes zeroed in place
            nc.vector.tensor_scalar(
                vis01[:], ac64_ps[:], 1.0, None, op0=ALU.add)

            # ---------------- source logits z ----------------
            z_row = rowp.tile([1, N], FD, tag="row")
            for c in range(NCHUNK):
                sl = slice(c * 512, (c + 1) * 512)
                h_ps = psp.tile([H, 512], FD, tag="mm")
                nc.tensor.matmul(h_ps[:], w1t[:], featsT[:, sl],
                                 start=True, stop=True)
                h_sb = smp.tile([H, 512], FD, tag="h_sb")
                nc.scalar.activation(
                    h_sb[:], h_ps[:], ACT.Relu, bias=b1[:, 0:1], scale=1.0)
                z_ps = psp.tile([1, 512], FD, tag="mm")
                nc.tensor.matmul(z_ps[:], w2t[:], h_sb[:],
                                 start=True, stop=True)
                nc.vector.scalar_tensor_tensor(
                    z_row[:, sl], z_ps[:], b2[:, 0:1], fch_row[:, sl],
                    op0=ALU.add, op1=ALU.mult)
            nc.sync.dma_start(d_z[:], z_row[:])

            # ---------------- top-8 sources ----------------
            zmax = smp.tile([1, 8], FD, tag="zmax")
            nc.vector.max(zmax[:], z_row[:])
            zidx = smp.tile([1, 8], U32, tag="zidx")
            nc.vector.max_index(zidx[:], zmax[:], z_row[:])
            zidx_f = smp.tile([1, 8], FD, tag="zidx_f")
            nc.vector.tensor_copy(zidx_f[:], zidx[:])
            nc.sync.dma_start(src_col[:], zidx_f[:])

            # ---------------- walk ----------------
            # layout: partition p = k*8 + s (walk k, chunk s); node
            # j = s*512 + f.  adj viewed as (4096*8, 512) row-chunks.
            # per-walk reductions go through a PE transpose to a (1, 64)
            # row, reduced in 8-groups along the free dim.
            adj_ck = d_adj.ap().rearrange("r (s c) -> (r s) c", c=512)

            def rep_walk_scalar(col8, tag):
                """(8,1) col -> (64,1) per-walk replication (repmat matmul)."""
                ps = psp.tile([64, 1], FD, tag="mm")
                nc.tensor.matmul(ps[:], repmat[:], col8[:],
                                 start=True, stop=True)
                sb = smp.tile([64, 1], FD, tag=tag)
                nc.scalar.copy(sb[:], ps[:])
                return sb

            def col_to_row(col, tag, n=8):
                """(n,1) -> (1,n) via PE transpose."""
                ps = psp.tile([1, n], FD, tag="mm")
                nc.tensor.transpose(ps[:], col[:, 0:1], ident[0:n, 0:n])
                sb = smp.tile([1, n], FD, tag=tag)
                nc.scalar.copy(sb[:], ps[:])
                return sb

            def row_to_col(row, tag):
                """(1,8) -> (8,1) via PE transpose."""
                ps = psp.tile([8, 1], FD, tag="mm")
                nc.tensor.transpose(ps[:], row[0:1, :], ident[0:1, 0:1])
                sb = smp.tile([8, 1], FD, tag=tag)
                nc.scalar.copy(sb[:], ps[:])
                return sb

            src_row = zidx_f        # (1, 8) source node ids as f32
            nc.vector.tensor_copy(path_f[:, 0:1], src_col[:])
            srcrep = rep_walk_scalar(src_col, "srcrep")
            srcadj = smp.tile([64, 1], FD, tag="srcadj")
            nc.vector.tensor_scalar(
                srcadj[:], srcrep[:], base512[:, 0:1], None,
                op0=ALU.subtract)
            nc.vector.scalar_tensor_tensor(
                vis01[:], iota512[:], srcadj[:, 0:1], vis01[:],
                op0=ALU.not_equal, op1=ALU.mult)

            cur_col = src_col
            xT = []
            with tc.tile_pool(name="wkp", bufs=2) as wkp:
                for pos in range(L):
                    cur_i = smp.tile([K, 1], I32, tag="cur_i")
                    nc.vector.tensor_copy(cur_i[:], cur_col[:])
                    frows = smp.tile([K, F], FD, tag="frows")
                    nc.gpsimd.indirect_dma_start(
                        out=frows[:], out_offset=None, in_=d_feats.ap(),
                        in_offset=bass.IndirectOffsetOnAxis(
                            ap=cur_i[:, 0:1], axis=0))
                    xt_ps = psp.tile([F, K], FD, tag="mm")
                    nc.tensor.transpose(xt_ps[:], frows[:], ident[0:K, 0:K])
                    xt = bigp.tile([F, K], FD, tag=f"xT{pos}")
                    nc.scalar.copy(xt[:], xt_ps[:])
                    xT.append(xt)
                    if pos == L - 1:
                        break
                    i = pos
                    # adj row-chunk gather: idx64 = cur*8 + s
                    currep = rep_walk_scalar(cur_col, "currep")
                    idx64f = smp.tile([64, 1], FD, tag="idx64f")
                    nc.vector.tensor_scalar(
                        idx64f[:], currep[:], 8.0, schunk[:, 0:1],
                        op0=ALU.mult, op1=ALU.add)
                    idx64 = smp.tile([64, 1], I32, tag="idx64")
                    nc.vector.tensor_copy(idx64[:], idx64f[:])
                    adjrows = wkp.tile([64, 512], FD, tag="adjrows")
                    nc.gpsimd.indirect_dma_start(
                        out=adjrows[:], out_offset=None, in_=adj_ck,
                        in_offset=bass.IndirectOffsetOnAxis(
                            ap=idx64[:, 0:1], axis=0))
                    eg_s = wkp.tile([64, 512], FD, tag="eg")
                    nc.sync.dma_start(eg_s[:], d_eg[i])
                    fr2 = smp.tile([K, F], FD, tag="fr2")
                    nc.scalar.square(fr2[:], frows[:])
                    sqcur = smp.tile([K, 1], FD, tag="sqcur")
                    nc.vector.tensor_reduce(
                        sqcur[:], fr2[:], axis=AX.X, op=ALU.add)
                    sqcur_r = rep_walk_scalar(sqcur, "sqcur_r")
                    # dots: 8 chunk matmuls -> (8, 4096) sbuf -> DMA
                    # reshape into the (64, 512) walk layout
                    dots8 = wkp.tile([K, N], FD, tag="dots8")
                    for hf in range(2):
                        dots_ps = psh.tile([K, NHALF], FD, tag="mmh")
                        for c in range(NCHUNK // 2):
                            sl_ps = slice(c * 512, (c + 1) * 512)
                            sl_f = slice(hf * NHALF + c * 512,
                                         hf * NHALF + (c + 1) * 512)
                            nc.tensor.matmul(
                                dots_ps[:, sl_ps], xT[pos][:],
                                featsT[:, sl_f], start=True, stop=True)
                        nc.scalar.copy(
                            dots8[:, hf * NHALF:(hf + 1) * NHALF],
                            dots_ps[:])
                    d2 = wkp.tile([64, 512], FD, tag="d2")
                    nc.sync.dma_start(
                        d2[:],
                        dots8[:].rearrange("k (s c) -> (k s) c", c=512))
                    nc.vector.scalar_tensor_tensor(
                        d2[:], d2[:], -2.0, sq64[:],
                        op0=ALU.mult, op1=ALU.add)
                    nc.vector.tensor_scalar(
                        d2[:], d2[:], sqcur_r[:, 0:1], 0.0,
                        op0=ALU.add, op1=ALU.max)
                    m2p = smp.tile([64, 1], FD, tag="m2p")
                    nc.vector.tensor_reduce(
                        m2p[:], d2[:], axis=AX.X, op=ALU.max)
                    m2row = col_to_row(m2p, "m2row", n=64)
                    m2w = smp.tile([1, 8], FD, tag="m2w")
                    nc.vector.tensor_reduce(
                        m2w[:], m2row[:].rearrange("p (a b) -> p a b", b=8),
                        axis=AX.X, op=ALU.max)
                    m2c = row_to_col(m2w, "m2c")
                    rm2 = smp.tile([K, 1], FD, tag="rm2")
                    nc.vector.reciprocal(rm2[:], m2c[:])
                    rm2_r = rep_walk_scalar(rm2, "rm2_r")
                    nc.scalar.activation(
                        d2[:], d2[:], ACT.Sqrt, bias=0.0,
                        scale=rm2_r[:, 0:1])
                    # d2 <- (guid + 1) * eg
                    nc.vector.scalar_tensor_tensor(
                        d2[:], d2[:], 1.0, eg_s[:],
                        op0=ALU.add, op1=ALU.mult)
                    # y (in adjrows): (adj * (vis*accb1)) * ((guid+1) * eg)
                    nc.vector.tensor_tensor(
                        adjrows[:], adjrows[:], vis01[:], op=ALU.mult)
                    nc.vector.tensor_tensor(
                        adjrows[:], adjrows[:], d2[:], op=ALU.mult)
                    # per-partition top-1 then global per-walk argmax with
                    # first-index tie-break via eq-mask + min-index
                    y8 = smp.tile([64, 8], FD, tag="y8")
                    nc.vector.max(y8[:], adjrows[:])
                    yidx = smp.tile([64, 8], U32, tag="yidx")
                    nc.vector.max_index(yidx[:], y8[:], adjrows[:])
                    jf = smp.tile([64, 1], FD, tag="jf")
                    nc.vector.tensor_copy(jf[:], yidx[:, 0:1])
                    nc.vector.tensor_scalar(
                        jf[:], jf[:], base512[:, 0:1], None, op0=ALU.add)
                    yvrow = col_to_row(y8[:, 0:1], "yvrow", n=64)
                    jfrow = col_to_row(jf, "jfrow", n=64)
                    maxv = smp.tile([1, 8], FD, tag="maxv")
                    nc.vector.tensor_reduce(
                        maxv[:],
                        yvrow[:].rearrange("p (a b) -> p a b", b=8),
                        axis=AX.X, op=ALU.max)
                    eqm = smp.tile([1, 64], FD, tag="eqm")
                    nc.vector.tensor_tensor(
                        eqm[:].rearrange("p (a b) -> p a b", b=8),
                        yvrow[:].rearrange("p (a b) -> p a b", b=8),
                        maxv[:].unsqueeze(2).to_broadcast([1, 8, 8]),
                        op=ALU.is_equal)
                    cand = smp.tile([1, 64], FD, tag="cand")
                    nc.vector.scalar_tensor_tensor(
                        cand[:], eqm[:], -65536.0, jfrow[:],
                        op0=ALU.mult, op1=ALU.add)
                    nc.vector.tensor_scalar(
                        cand[:], cand[:], 65536.0, None, op0=ALU.add)
                    samp_row = smp.tile([1, 8], FD, tag="samp_row")
                    nc.vector.tensor_reduce(
                        samp_row[:],
                        cand[:].rearrange("p (a b) -> p a b", b=8),
                        axis=AX.X, op=ALU.min)
                    # teleport select in row form
                    tel = smp.tile([1, 8], FD, tag="tel")
                    nc.vector.tensor_scalar(
                        tel[:], uni_t[i][:], float(np.float32(ALPHA)), None,
                        op0=ALU.is_lt)
                    dsrc = smp.tile([1, 8], FD, tag="dsrc")
                    nc.vector.tensor_tensor(
                        dsrc[:], src_row[:], samp_row[:], op=ALU.subtract)
                    td = smp.tile([1, 8], FD, tag="td")
                    nc.vector.tensor_tensor(
                        td[:], tel[:], dsrc[:], op=ALU.mult)
                    nxt_row = smp.tile([1, 8], FD, tag=f"nxtr{i}")
                    nc.vector.tensor_tensor(
                        nxt_row[:], td[:], samp_row[:], op=ALU.add)
                    nxt_col = row_to_col(nxt_row, f"nxtc{i}")
                    nc.vector.tensor_copy(path_f[:, i + 1:i + 2], nxt_col[:])
                    nxtrep = rep_walk_scalar(nxt_col, "nxtrep")
                    nxtadj = smp.tile([64, 1], FD, tag="nxtadj")
                    nc.vector.tensor_scalar(
                        nxtadj[:], nxtrep[:], base512[:, 0:1], None,
                        op0=ALU.subtract)
                    nc.vector.scalar_tensor_tensor(
                        vis01[:], iota512[:], nxtadj[:, 0:1], vis01[:],
                        op0=ALU.not_equal, op1=ALU.mult)
                    cur_col = nxt_col

            paths_i = smp.tile([K, L], I32, tag="paths_i")
            nc.vector.tensor_copy(paths_i[:], path_f[:])
            nc.sync.dma_start(d_paths[:], paths_i[:])

            # ---------------- GRU ----------------
            def gru_cell(h, xt, pre, hp):
                ps_i_rz = psp.tile([2 * H, K], FD, tag="mm")
                nc.tensor.matmul(ps_i_rz[:], gru[f"{pre}_ih_rzT"][:], xt[:],
                                 start=True, stop=True)
                ps_h_rz = psp.tile([2 * H, K], FD, tag="mm")
                nc.tensor.matmul(ps_h_rz[:], gru[f"{pre}_hh_rzT"][:], h[:],
                                 start=True, stop=True)
                gh_rz = smp.tile([2 * H, K], FD, tag="gh_rz")
                nc.vector.tensor_scalar(
                    gh_rz[:], ps_h_rz[:], gru[f"{pre}_bhh_rz"][:, 0:1], None,
                    op0=ALU.add)
                s_rz = smp.tile([2 * H, K], FD, tag="s_rz")
                nc.vector.scalar_tensor_tensor(
                    s_rz[:], ps_i_rz[:], gru[f"{pre}_bih_rz"][:, 0:1],
                    gh_rz[:], op0=ALU.add, op1=ALU.add)
                rz = smp.tile([2 * H, K], FD, tag="rz")
                nc.scalar.activation(rz[:], s_rz[:], ACT.Sigmoid)
                ps_i_n = psp.tile([H, K], FD, tag="mm")
                nc.tensor.matmul(ps_i_n[:], gru[f"{pre}_ih_nT"][:], xt[:],
                                 start=True, stop=True)
                ps_h_n = psp.tile([H, K], FD, tag="mm")
                nc.tensor.matmul(ps_h_n[:], gru[f"{pre}_hh_nT"][:], h[:],
                                 start=True, stop=True)
                inn = smp.tile([H, K], FD, tag="inn")
                nc.vector.tensor_scalar(
                    inn[:], ps_i_n[:], gru[f"{pre}_bih_n"][:, 0:1], None,
                    op0=ALU.add)
                hn = smp.tile([H, K], FD, tag="hn")
                nc.vector.tensor_scalar(
                    hn[:], ps_h_n[:], gru[f"{pre}_bhh_n"][:, 0:1], None,
                    op0=ALU.add)
                rh = smp.tile([H, K], FD, tag="rh")
                nc.vector.tensor_tensor(
                    rh[:], rz[0:H, :], hn[:], op=ALU.mult)
                npre = smp.tile([H, K], FD, tag="npre")
                nc.vector.tensor_tensor(npre[:], inn[:], rh[:], op=ALU.add)
                ng = smp.tile([H, K], FD, tag="ng")
                nc.scalar.activation(ng[:], npre[:], ACT.Tanh)
                zt = smp.tile([H, K], FD, tag="zt")
                nc.scalar.copy(zt[:], rz[H:2 * H, :])
                omz = smp.tile([H, K], FD, tag="omz")
                nc.vector.tensor_scalar(
                    omz[:], zt[:], -1.0, 1.0, op0=ALU.mult, op1=ALU.add)
                tb = smp.tile([H, K], FD, tag="tb")
                nc.vector.tensor_tensor(tb[:], omz[:], ng[:], op=ALU.mult)
                tcz = smp.tile([H, K], FD, tag="tc2")
                nc.vector.tensor_tensor(tcz[:], zt[:], h[:], op=ALU.mult)
                h2 = smp.tile([H, K], FD, tag=f"h_{pre}{hp}")
                nc.vector.tensor_tensor(h2[:], tb[:], tcz[:], op=ALU.add)
                return h2

            h_f = smp.tile([H, K], FD, tag="h_f0")
            nc.vector.memset(h_f[:], 0.0)
            for t in range(L):
                h_f = gru_cell(h_f, xT[t], "f", t + 1)
            h_b0 = smp.tile([H, K], FD, tag="h_b0")
            nc.vector.memset(h_b0[:], 0.0)
            h_b = gru_cell(h_b0, xT[L - 1], "b", 1)

            nc.vector.tensor_copy(pfT[0:H, :], h_f[:])
            nc.vector.tensor_copy(pfT[H:2 * H, :], h_b[:])
            nc.sync.dma_start(d_pfT[:], pfT[:])

            # ---------------- scorer ----------------
            ps1 = psp.tile([H, K], FD, tag="mm")
            nc.tensor.matmul(ps1[:], wps1t[:], pfT[:], start=True, stop=True)
            hs = smp.tile([H, K], FD, tag="hs")
            nc.scalar.activation(
                hs[:], ps1[:], ACT.Relu, bias=bps1[:, 0:1], scale=1.0)
            ps2 = psp.tile([1, K], FD, tag="mm")
            nc.tensor.matmul(ps2[:], wps2t[:], hs[:], start=True, stop=True)
            scores = smp.tile([1, K], FD, tag="scores")
            nc.scalar.activation(
                scores[:], ps2[:], ACT.Sigmoid, bias=bps2[:, 0:1], scale=1.0)
            nc.sync.dma_start(d_scores[:], scores[:])

    return nc


# --------------------------------------------------------------------------
# Host orchestration
# --------------------------------------------------------------------------

def _per_core_inputs(inputs, b, eg, uni):
    f32 = np.float32
    def c(x):
        return np.ascontiguousarray(np.asarray(x, dtype=f32))
    nf = c(inputs["node_features"][b])
    m = {
        "nf": nf,
        "feats": c(nf[T - 1]),
        "adj": c(inputs["adj_matrix"][b]),
        "w1t": c(inputs["W_sp1"].T),
        "b1": c(inputs["b_sp1"]).reshape(H, 1),
        "w2t": c(inputs["W_sp2"].T),
        "b2": c(inputs["b_sp2"]).reshape(1, 1),
        "wps1t": c(inputs["W_ps1"].T),
        "bps1": c(inputs["b_ps1"]).reshape(H, 1),
        "wps2t": c(inputs["W_ps2"].T),
        "bps2": c(inputs["b_ps2"]).reshape(1, 1),
        "eg": c(np.swapaxes(eg[b], 0, 1)).reshape(L - 1, 64, 512),
        "uni": c(np.swapaxes(uni[b], 0, 1))[:, None, :],  # (4, 1, K)
        "ident": np.eye(128, dtype=f32),
        "selmat": (np.arange(64)[None, :] % 8 == np.arange(8)[:, None]
                   ).astype(f32),
        "repmat": (np.arange(64)[None, :] // 8 == np.arange(8)[:, None]
                   ).astype(f32),
        "base512": ((np.arange(64) % 8) * 512).astype(f32).reshape(64, 1),
        "schunk": (np.arange(64) % 8).astype(f32).reshape(64, 1),
    }
    for pre, wih, whh, bih, bhh in (
        ("f", "Wih_f", "Whh_f", "bih_f", "bhh_f"),
        ("b", "Wih_b", "Whh_b", "bih_b", "bhh_b"),
    ):
        Wih, Whh = inputs[wih], inputs[whh]
        Bih, Bhh = inputs[bih], inputs[bhh]
        m[f"{pre}_ih_rzT"] = c(Wih[0:2 * H].T)
        m[f"{pre}_ih_nT"] = c(Wih[2 * H:3 * H].T)
        m[f"{pre}_hh_rzT"] = c(Whh[0:2 * H].T)
        m[f"{pre}_hh_nT"] = c(Whh[2 * H:3 * H].T)
        m[f"{pre}_bih_rz"] = c(Bih[0:2 * H]).reshape(2 * H, 1)
        m[f"{pre}_bih_n"] = c(Bih[2 * H:3 * H]).reshape(H, 1)
        m[f"{pre}_bhh_rz"] = c(Bhh[0:2 * H]).reshape(2 * H, 1)
        m[f"{pre}_bhh_n"] = c(Bhh[2 * H:3 * H]).reshape(H, 1)
    return m


_NC_CACHE = None
LAST_EXEC_NS = None


def _tunnel_ok(timeout=20.0):
    """Quick health probe of the axon relay before touching PJRT (a dead
    tunnel makes backend init hang indefinitely). Native-device setups
    (no relay env) skip the probe."""
    if not os.environ.get("TRN_TERMINAL_POOL_IPS"):
        return True
    import http.client
    try:
        conn = http.client.HTTPConnection("127.0.0.1", 8083, timeout=timeout)
        conn.request(
            "GET", "/init?rank=4294967295&topology=trn2.8x1&n_slices=1")
        resp = conn.getresponse()
        resp.read()
        conn.close()
        return resp.status == 200
    except Exception:
        return False


def _run_sim(nc, in_maps):
    """CoreSim fallback/debug path (KERNEL_BASS_SIM=1)."""
    from concourse import bass_interp
    outs = []
    for m in in_maps:
        sim = bass_interp.CoreSim(nc)
        for name, val in m.items():
            sim.tensor(name)[:] = val
        sim.simulate()
        outs.append({nm: np.array(sim.tensor(nm))
                     for nm in ("paths_o", "pfT_o", "scores_o")})
    return outs


def kernel(**inputs):
    global _NC_CACHE
    eg, uni = _host_rng()
    in_maps = [_per_core_inputs(inputs, b, eg, uni) for b in range(B)]
    global LAST_EXEC_NS
    use_sim = os.environ.get("KERNEL_BASS_SIM") == "1" or not _tunnel_ok()
    if not use_sim:
        try:
            if _NC_CACHE is None:
                _NC_CACHE = build_nc(num_devices=B)
            trace = os.environ.get("KERNEL_BASS_TRACE") == "1"
            res = run_bass_kernel_spmd(
                _NC_CACHE, in_maps, core_ids=list(range(B)), trace=trace)
            results = res.results
            if res.exec_time_ns is not None:
                LAST_EXEC_NS = res.exec_time_ns
        except Exception as e:
            import traceback
            traceback.print_exc()
            print(f"kernel: device path failed ({type(e).__name__}); "
                  "falling back to CoreSim", flush=True)
            use_sim = True
    if use_sim:
        nc1 = build_nc(num_devices=1)
        results = _run_sim(nc1, in_maps)
    paths = np.stack([results[b]["paths_o"] for b in range(B)])
    pf = np.stack([results[b]["pfT_o"].T for b in range(B)])
    scores = np.stack([results[b]["scores_o"][0] for b in range(B)])
    paths = paths.astype(np.int32)
    pf = pf.astype(np.float32)
    # weights = softmax over K of sigmoid scores
    sc = scores.astype(np.float32)[..., None]          # (B, K, 1)
    e = np.exp(sc - sc.max(axis=1, keepdims=True))
    weights = (e / e.sum(axis=1, keepdims=True)).astype(np.float32)
    return paths, weights, pf


# revision 27
# speedup vs baseline: 1.0282x; 1.0282x over previous
"""PathDiscovery Trainium2 Bass kernel.

Full pipeline on-device, data-parallel over batch (1 batch element per
NeuronCore, 8 cores):
  - fchange: mean |nf[t+1]-nf[t]| over (t, F)
  - source MLP -> z = logits * fchange -> top-8 (max8/max_index)
  - guided random walk with restart, 8 walks x 4 steps, vectorized over
    walks on partitions; the categorical sample is reproduced exactly as
    argmax_j probs_j * exp(gumbel_j) with host-precomputed (input
    independent) gumbel noise; adj/feature rows fetched by indirect DMA
  - bidirectional GRU encoding of gathered path features + path scorer

Host side: shard inputs over cores, precompute exp(gumbel)/uniform draws
from jax CPU threefry (deterministic, input-independent), pre-transpose
small weights, assemble outputs, softmax over K for weights.
"""

import os
import subprocess
import sys
import tempfile

import numpy as np

import concourse.bass as bass
import concourse.bacc as bacc
import concourse.mybir as mybir
from concourse import tile
from concourse.bass_utils import run_bass_kernel_spmd

B, T, N, F = 8, 12, 4096, 64
H = 64
K = 8
L = 5
ALPHA = 0.15
NT = N // 128          # 32 node tiles of 128
NCHUNK = N // 512      # 8 matmul chunks of 512
NHALF = N // 2
FD = mybir.dt.float32
I32 = mybir.dt.int32
U32 = mybir.dt.uint32
ALU = mybir.AluOpType
ACT = mybir.ActivationFunctionType
AX = mybir.AxisListType


# --------------------------------------------------------------------------
# Host-side RNG precompute (input independent; must match jax CPU threefry)
# --------------------------------------------------------------------------

_RNG_CACHE = None

_RNG_SCRIPT = r"""
import numpy as np
import jax, jax.numpy as jnp
B, K, L, N = 8, 8, 5, 4096
wkeys = jax.random.split(jax.random.key(42), B * K).reshape(B, K)
GUM = np.zeros((B, K, L - 1, N), np.float32)
UNI = np.zeros((B, K, L - 1), np.float32)
for b in range(B):
    for k in range(K):
        step_keys = jax.random.split(wkeys[b, k], L - 1)
        for i in range(L - 1):
            ku, kc = jax.random.split(step_keys[i])
            GUM[b, k, i] = np.asarray(jax.random.gumbel(kc, (N,), jnp.float32))
            UNI[b, k, i] = float(jax.random.uniform(ku))
np.savez(OUT_PATH, gum=GUM, uni=UNI)
"""


def _host_rng():
    """exp(gumbel) (B,K,4,N) f32 and uniforms (B,K,4) f32, via jax CPU."""
    global _RNG_CACHE
    if _RNG_CACHE is not None:
        return _RNG_CACHE
    # jax in this process may be bound to the axon backend; compute in a
    # clean subprocess pinned to the CPU backend so the threefry bits match
    # the reference implementation exactly.
    import jax  # just to locate the installed jax for the subprocess

    site_dir = os.path.dirname(os.path.dirname(jax.__file__))
    env = dict(os.environ)
    env.pop("TRN_TERMINAL_POOL_IPS", None)  # disables axon sitecustomize boot
    env["JAX_PLATFORMS"] = "cpu"
    env["PYTHONPATH"] = site_dir + os.pathsep + env.get("PYTHONPATH", "")
    with tempfile.TemporaryDirectory() as td:
        out_path = os.path.join(td, "rng.npz")
        script = f"OUT_PATH = {out_path!r}\n" + _RNG_SCRIPT
        subprocess.run(
            [sys.executable, "-c", script], env=env, check=True,
            capture_output=True,
        )
        dat = np.load(out_path)
        gum, uni = dat["gum"], dat["uni"]
    eg = np.exp(gum.astype(np.float64)).astype(np.float32)
    _RNG_CACHE = (eg, uni)
    return _RNG_CACHE


# --------------------------------------------------------------------------
# Bass kernel builder
# --------------------------------------------------------------------------

def build_nc(num_devices=1, debug=False):
    nc = bacc.Bacc(
        "TRN2",
        target_bir_lowering=False,
        debug=debug,
        num_devices=num_devices,
    )

    d_nf = nc.dram_tensor("nf", [T, N, F], FD, kind="ExternalInput")
    d_feats = nc.dram_tensor("feats", [N, F], FD, kind="ExternalInput")
    d_adj = nc.dram_tensor("adj", [N, N], FD, kind="ExternalInput")
    d_w1t = nc.dram_tensor("w1t", [F, H], FD, kind="ExternalInput")
    d_b1 = nc.dram_tensor("b1", [H, 1], FD, kind="ExternalInput")
    d_w2t = nc.dram_tensor("w2t", [H, 1], FD, kind="ExternalInput")
    d_b2 = nc.dram_tensor("b2", [1, 1], FD, kind="ExternalInput")
    gru_dram = {}
    for pre in ("f", "b"):
        for nm, shape in (
            ("ih_rzT", [F, 2 * H]), ("ih_nT", [F, H]),
            ("hh_rzT", [H, 2 * H]), ("hh_nT", [H, H]),
            ("bih_rz", [2 * H, 1]), ("bih_n", [H, 1]),
            ("bhh_rz", [2 * H, 1]), ("bhh_n", [H, 1]),
        ):
            gru_dram[f"{pre}_{nm}"] = nc.dram_tensor(
                f"{pre}_{nm}", shape, FD, kind="ExternalInput")
    d_wps1t = nc.dram_tensor("wps1t", [2 * H, H], FD, kind="ExternalInput")
    d_bps1 = nc.dram_tensor("bps1", [H, 1], FD, kind="ExternalInput")
    d_wps2t = nc.dram_tensor("wps2t", [H, 1], FD, kind="ExternalInput")
    d_bps2 = nc.dram_tensor("bps2", [1, 1], FD, kind="ExternalInput")
    d_eg = nc.dram_tensor("eg", [L - 1, 64, 512], FD, kind="ExternalInput")
    d_uni = nc.dram_tensor("uni", [L - 1, 1, K], FD, kind="ExternalInput")
    d_ident = nc.dram_tensor("ident", [128, 128], FD, kind="ExternalInput")
    d_selmat = nc.dram_tensor("selmat", [8, 64], FD, kind="ExternalInput")
    d_repmat = nc.dram_tensor("repmat", [8, 64], FD, kind="ExternalInput")
    d_base512 = nc.dram_tensor("base512", [64, 1], FD, kind="ExternalInput")
    d_schunk = nc.dram_tensor("schunk", [64, 1], FD, kind="ExternalInput")

    d_paths = nc.dram_tensor("paths_o", [K, L], I32, kind="ExternalOutput")
    d_pfT = nc.dram_tensor("pfT_o", [2 * H, K], FD, kind="ExternalOutput")
    d_scores = nc.dram_tensor("scores_o", [1, K], FD, kind="ExternalOutput")
    d_fch = nc.dram_tensor("fch_o", [1, N], FD, kind="ExternalOutput")
    d_z = nc.dram_tensor("z_o", [1, N], FD, kind="ExternalOutput")

    with tile.TileContext(nc) as tc:
        with (
            tc.tile_pool(name="const", bufs=1) as constp,
            tc.tile_pool(name="big", bufs=1) as bigp,
            tc.tile_pool(name="row", bufs=2) as rowp,
            tc.tile_pool(name="small", bufs=2) as smp,
            tc.tile_pool(name="ps", bufs=3, space="PSUM") as psp,
            tc.tile_pool(name="psh", bufs=1, space="PSUM") as psh,
        ):
            # ---------------- constants ----------------
            ident = constp.tile([128, 128], FD, tag="ident")
            nc.sync.dma_start(ident[:], d_ident[:])
            w1t = constp.tile([F, H], FD, tag="w1t")
            nc.sync.dma_start(w1t[:], d_w1t[:])
            b1 = constp.tile([H, 1], FD, tag="b1")
            nc.sync.dma_start(b1[:], d_b1[:])
            w2t = constp.tile([H, 1], FD, tag="w2t")
            nc.sync.dma_start(w2t[:], d_w2t[:])
            b2 = constp.tile([1, 1], FD, tag="b2")
            nc.sync.dma_start(b2[:], d_b2[:])
            gru = {}
            for nm, dtr in gru_dram.items():
                t_ = constp.tile(list(dtr.shape), FD, tag=nm)
                nc.sync.dma_start(t_[:], dtr[:])
                gru[nm] = t_
            wps1t = constp.tile([2 * H, H], FD, tag="wps1t")
            nc.sync.dma_start(wps1t[:], d_wps1t[:])
            bps1 = constp.tile([H, 1], FD, tag="bps1")
            nc.sync.dma_start(bps1[:], d_bps1[:])
            wps2t = constp.tile([H, 1], FD, tag="wps2t")
            nc.sync.dma_start(wps2t[:], d_wps2t[:])
            bps2 = constp.tile([1, 1], FD, tag="bps2")
            nc.sync.dma_start(bps2[:], d_bps2[:])

            ones_f8 = constp.tile([F, K], FD, tag="ones_f8")
            nc.vector.memset(ones_f8[:], 1.0)
            selmat = constp.tile([8, 64], FD, tag="selmat")
            nc.sync.dma_start(selmat[:], d_selmat[:])
            repmat = constp.tile([8, 64], FD, tag="repmat")
            nc.sync.dma_start(repmat[:], d_repmat[:])
            base512 = constp.tile([64, 1], FD, tag="base512")
            nc.sync.dma_start(base512[:], d_base512[:])
            schunk = constp.tile([64, 1], FD, tag="schunk")
            nc.sync.dma_start(schunk[:], d_schunk[:])

            iota512 = constp.tile([64, 512], FD, tag="iota512")
            nc.gpsimd.iota(
                iota512[:], pattern=[[1, 512]], base=0, channel_multiplier=0,
                allow_small_or_imprecise_dtypes=True,
            )

            uni_t = []
            for s in range(L - 1):
                t_ = constp.tile([1, K], FD, tag=f"uni{s}")
                nc.sync.dma_start(t_[:], d_uni[s])
                uni_t.append(t_)

            # persistent big tiles
            featsT = bigp.tile([F, N], FD, tag="featsT")
            sq64 = bigp.tile([64, 512], FD, tag="sq64")
            vis01 = bigp.tile([64, 512], FD, tag="vis01")
            src_col = bigp.tile([K, 1], FD, tag="src_col")
            path_f = bigp.tile([K, L], FD, tag="path_f")
            pfT = bigp.tile([2 * H, K], FD, tag="pfT")

            fch_row = rowp.tile([1, N], FD, tag="row")

            # ---------------- phase 1: fchange + featsT (scoped pools) ----
            with (
                tc.tile_pool(name="nfs", bufs=4) as nfp,
                tc.tile_pool(name="dts", bufs=3) as dtp,
            ):
                acc = nfp.tile([128, NT], FD, tag="acc")
                nc.vector.memset(acc[:], 0.0)
                nf_view = d_nf.ap().rearrange("t (a p) f -> t p a f", p=128)
                prev = nfp.tile([128, NT * F], FD, tag="nft")
                nc.gpsimd.dma_start(prev[:], nf_view[0])
                for t in range(1, T):
                    curt = nfp.tile([128, NT * F], FD, tag="nft")
                    nc.gpsimd.dma_start(curt[:], nf_view[t])
                    dtile = dtp.tile([128, NT * F], FD, tag="dtile")
                    nc.vector.tensor_tensor(
                        dtile[:], curt[:], prev[:], op=ALU.subtract)
                    r = smp.tile([128, NT], FD, tag="red")
                    nc.vector.tensor_reduce(
                        r[:], dtile[:].rearrange("p (a f) -> p a f", f=F),
                        axis=AX.X, op=ALU.add, apply_absolute_value=True,
                    )
                    nc.vector.tensor_tensor(acc[:], acc[:], r[:], op=ALU.add)
                    prev = curt
                # prev holds nf[T-1] = feats in (p, a, f) layout
                # fchange = acc / 704 -> transpose -> row
                fchm = smp.tile([128, NT], FD, tag="fchm")
                nc.vector.tensor_scalar(
                    fchm[:], acc[:],
                    float(np.float32(1.0) / np.float32((T - 1) * F)), None,
                    op0=ALU.mult)
                fch_ps = psp.tile([NT, 128], FD, tag="mm")
                nc.tensor.transpose(fch_ps[:], fchm[:], ident[:])
                fch_t = smp.tile([NT, 128], FD, tag="fch_t")
                nc.scalar.copy(fch_t[:], fch_ps[:])
                nc.sync.dma_start(fch_row[:], fch_t[:])
                nc.sync.dma_start(d_fch[:], fch_row[:])

                # featsT from transposed 128x64 tiles of feats
                for a in range(NT):
                    tp = psp.tile([F, 128], FD, tag="mm")
                    nc.tensor.transpose(
                        tp[:], prev[:, a * F:(a + 1) * F], ident[:])
                    nc.scalar.copy(featsT[:, a * 128:(a + 1) * 128], tp[:])

            # ---------------- sq -> sq64 (64, 512) ----------------
            sq_row = rowp.tile([1, N], FD, tag="row")
            for c in range(NCHUNK):
                sl = slice(c * 512, (c + 1) * 512)
                sqc = smp.tile([F, 512], FD, tag="sqc")
                nc.scalar.square(sqc[:], featsT[:, sl])
                sq_ps = psp.tile([1, 512], FD, tag="mm")
                nc.tensor.matmul(sq_ps[:], ones_f8[:, 0:1], sqc[:],
                                 start=True, stop=True)
                nc.scalar.copy(sq_row[:, sl], sq_ps[:])
            sq_rs = smp.tile([8, 512], FD, tag="rs8")
            nc.sync.dma_start(sq_rs[:], sq_row[:])
            sq64_ps = psp.tile([64, 512], FD, tag="mm")
            nc.tensor.matmul(sq64_ps[:], selmat[:], sq_rs[:],
                             start=True, stop=True)
            nc.scalar.copy(sq64[:], sq64_ps[:])

            # ---------------- acc_bias -> vis01 = (1 + accb) ----------
            S = smp.tile([1, 1], FD, tag="S")
            nc.vector.tensor_reduce(
                S[:], fch_row[:], axis=AX.X, op=ALU.add)
            Sp = smp.tile([1, 1], FD, tag="Sp")
            nc.vector.tensor_scalar(Sp[:], S[:], 1e-8, None, op0=ALU.add)
            rSp = smp.tile([1, 1], FD, tag="rSp")
            nc.vector.reciprocal(rSp[:], Sp[:])
            accb_row = rowp.tile([1, N], FD, tag="row")
            nc.vector.tensor_scalar(
                accb_row[:], fch_row[:], rSp[:, 0:1], None, op0=ALU.mult)
            ac_rs = smp.tile([8, 512], FD, tag="rs8")
            nc.sync.dma_start(ac_rs[:], accb_row[:])
            ac64_ps = psp.tile([64, 512], FD, tag="mm")
            nc.tensor.matmul(ac64_ps[:], selmat[:], ac_rs[:],
                             start=True, stop=True)
            # vis01 starts as (1 + acc_bias); visited nodes zeroed in place
            nc.vector.tensor_scalar(
                vis01[:], ac64_ps[:], 1.0, None, op0=ALU.add)

            # ---------------- source logits z ----------------
            z_row = rowp.tile([1, N], FD, tag="row")
            for c in range(NCHUNK):
                sl = slice(c * 512, (c + 1) * 512)
                h_ps = psp.tile([H, 512], FD, tag="mm")
                nc.tensor.matmul(h_ps[:], w1t[:], featsT[:, sl],
                                 start=True, stop=True)
                h_sb = smp.tile([H, 512], FD, tag="h_sb")
                nc.scalar.activation(
                    h_sb[:], h_ps[:], ACT.Relu, bias=b1[:, 0:1], scale=1.0)
                z_ps = psp.tile([1, 512], FD, tag="mm")
                nc.tensor.matmul(z_ps[:], w2t[:], h_sb[:],
                                 start=True, stop=True)
                nc.vector.scalar_tensor_tensor(
                    z_row[:, sl], z_ps[:], b2[:, 0:1], fch_row[:, sl],
                    op0=ALU.add, op1=ALU.mult)
            nc.sync.dma_start(d_z[:], z_row[:])

            # ---------------- top-8 sources ----------------
            zmax = smp.tile([1, 8], FD, tag="zmax")
            nc.vector.max(zmax[:], z_row[:])
            zidx = smp.tile([1, 8], U32, tag="zidx")
            nc.vector.max_index(zidx[:], zmax[:], z_row[:])
            zidx_f = smp.tile([1, 8], FD, tag="zidx_f")
            nc.vector.tensor_copy(zidx_f[:], zidx[:])
            nc.sync.dma_start(src_col[:], zidx_f[:])

            # ---------------- walk ----------------
            # layout: partition p = k*8 + s (walk k, chunk s); node
            # j = s*512 + f.  adj viewed as (4096*8, 512) row-chunks.
            # per-walk reductions go through a PE transpose to a (1, 64)
            # row, reduced in 8-groups along the free dim.
            adj_ck = d_adj.ap().rearrange("r (s c) -> (r s) c", c=512)

            def rep_walk_scalar(col8, tag):
                """(8,1) col -> (64,1) per-walk replication (repmat matmul)."""
                ps = psp.tile([64, 1], FD, tag="mm")
                nc.tensor.matmul(ps[:], repmat[:], col8[:],
                                 start=True, stop=True)
                sb = smp.tile([64, 1], FD, tag=tag)
                nc.scalar.copy(sb[:], ps[:])
                return sb

            def col_to_row(col, tag, n=8):
                """(n,1) -> (1,n) via PE transpose."""
                ps = psp.tile([1, n], FD, tag="mm")
                nc.tensor.transpose(ps[:], col[:, 0:1], ident[0:n, 0:n])
                sb = smp.tile([1, n], FD, tag=tag)
                nc.scalar.copy(sb[:], ps[:])
                return sb

            def row_to_col(row, tag):
                """(1,8) -> (8,1) via PE transpose."""
                ps = psp.tile([8, 1], FD, tag="mm")
                nc.tensor.transpose(ps[:], row[0:1, :], ident[0:1, 0:1])
                sb = smp.tile([8, 1], FD, tag=tag)
                nc.scalar.copy(sb[:], ps[:])
                return sb

            src_row = zidx_f        # (1, 8) source node ids as f32
            nc.vector.tensor_copy(path_f[:, 0:1], src_col[:])
            srcrep = rep_walk_scalar(src_col, "srcrep")
            srcadj = smp.tile([64, 1], FD, tag="srcadj")
            nc.vector.tensor_scalar(
                srcadj[:], srcrep[:], base512[:, 0:1], None,
                op0=ALU.subtract)
            nc.vector.scalar_tensor_tensor(
                vis01[:], iota512[:], srcadj[:, 0:1], vis01[:],
                op0=ALU.not_equal, op1=ALU.mult)

            cur_col = src_col
            xT = []
            with tc.tile_pool(name="wkp", bufs=2) as wkp:
                for pos in range(L):
                    cur_i = smp.tile([K, 1], I32, tag="cur_i")
                    nc.vector.tensor_copy(cur_i[:], cur_col[:])
                    frows = smp.tile([K, F], FD, tag="frows")
                    nc.gpsimd.indirect_dma_start(
                        out=frows[:], out_offset=None, in_=d_feats.ap(),
                        in_offset=bass.IndirectOffsetOnAxis(
                            ap=cur_i[:, 0:1], axis=0))
                    xt_ps = psp.tile([F, K], FD, tag="mm")
                    nc.tensor.transpose(xt_ps[:], frows[:], ident[0:K, 0:K])
                    xt = bigp.tile([F, K], FD, tag=f"xT{pos}")
                    nc.scalar.copy(xt[:], xt_ps[:])
                    xT.append(xt)
                    if pos == L - 1:
                        break
                    i = pos
                    # adj row-chunk gather: idx64 = cur*8 + s
                    currep = rep_walk_scalar(cur_col, "currep")
                    idx64f = smp.tile([64, 1], FD, tag="idx64f")
                    nc.vector.tensor_scalar(
                        idx64f[:], currep[:], 8.0, schunk[:, 0:1],
                        op0=ALU.mult, op1=ALU.add)
                    idx64 = smp.tile([64, 1], I32, tag="idx64")
                    nc.vector.tensor_copy(idx64[:], idx64f[:])
                    adjrows = wkp.tile([64, 512], FD, tag="adjrows")
                    nc.gpsimd.indirect_dma_start(
                        out=adjrows[:], out_offset=None, in_=adj_ck,
                        in_offset=bass.IndirectOffsetOnAxis(
                            ap=idx64[:, 0:1], axis=0))
                    eg_s = wkp.tile([64, 512], FD, tag="eg")
                    nc.sync.dma_start(eg_s[:], d_eg[i])
                    fr2 = smp.tile([K, F], FD, tag="fr2")
                    nc.scalar.square(fr2[:], frows[:])
                    sqcur = smp.tile([K, 1], FD, tag="sqcur")
                    nc.vector.tensor_reduce(
                        sqcur[:], fr2[:], axis=AX.X, op=ALU.add)
                    sqcur_r = rep_walk_scalar(sqcur, "sqcur_r")
                    # dots: 8 chunk matmuls -> (8, 4096) sbuf -> DMA
                    # reshape into the (64, 512) walk layout
                    dots8 = wkp.tile([K, N], FD, tag="dots8")
                    for hf in range(2):
                        dots_ps = psh.tile([K, NHALF], FD, tag="mmh")
                        for c in range(NCHUNK // 2):
                            sl_ps = slice(c * 512, (c + 1) * 512)
                            sl_f = slice(hf * NHALF + c * 512,
                                         hf * NHALF + (c + 1) * 512)
                            nc.tensor.matmul(
                                dots_ps[:, sl_ps], xT[pos][:],
                                featsT[:, sl_f], start=True, stop=True)
                        nc.scalar.copy(
                            dots8[:, hf * NHALF:(hf + 1) * NHALF],
                            dots_ps[:])
                    d2 = wkp.tile([64, 512], FD, tag="d2")
                    nc.sync.dma_start(
                        d2[:],
                        dots8[:].rearrange("k (s c) -> (k s) c", c=512))
                    nc.vector.scalar_tensor_tensor(
                        d2[:], d2[:], -2.0, sq64[:],
                        op0=ALU.mult, op1=ALU.add)
                    nc.vector.tensor_scalar(
                        d2[:], d2[:], sqcur_r[:, 0:1], 0.0,
                        op0=ALU.add, op1=ALU.max)
                    m2p = smp.tile([64, 1], FD, tag="m2p")
                    nc.vector.tensor_reduce(
                        m2p[:], d2[:], axis=AX.X, op=ALU.max)
                    m2row = col_to_row(m2p, "m2row", n=64)
                    m2w = smp.tile([1, 8], FD, tag="m2w")
                    nc.vector.tensor_reduce(
                        m2w[:], m2row[:].rearrange("p (a b) -> p a b", b=8),
                        axis=AX.X, op=ALU.max)
                    m2c = row_to_col(m2w, "m2c")
                    rm2 = smp.tile([K, 1], FD, tag="rm2")
                    nc.vector.reciprocal(rm2[:], m2c[:])
                    rm2_r = rep_walk_scalar(rm2, "rm2_r")
                    nc.scalar.activation(
                        d2[:], d2[:], ACT.Sqrt, bias=0.0,
                        scale=rm2_r[:, 0:1])
                    # d2 <- (guid + 1) * eg
                    nc.vector.scalar_tensor_tensor(
                        d2[:], d2[:], 1.0, eg_s[:],
                        op0=ALU.add, op1=ALU.mult)
                    # y (in adjrows): (adj * (vis*accb1)) * ((guid+1) * eg)
                    nc.vector.tensor_tensor(
                        adjrows[:], adjrows[:], vis01[:], op=ALU.mult)
                    nc.vector.tensor_tensor(
                        adjrows[:], adjrows[:], d2[:], op=ALU.mult)
                    # per-partition top-1 then global per-walk argmax with
                    # first-index tie-break via eq-mask + min-index
                    y8 = smp.tile([64, 8], FD, tag="y8")
                    nc.vector.max(y8[:], adjrows[:])
                    yidx = smp.tile([64, 8], U32, tag="yidx")
                    nc.vector.max_index(yidx[:], y8[:], adjrows[:])
                    jf = smp.tile([64, 1], FD, tag="jf")
                    nc.vector.tensor_copy(jf[:], yidx[:, 0:1])
                    nc.vector.tensor_scalar(
                        jf[:], jf[:], base512[:, 0:1], None, op0=ALU.add)
                    yvrow = col_to_row(y8[:, 0:1], "yvrow", n=64)
                    jfrow = col_to_row(jf, "jfrow", n=64)
                    maxv = smp.tile([1, 8], FD, tag="maxv")
                    nc.vector.tensor_reduce(
                        maxv[:],
                        yvrow[:].rearrange("p (a b) -> p a b", b=8),
                        axis=AX.X, op=ALU.max)
                    eqm = smp.tile([1, 64], FD, tag="eqm")
                    nc.vector.tensor_tensor(
                        eqm[:].rearrange("p (a b) -> p a b", b=8),
                        yvrow[:].rearrange("p (a b) -> p a b", b=8),
                        maxv[:].unsqueeze(2).to_broadcast([1, 8, 8]),
                        op=ALU.is_equal)
                    cand = smp.tile([1, 64], FD, tag="cand")
                    nc.vector.scalar_tensor_tensor(
                        cand[:], eqm[:], -65536.0, jfrow[:],
                        op0=ALU.mult, op1=ALU.add)
                    nc.vector.tensor_scalar(
                        cand[:], cand[:], 65536.0, None, op0=ALU.add)
                    samp_row = smp.tile([1, 8], FD, tag="samp_row")
                    nc.vector.tensor_reduce(
                        samp_row[:],
                        cand[:].rearrange("p (a b) -> p a b", b=8),
                        axis=AX.X, op=ALU.min)
                    # teleport select in row form
                    tel = smp.tile([1, 8], FD, tag="tel")
                    nc.vector.tensor_scalar(
                        tel[:], uni_t[i][:], float(np.float32(ALPHA)), None,
                        op0=ALU.is_lt)
                    dsrc = smp.tile([1, 8], FD, tag="dsrc")
                    nc.vector.tensor_tensor(
                        dsrc[:], src_row[:], samp_row[:], op=ALU.subtract)
                    td = smp.tile([1, 8], FD, tag="td")
                    nc.vector.tensor_tensor(
                        td[:], tel[:], dsrc[:], op=ALU.mult)
                    nxt_row = smp.tile([1, 8], FD, tag=f"nxtr{i}")
                    nc.vector.tensor_tensor(
                        nxt_row[:], td[:], samp_row[:], op=ALU.add)
                    nxt_col = row_to_col(nxt_row, f"nxtc{i}")
                    nc.vector.tensor_copy(path_f[:, i + 1:i + 2], nxt_col[:])
                    nxtrep = rep_walk_scalar(nxt_col, "nxtrep")
                    nxtadj = smp.tile([64, 1], FD, tag="nxtadj")
                    nc.vector.tensor_scalar(
                        nxtadj[:], nxtrep[:], base512[:, 0:1], None,
                        op0=ALU.subtract)
                    nc.vector.scalar_tensor_tensor(
                        vis01[:], iota512[:], nxtadj[:, 0:1], vis01[:],
                        op0=ALU.not_equal, op1=ALU.mult)
                    cur_col = nxt_col

            paths_i = smp.tile([K, L], I32, tag="paths_i")
            nc.vector.tensor_copy(paths_i[:], path_f[:])
            nc.sync.dma_start(d_paths[:], paths_i[:])

            # ---------------- GRU ----------------
            def gru_cell(h, xt, pre, hp):
                ps_i_rz = psp.tile([2 * H, K], FD, tag="mm")
                nc.tensor.matmul(ps_i_rz[:], gru[f"{pre}_ih_rzT"][:], xt[:],
                                 start=True, stop=True)
                ps_h_rz = psp.tile([2 * H, K], FD, tag="mm")
                nc.tensor.matmul(ps_h_rz[:], gru[f"{pre}_hh_rzT"][:], h[:],
                                 start=True, stop=True)
                gh_rz = smp.tile([2 * H, K], FD, tag="gh_rz")
                nc.vector.tensor_scalar(
                    gh_rz[:], ps_h_rz[:], gru[f"{pre}_bhh_rz"][:, 0:1], None,
                    op0=ALU.add)
                s_rz = smp.tile([2 * H, K], FD, tag="s_rz")
                nc.vector.scalar_tensor_tensor(
                    s_rz[:], ps_i_rz[:], gru[f"{pre}_bih_rz"][:, 0:1],
                    gh_rz[:], op0=ALU.add, op1=ALU.add)
                rz = smp.tile([2 * H, K], FD, tag="rz")
                nc.scalar.activation(rz[:], s_rz[:], ACT.Sigmoid)
                ps_i_n = psp.tile([H, K], FD, tag="mm")
                nc.tensor.matmul(ps_i_n[:], gru[f"{pre}_ih_nT"][:], xt[:],
                                 start=True, stop=True)
                ps_h_n = psp.tile([H, K], FD, tag="mm")
                nc.tensor.matmul(ps_h_n[:], gru[f"{pre}_hh_nT"][:], h[:],
                                 start=True, stop=True)
                inn = smp.tile([H, K], FD, tag="inn")
                nc.vector.tensor_scalar(
                    inn[:], ps_i_n[:], gru[f"{pre}_bih_n"][:, 0:1], None,
                    op0=ALU.add)
                hn = smp.tile([H, K], FD, tag="hn")
                nc.vector.tensor_scalar(
                    hn[:], ps_h_n[:], gru[f"{pre}_bhh_n"][:, 0:1], None,
                    op0=ALU.add)
                rh = smp.tile([H, K], FD, tag="rh")
                nc.vector.tensor_tensor(
                    rh[:], rz[0:H, :], hn[:], op=ALU.mult)
                npre = smp.tile([H, K], FD, tag="npre")
                nc.vector.tensor_tensor(npre[:], inn[:], rh[:], op=ALU.add)
                ng = smp.tile([H, K], FD, tag="ng")
                nc.scalar.activation(ng[:], npre[:], ACT.Tanh)
                zt = smp.tile([H, K], FD, tag="zt")
                nc.scalar.copy(zt[:], rz[H:2 * H, :])
                omz = smp.tile([H, K], FD, tag="omz")
                nc.vector.tensor_scalar(
                    omz[:], zt[:], -1.0, 1.0, op0=ALU.mult, op1=ALU.add)
                tb = smp.tile([H, K], FD, tag="tb")
                nc.vector.tensor_tensor(tb[:], omz[:], ng[:], op=ALU.mult)
                tcz = smp.tile([H, K], FD, tag="tc2")
                nc.vector.tensor_tensor(tcz[:], zt[:], h[:], op=ALU.mult)
                h2 = smp.tile([H, K], FD, tag=f"h_{pre}{hp}")
                nc.vector.tensor_tensor(h2[:], tb[:], tcz[:], op=ALU.add)
                return h2

            h_f = smp.tile([H, K], FD, tag="h_f0")
            nc.vector.memset(h_f[:], 0.0)
            for t in range(L):
                h_f = gru_cell(h_f, xT[t], "f", t + 1)
            h_b0 = smp.tile([H, K], FD, tag="h_b0")
            nc.vector.memset(h_b0[:], 0.0)
            h_b = gru_cell(h_b0, xT[L - 1], "b", 1)

            nc.vector.tensor_copy(pfT[0:H, :], h_f[:])
            nc.vector.tensor_copy(pfT[H:2 * H, :], h_b[:])
            nc.sync.dma_start(d_pfT[:], pfT[:])

            # ---------------- scorer ----------------
            ps1 = psp.tile([H, K], FD, tag="mm")
            nc.tensor.matmul(ps1[:], wps1t[:], pfT[:], start=True, stop=True)
            hs = smp.tile([H, K], FD, tag="hs")
            nc.scalar.activation(
                hs[:], ps1[:], ACT.Relu, bias=bps1[:, 0:1], scale=1.0)
            ps2 = psp.tile([1, K], FD, tag="mm")
            nc.tensor.matmul(ps2[:], wps2t[:], hs[:], start=True, stop=True)
            scores = smp.tile([1, K], FD, tag="scores")
            nc.scalar.activation(
                scores[:], ps2[:], ACT.Sigmoid, bias=bps2[:, 0:1], scale=1.0)
            nc.sync.dma_start(d_scores[:], scores[:])

    return nc


# --------------------------------------------------------------------------
# Host orchestration
# --------------------------------------------------------------------------

def _per_core_inputs(inputs, b, eg, uni):
    f32 = np.float32
    def c(x):
        return np.ascontiguousarray(np.asarray(x, dtype=f32))
    nf = c(inputs["node_features"][b])
    m = {
        "nf": nf,
        "feats": c(nf[T - 1]),
        "adj": c(inputs["adj_matrix"][b]),
        "w1t": c(inputs["W_sp1"].T),
        "b1": c(inputs["b_sp1"]).reshape(H, 1),
        "w2t": c(inputs["W_sp2"].T),
        "b2": c(inputs["b_sp2"]).reshape(1, 1),
        "wps1t": c(inputs["W_ps1"].T),
        "bps1": c(inputs["b_ps1"]).reshape(H, 1),
        "wps2t": c(inputs["W_ps2"].T),
        "bps2": c(inputs["b_ps2"]).reshape(1, 1),
        "eg": c(np.swapaxes(eg[b], 0, 1)).reshape(L - 1, 64, 512),
        "uni": c(np.swapaxes(uni[b], 0, 1))[:, None, :],  # (4, 1, K)
        "ident": np.eye(128, dtype=f32),
        "selmat": (np.arange(64)[None, :] % 8 == np.arange(8)[:, None]
                   ).astype(f32),
        "repmat": (np.arange(64)[None, :] // 8 == np.arange(8)[:, None]
                   ).astype(f32),
        "base512": ((np.arange(64) % 8) * 512).astype(f32).reshape(64, 1),
        "schunk": (np.arange(64) % 8).astype(f32).reshape(64, 1),
    }
    for pre, wih, whh, bih, bhh in (
        ("f", "Wih_f", "Whh_f", "bih_f", "bhh_f"),
        ("b", "Wih_b", "Whh_b", "bih_b", "bhh_b"),
    ):
        Wih, Whh = inputs[wih], inputs[whh]
        Bih, Bhh = inputs[bih], inputs[bhh]
        m[f"{pre}_ih_rzT"] = c(Wih[0:2 * H].T)
        m[f"{pre}_ih_nT"] = c(Wih[2 * H:3 * H].T)
        m[f"{pre}_hh_rzT"] = c(Whh[0:2 * H].T)
        m[f"{pre}_hh_nT"] = c(Whh[2 * H:3 * H].T)
        m[f"{pre}_bih_rz"] = c(Bih[0:2 * H]).reshape(2 * H, 1)
        m[f"{pre}_bih_n"] = c(Bih[2 * H:3 * H]).reshape(H, 1)
        m[f"{pre}_bhh_rz"] = c(Bhh[0:2 * H]).reshape(2 * H, 1)
        m[f"{pre}_bhh_n"] = c(Bhh[2 * H:3 * H]).reshape(H, 1)
    return m


_NC_CACHE = None
LAST_EXEC_NS = None


def _tunnel_ok(timeout=20.0):
    """Quick health probe of the axon relay before touching PJRT (a dead
    tunnel makes backend init hang indefinitely). Native-device setups
    (no relay env) skip the probe."""
    if not os.environ.get("TRN_TERMINAL_POOL_IPS"):
        return True
    import http.client
    try:
        conn = http.client.HTTPConnection("127.0.0.1", 8083, timeout=timeout)
        conn.request(
            "GET", "/init?rank=4294967295&topology=trn2.8x1&n_slices=1")
        resp = conn.getresponse()
        resp.read()
        conn.close()
        return resp.status == 200
    except Exception:
        return False


def _run_sim(nc, in_maps):
    """CoreSim fallback/debug path (KERNEL_BASS_SIM=1)."""
    from concourse import bass_interp
    outs = []
    for m in in_maps:
        sim = bass_interp.CoreSim(nc)
        for name, val in m.items():
            sim.tensor(name)[:] = val
        sim.simulate()
        outs.append({nm: np.array(sim.tensor(nm))
                     for nm in ("paths_o", "pfT_o", "scores_o")})
    return outs


def kernel(**inputs):
    global _NC_CACHE
    eg, uni = _host_rng()
    in_maps = [_per_core_inputs(inputs, b, eg, uni) for b in range(B)]
    global LAST_EXEC_NS
    use_sim = os.environ.get("KERNEL_BASS_SIM") == "1" or not _tunnel_ok()
    if not use_sim:
        try:
            if _NC_CACHE is None:
                _NC_CACHE = build_nc(num_devices=B)
            trace = os.environ.get("KERNEL_BASS_TRACE") == "1"
            res = run_bass_kernel_spmd(
                _NC_CACHE, in_maps, core_ids=list(range(B)), trace=trace)
            results = res.results
            if res.exec_time_ns is not None:
                LAST_EXEC_NS = res.exec_time_ns
        except Exception as e:
            import traceback
            traceback.print_exc()
            print(f"kernel: device path failed ({type(e).__name__}); "
                  "falling back to CoreSim", flush=True)
            use_sim = True
    if use_sim:
        nc1 = build_nc(num_devices=1)
        results = _run_sim(nc1, in_maps)
    paths = np.stack([results[b]["paths_o"] for b in range(B)])
    pf = np.stack([results[b]["pfT_o"].T for b in range(B)])
    scores = np.stack([results[b]["scores_o"][0] for b in range(B)])
    paths = paths.astype(np.int32)
    pf = pf.astype(np.float32)
    # weights = softmax over K of sigmoid scores
    sc = scores.astype(np.float32)[..., None]          # (B, K, 1)
    e = np.exp(sc - sc.max(axis=1, keepdims=True))
    weights = (e / e.sum(axis=1, keepdims=True)).astype(np.float32)
    return paths, weights, pf


# revision 33
# speedup vs baseline: 1.0700x; 1.0407x over previous
"""PathDiscovery Trainium2 Bass kernel.

Full pipeline on-device, data-parallel over batch (1 batch element per
NeuronCore, 8 cores):
  - fchange: mean |nf[t+1]-nf[t]| over (t, F)
  - source MLP -> z = logits * fchange -> top-8 (max8/max_index)
  - guided random walk with restart, 8 walks x 4 steps, vectorized over
    walks on partitions; the categorical sample is reproduced exactly as
    argmax_j probs_j * exp(gumbel_j) with host-precomputed (input
    independent) gumbel noise; adj/feature rows fetched by indirect DMA
  - bidirectional GRU encoding of gathered path features + path scorer

Host side: shard inputs over cores, precompute exp(gumbel)/uniform draws
from jax CPU threefry (deterministic, input-independent), pre-transpose
small weights, assemble outputs, softmax over K for weights.
"""

import os
import subprocess
import sys
import tempfile

import numpy as np

import concourse.bass as bass
import concourse.bacc as bacc
import concourse.mybir as mybir
from concourse import tile
from concourse.bass_utils import run_bass_kernel_spmd

B, T, N, F = 8, 12, 4096, 64
H = 64
K = 8
L = 5
ALPHA = 0.15
NT = N // 128          # 32 node tiles of 128
NCHUNK = N // 512      # 8 matmul chunks of 512
NHALF = N // 2
FD = mybir.dt.float32
I32 = mybir.dt.int32
U32 = mybir.dt.uint32
ALU = mybir.AluOpType
ACT = mybir.ActivationFunctionType
AX = mybir.AxisListType


# --------------------------------------------------------------------------
# Host-side RNG precompute (input independent; must match jax CPU threefry)
# --------------------------------------------------------------------------

_RNG_CACHE = None

_RNG_SCRIPT = r"""
import numpy as np
import jax, jax.numpy as jnp
B, K, L, N = 8, 8, 5, 4096
wkeys = jax.random.split(jax.random.key(42), B * K).reshape(B, K)
GUM = np.zeros((B, K, L - 1, N), np.float32)
UNI = np.zeros((B, K, L - 1), np.float32)
for b in range(B):
    for k in range(K):
        step_keys = jax.random.split(wkeys[b, k], L - 1)
        for i in range(L - 1):
            ku, kc = jax.random.split(step_keys[i])
            GUM[b, k, i] = np.asarray(jax.random.gumbel(kc, (N,), jnp.float32))
            UNI[b, k, i] = float(jax.random.uniform(ku))
np.savez(OUT_PATH, gum=GUM, uni=UNI)
"""


def _host_rng():
    """exp(gumbel) (B,K,4,N) f32 and uniforms (B,K,4) f32, via jax CPU."""
    global _RNG_CACHE
    if _RNG_CACHE is not None:
        return _RNG_CACHE
    # jax in this process may be bound to the axon backend; compute in a
    # clean subprocess pinned to the CPU backend so the threefry bits match
    # the reference implementation exactly.
    import jax  # just to locate the installed jax for the subprocess

    site_dir = os.path.dirname(os.path.dirname(jax.__file__))
    env = dict(os.environ)
    env.pop("TRN_TERMINAL_POOL_IPS", None)  # disables axon sitecustomize boot
    env["JAX_PLATFORMS"] = "cpu"
    env["PYTHONPATH"] = site_dir + os.pathsep + env.get("PYTHONPATH", "")
    with tempfile.TemporaryDirectory() as td:
        out_path = os.path.join(td, "rng.npz")
        script = f"OUT_PATH = {out_path!r}\n" + _RNG_SCRIPT
        subprocess.run(
            [sys.executable, "-c", script], env=env, check=True,
            capture_output=True,
        )
        dat = np.load(out_path)
        gum, uni = dat["gum"], dat["uni"]
    eg = np.exp(gum.astype(np.float64)).astype(np.float32)
    _RNG_CACHE = (eg, uni)
    return _RNG_CACHE


# --------------------------------------------------------------------------
# Bass kernel builder
# --------------------------------------------------------------------------

def build_nc(num_devices=1, debug=False):
    nc = bacc.Bacc(
        "TRN2",
        target_bir_lowering=False,
        debug=debug,
        num_devices=num_devices,
    )

    d_nf = nc.dram_tensor("nf", [T, N, F], FD, kind="ExternalInput")
    d_feats = nc.dram_tensor("feats", [N, F], FD, kind="ExternalInput")
    d_adj = nc.dram_tensor("adj", [N, N], FD, kind="ExternalInput")
    d_w1t = nc.dram_tensor("w1t", [F, H], FD, kind="ExternalInput")
    d_b1 = nc.dram_tensor("b1", [H, 1], FD, kind="ExternalInput")
    d_w2t = nc.dram_tensor("w2t", [H, 1], FD, kind="ExternalInput")
    d_b2 = nc.dram_tensor("b2", [1, 1], FD, kind="ExternalInput")
    gru_dram = {}
    for pre in ("f", "b"):
        for nm, shape in (
            ("ih_rzT", [F, 2 * H]), ("ih_nT", [F, H]),
            ("hh_rzT", [H, 2 * H]), ("hh_nT", [H, H]),
            ("bih_rz", [2 * H, 1]), ("bih_n", [H, 1]),
            ("bhh_rz", [2 * H, 1]), ("bhh_n", [H, 1]),
        ):
            gru_dram[f"{pre}_{nm}"] = nc.dram_tensor(
                f"{pre}_{nm}", shape, FD, kind="ExternalInput")
    d_wps1t = nc.dram_tensor("wps1t", [2 * H, H], FD, kind="ExternalInput")
    d_bps1 = nc.dram_tensor("bps1", [H, 1], FD, kind="ExternalInput")
    d_wps2t = nc.dram_tensor("wps2t", [H, 1], FD, kind="ExternalInput")
    d_bps2 = nc.dram_tensor("bps2", [1, 1], FD, kind="ExternalInput")
    d_eg = nc.dram_tensor("eg", [L - 1, 64, 512], FD, kind="ExternalInput")
    d_uni = nc.dram_tensor("uni", [L - 1, 1, K], FD, kind="ExternalInput")
    d_ident = nc.dram_tensor("ident", [128, 128], FD, kind="ExternalInput")
    d_selmat = nc.dram_tensor("selmat", [8, 64], FD, kind="ExternalInput")
    d_repmat = nc.dram_tensor("repmat", [8, 64], FD, kind="ExternalInput")
    d_base512 = nc.dram_tensor("base512", [64, 1], FD, kind="ExternalInput")
    d_schunk = nc.dram_tensor("schunk", [64, 1], FD, kind="ExternalInput")

    d_paths = nc.dram_tensor("paths_o", [K, L], I32, kind="ExternalOutput")
    d_pfT = nc.dram_tensor("pfT_o", [2 * H, K], FD, kind="ExternalOutput")
    d_scores = nc.dram_tensor("scores_o", [1, K], FD, kind="ExternalOutput")
    d_fch = nc.dram_tensor("fch_o", [1, N], FD, kind="ExternalOutput")
    d_z = nc.dram_tensor("z_o", [1, N], FD, kind="ExternalOutput")

    with tile.TileContext(nc) as tc:
        with (
            tc.tile_pool(name="const", bufs=1) as constp,
            tc.tile_pool(name="big", bufs=1) as bigp,
            tc.tile_pool(name="row", bufs=3) as rowp,
            tc.tile_pool(name="small", bufs=3) as smp,
            tc.tile_pool(name="ps", bufs=3, space="PSUM") as psp,
            tc.tile_pool(name="psh", bufs=2, space="PSUM") as psh,
        ):
            # ---------------- constants ----------------
            ident = constp.tile([128, 128], FD, tag="ident")
            nc.sync.dma_start(ident[:], d_ident[:])
            w1t = constp.tile([F, H], FD, tag="w1t")
            nc.sync.dma_start(w1t[:], d_w1t[:])
            b1 = constp.tile([H, 1], FD, tag="b1")
            nc.sync.dma_start(b1[:], d_b1[:])
            w2t = constp.tile([H, 1], FD, tag="w2t")
            nc.sync.dma_start(w2t[:], d_w2t[:])
            b2 = constp.tile([1, 1], FD, tag="b2")
            nc.sync.dma_start(b2[:], d_b2[:])
            gru = {}
            for nm, dtr in gru_dram.items():
                t_ = constp.tile(list(dtr.shape), FD, tag=nm)
                nc.sync.dma_start(t_[:], dtr[:])
                gru[nm] = t_
            wps1t = constp.tile([2 * H, H], FD, tag="wps1t")
            nc.sync.dma_start(wps1t[:], d_wps1t[:])
            bps1 = constp.tile([H, 1], FD, tag="bps1")
            nc.sync.dma_start(bps1[:], d_bps1[:])
            wps2t = constp.tile([H, 1], FD, tag="wps2t")
            nc.sync.dma_start(wps2t[:], d_wps2t[:])
            bps2 = constp.tile([1, 1], FD, tag="bps2")
            nc.sync.dma_start(bps2[:], d_bps2[:])

            ones_f8 = constp.tile([F, K], FD, tag="ones_f8")
            nc.vector.memset(ones_f8[:], 1.0)
            selmat = constp.tile([8, 64], FD, tag="selmat")
            nc.sync.dma_start(selmat[:], d_selmat[:])
            repmat = constp.tile([8, 64], FD, tag="repmat")
            nc.sync.dma_start(repmat[:], d_repmat[:])
            base512 = constp.tile([64, 1], FD, tag="base512")
            nc.sync.dma_start(base512[:], d_base512[:])
            schunk = constp.tile([64, 1], FD, tag="schunk")
            nc.sync.dma_start(schunk[:], d_schunk[:])

            iota512 = constp.tile([64, 512], FD, tag="iota512")
            nc.gpsimd.iota(
                iota512[:], pattern=[[1, 512]], base=0, channel_multiplier=0,
                allow_small_or_imprecise_dtypes=True,
            )

            uni_t = []
            for s in range(L - 1):
                t_ = constp.tile([1, K], FD, tag=f"uni{s}")
                nc.sync.dma_start(t_[:], d_uni[s])
                uni_t.append(t_)

            # persistent big tiles
            featsT = bigp.tile([F, N], FD, tag="featsT")
            sq64 = bigp.tile([64, 512], FD, tag="sq64")
            vis01 = bigp.tile([64, 512], FD, tag="vis01")
            src_col = bigp.tile([K, 1], FD, tag="src_col")
            path_f = bigp.tile([K, L], FD, tag="path_f")
            pfT = bigp.tile([2 * H, K], FD, tag="pfT")

            fch_row = rowp.tile([1, N], FD, tag="row")

            # ---------------- phase 1: fchange + featsT (scoped pools) ----
            with (
                tc.tile_pool(name="nfs", bufs=4) as nfp,
                tc.tile_pool(name="dts", bufs=3) as dtp,
            ):
                acc = nfp.tile([128, NT], FD, tag="acc")
                nc.vector.memset(acc[:], 0.0)
                nf_view = d_nf.ap().rearrange("t (a p) f -> t p a f", p=128)
                # nf[T-1] (= feats) is also a standalone input: load it
                # up front so featsT/sq/z matmuls overlap the 12.5MB stream
                feats2d = nfp.tile([128, NT * F], FD, tag="feats2d")
                nc.gpsimd.dma_start(
                    feats2d[:], d_feats.ap().rearrange(
                        "(a p) f -> p a f", p=128))
                prev = nfp.tile([128, NT * F], FD, tag="nft")
                nc.gpsimd.dma_start(prev[:], nf_view[0])
                for t in range(1, T):
                    curt = nfp.tile([128, NT * F], FD, tag="nft")
                    nc.gpsimd.dma_start(curt[:], nf_view[t])
                    dtile = dtp.tile([128, NT * F], FD, tag="dtile")
                    nc.vector.tensor_tensor(
                        dtile[:], curt[:], prev[:], op=ALU.subtract)
                    r = smp.tile([128, NT], FD, tag="red")
                    nc.vector.tensor_reduce(
                        r[:], dtile[:].rearrange("p (a f) -> p a f", f=F),
                        axis=AX.X, op=ALU.add, apply_absolute_value=True,
                    )
                    nc.vector.tensor_tensor(acc[:], acc[:], r[:], op=ALU.add)
                    prev = curt
                # fchange = acc / 704 -> transpose -> row
                fchm = smp.tile([128, NT], FD, tag="fchm")
                nc.vector.tensor_scalar(
                    fchm[:], acc[:],
                    float(np.float32(1.0) / np.float32((T - 1) * F)), None,
                    op0=ALU.mult)
                fch_ps = psp.tile([NT, 128], FD, tag="mm")
                nc.tensor.transpose(fch_ps[:], fchm[:], ident[:])
                fch_t = smp.tile([NT, 128], FD, tag="fch_t")
                nc.scalar.copy(fch_t[:], fch_ps[:])
                nc.sync.dma_start(fch_row[:], fch_t[:])
                nc.sync.dma_start(d_fch[:], fch_row[:])

                # featsT from transposed 128x64 tiles of feats
                for a in range(NT):
                    tp = psp.tile([F, 128], FD, tag="mm")
                    nc.tensor.transpose(
                        tp[:], feats2d[:, a * F:(a + 1) * F], ident[:])
                    nc.scalar.copy(featsT[:, a * 128:(a + 1) * 128], tp[:])

            # ---------------- sq -> sq64 (64, 512) ----------------
            sq_row = rowp.tile([1, N], FD, tag="row")
            for c in range(NCHUNK):
                sl = slice(c * 512, (c + 1) * 512)
                sqc = smp.tile([F, 512], FD, tag="sqc")
                nc.scalar.square(sqc[:], featsT[:, sl])
                sq_ps = psp.tile([1, 512], FD, tag="mm")
                nc.tensor.matmul(sq_ps[:], ones_f8[:, 0:1], sqc[:],
                                 start=True, stop=True)
                nc.scalar.copy(sq_row[:, sl], sq_ps[:])
            sq_rs = smp.tile([8, 512], FD, tag="rs8")
            nc.sync.dma_start(sq_rs[:], sq_row[:])
            sq64_ps = psp.tile([64, 512], FD, tag="mm")
            nc.tensor.matmul(sq64_ps[:], selmat[:], sq_rs[:],
                             start=True, stop=True)
            nc.scalar.copy(sq64[:], sq64_ps[:])

            # ---------------- acc_bias -> vis01 = (1 + accb) ----------
            S = smp.tile([1, 1], FD, tag="S")
            nc.vector.tensor_reduce(
                S[:], fch_row[:], axis=AX.X, op=ALU.add)
            Sp = smp.tile([1, 1], FD, tag="Sp")
            nc.vector.tensor_scalar(Sp[:], S[:], 1e-8, None, op0=ALU.add)
            rSp = smp.tile([1, 1], FD, tag="rSp")
            nc.vector.reciprocal(rSp[:], Sp[:])
            accb_row = rowp.tile([1, N], FD, tag="row")
            nc.vector.tensor_scalar(
                accb_row[:], fch_row[:], rSp[:, 0:1], None, op0=ALU.mult)
            ac_rs = smp.tile([8, 512], FD, tag="rs8")
            nc.sync.dma_start(ac_rs[:], accb_row[:])
            ac64_ps = psp.tile([64, 512], FD, tag="mm")
            nc.tensor.matmul(ac64_ps[:], selmat[:], ac_rs[:],
                             start=True, stop=True)
            # vis01 starts as (1 + acc_bias); visited nodes zeroed in place
            nc.vector.tensor_scalar(
                vis01[:], ac64_ps[:], 1.0, None, op0=ALU.add)

            # ---------------- source logits z ----------------
            # raw logits don't need fchange -> computed during the stream
            zraw_row = rowp.tile([1, N], FD, tag="row")
            for c in range(NCHUNK):
                sl = slice(c * 512, (c + 1) * 512)
                h_ps = psp.tile([H, 512], FD, tag="mm")
                nc.tensor.matmul(h_ps[:], w1t[:], featsT[:, sl],
                                 start=True, stop=True)
                h_sb = smp.tile([H, 512], FD, tag="h_sb")
                nc.scalar.activation(
                    h_sb[:], h_ps[:], ACT.Relu, bias=b1[:, 0:1], scale=1.0)
                z_ps = psp.tile([1, 512], FD, tag="mm")
                nc.tensor.matmul(z_ps[:], w2t[:], h_sb[:],
                                 start=True, stop=True)
                nc.scalar.copy(zraw_row[:, sl], z_ps[:])
            z_row = rowp.tile([1, N], FD, tag="row")
            for c in range(NCHUNK):
                sl = slice(c * 512, (c + 1) * 512)
                nc.vector.scalar_tensor_tensor(
                    z_row[:, sl], zraw_row[:, sl], b2[:, 0:1],
                    fch_row[:, sl], op0=ALU.add, op1=ALU.mult)
            nc.sync.dma_start(d_z[:], z_row[:])

            # ---------------- top-8 sources ----------------
            zmax = smp.tile([1, 8], FD, tag="zmax")
            nc.vector.max(zmax[:], z_row[:])
            zidx = smp.tile([1, 8], U32, tag="zidx")
            nc.vector.max_index(zidx[:], zmax[:], z_row[:])
            zidx_f = smp.tile([1, 8], FD, tag="zidx_f")
            nc.vector.tensor_copy(zidx_f[:], zidx[:])
            nc.sync.dma_start(src_col[:], zidx_f[:])

            # ---------------- walk ----------------
            # layout: partition p = k*8 + s (walk k, chunk s); node
            # j = s*512 + f.  adj viewed as (4096*8, 512) row-chunks.
            # per-walk reductions go through a PE transpose to a (1, 64)
            # row, reduced in 8-groups along the free dim.
            adj_ck = d_adj.ap().rearrange("r (s c) -> (r s) c", c=512)

            def rep_walk_scalar(col8, tag):
                """(8,1) col -> (64,1) per-walk replication (repmat matmul)."""
                ps = psp.tile([64, 1], FD, tag="mm")
                nc.tensor.matmul(ps[:], repmat[:], col8[:],
                                 start=True, stop=True)
                sb = smp.tile([64, 1], FD, tag=tag)
                nc.scalar.copy(sb[:], ps[:])
                return sb

            def col_to_row(col, tag, n=8):
                """(n,1) -> (1,n) via PE transpose."""
                ps = psp.tile([1, n], FD, tag="mm")
                nc.tensor.transpose(ps[:], col[:, 0:1], ident[0:n, 0:n])
                sb = smp.tile([1, n], FD, tag=tag)
                nc.scalar.copy(sb[:], ps[:])
                return sb

            def row_to_col(row, tag):
                """(1,8) -> (8,1) via PE transpose."""
                ps = psp.tile([8, 1], FD, tag="mm")
                nc.tensor.transpose(ps[:], row[0:1, :], ident[0:1, 0:1])
                sb = smp.tile([8, 1], FD, tag=tag)
                nc.scalar.copy(sb[:], ps[:])
                return sb

            src_row = zidx_f        # (1, 8) source node ids as f32
            nc.vector.tensor_copy(path_f[:, 0:1], src_col[:])
            srcrep = rep_walk_scalar(src_col, "srcrep")
            srcadj = smp.tile([64, 1], FD, tag="srcadj")
            nc.vector.tensor_scalar(
                srcadj[:], srcrep[:], base512[:, 0:1], None,
                op0=ALU.subtract)
            nc.vector.scalar_tensor_tensor(
                vis01[:], iota512[:], srcadj[:, 0:1], vis01[:],
                op0=ALU.not_equal, op1=ALU.mult)

            cur_col = src_col
            xT = []
            with tc.tile_pool(name="wkp", bufs=3) as wkp:
                for pos in range(L):
                    cur_i = smp.tile([K, 1], I32, tag="cur_i")
                    nc.vector.tensor_copy(cur_i[:], cur_col[:])
                    frows = smp.tile([K, F], FD, tag="frows")
                    nc.gpsimd.indirect_dma_start(
                        out=frows[:], out_offset=None, in_=d_feats.ap(),
                        in_offset=bass.IndirectOffsetOnAxis(
                            ap=cur_i[:, 0:1], axis=0))
                    xt_ps = psp.tile([F, K], FD, tag="mm")
                    nc.tensor.transpose(xt_ps[:], frows[:], ident[0:K, 0:K])
                    xt = bigp.tile([F, K], FD, tag=f"xT{pos}")
                    nc.scalar.copy(xt[:], xt_ps[:])
                    xT.append(xt)
                    if pos == L - 1:
                        break
                    i = pos
                    # adj row-chunk gather: idx64 = cur*8 + s
                    currep = rep_walk_scalar(cur_col, "currep")
                    idx64f = smp.tile([64, 1], FD, tag="idx64f")
                    nc.vector.tensor_scalar(
                        idx64f[:], currep[:], 8.0, schunk[:, 0:1],
                        op0=ALU.mult, op1=ALU.add)
                    idx64 = smp.tile([64, 1], I32, tag="idx64")
                    nc.vector.tensor_copy(idx64[:], idx64f[:])
                    adjrows = wkp.tile([64, 512], FD, tag="adjrows")
                    nc.gpsimd.indirect_dma_start(
                        out=adjrows[:], out_offset=None, in_=adj_ck,
                        in_offset=bass.IndirectOffsetOnAxis(
                            ap=idx64[:, 0:1], axis=0))
                    eg_s = wkp.tile([64, 512], FD, tag="eg")
                    nc.sync.dma_start(eg_s[:], d_eg[i])
                    fr2 = smp.tile([K, F], FD, tag="fr2")
                    nc.scalar.square(fr2[:], frows[:])
                    sqcur = smp.tile([K, 1], FD, tag="sqcur")
                    nc.vector.tensor_reduce(
                        sqcur[:], fr2[:], axis=AX.X, op=ALU.add)
                    sqcur_r = rep_walk_scalar(sqcur, "sqcur_r")
                    # dots: 8 chunk matmuls -> (8, 4096) sbuf -> DMA
                    # reshape into the (64, 512) walk layout
                    dots8 = wkp.tile([K, N], FD, tag="dots8")
                    NQ = N // 4
                    for hf in range(4):
                        dots_ps = psh.tile([K, NQ], FD, tag="mmh")
                        for c in range(2):
                            sl_ps = slice(c * 512, (c + 1) * 512)
                            sl_f = slice(hf * NQ + c * 512,
                                         hf * NQ + (c + 1) * 512)
                            nc.tensor.matmul(
                                dots_ps[:, sl_ps], xT[pos][:],
                                featsT[:, sl_f], start=True, stop=True)
                        nc.scalar.copy(
                            dots8[:, hf * NQ:(hf + 1) * NQ], dots_ps[:])
                    d2 = wkp.tile([64, 512], FD, tag="d2")
                    nc.sync.dma_start(
                        d2[:],
                        dots8[:].rearrange("k (s c) -> (k s) c", c=512))
                    nc.vector.scalar_tensor_tensor(
                        d2[:], d2[:], -2.0, sq64[:],
                        op0=ALU.mult, op1=ALU.add)
                    nc.vector.tensor_scalar(
                        d2[:], d2[:], sqcur_r[:, 0:1], 0.0,
                        op0=ALU.add, op1=ALU.max)
                    m2p = smp.tile([64, 1], FD, tag="m2p")
                    nc.vector.tensor_reduce(
                        m2p[:], d2[:], axis=AX.X, op=ALU.max)
                    m2row = col_to_row(m2p, "m2row", n=64)
                    m2w = smp.tile([1, 8], FD, tag="m2w")
                    nc.vector.tensor_reduce(
                        m2w[:], m2row[:].rearrange("p (a b) -> p a b", b=8),
                        axis=AX.X, op=ALU.max)
                    m2c = row_to_col(m2w, "m2c")
                    rm2 = smp.tile([K, 1], FD, tag="rm2")
                    nc.vector.reciprocal(rm2[:], m2c[:])
                    rm2_r = rep_walk_scalar(rm2, "rm2_r")
                    nc.scalar.activation(
                        d2[:], d2[:], ACT.Sqrt, bias=0.0,
                        scale=rm2_r[:, 0:1])
                    # d2 <- (guid + 1) * eg
                    nc.vector.scalar_tensor_tensor(
                        d2[:], d2[:], 1.0, eg_s[:],
                        op0=ALU.add, op1=ALU.mult)
                    # y (in adjrows): (adj * (vis*accb1)) * ((guid+1) * eg)
                    nc.vector.tensor_tensor(
                        adjrows[:], adjrows[:], vis01[:], op=ALU.mult)
                    nc.vector.tensor_tensor(
                        adjrows[:], adjrows[:], d2[:], op=ALU.mult)
                    # per-partition top-1 then global per-walk argmax with
                    # first-index tie-break via eq-mask + min-index
                    y8 = smp.tile([64, 8], FD, tag="y8")
                    nc.vector.max(y8[:], adjrows[:])
                    yidx = smp.tile([64, 8], U32, tag="yidx")
                    nc.vector.max_index(yidx[:], y8[:], adjrows[:])
                    jf = smp.tile([64, 1], FD, tag="jf")
                    nc.vector.tensor_copy(jf[:], yidx[:, 0:1])
                    nc.vector.tensor_scalar(
                        jf[:], jf[:], base512[:, 0:1], None, op0=ALU.add)
                    yvrow = col_to_row(y8[:, 0:1], "yvrow", n=64)
                    jfrow = col_to_row(jf, "jfrow", n=64)
                    maxv = smp.tile([1, 8], FD, tag="maxv")
                    nc.vector.tensor_reduce(
                        maxv[:],
                        yvrow[:].rearrange("p (a b) -> p a b", b=8),
                        axis=AX.X, op=ALU.max)
                    eqm = smp.tile([1, 64], FD, tag="eqm")
                    nc.vector.tensor_tensor(
                        eqm[:].rearrange("p (a b) -> p a b", b=8),
                        yvrow[:].rearrange("p (a b) -> p a b", b=8),
                        maxv[:].unsqueeze(2).to_broadcast([1, 8, 8]),
                        op=ALU.is_equal)
                    cand = smp.tile([1, 64], FD, tag="cand")
                    nc.vector.scalar_tensor_tensor(
                        cand[:], eqm[:], -65536.0, jfrow[:],
                        op0=ALU.mult, op1=ALU.add)
                    nc.vector.tensor_scalar(
                        cand[:], cand[:], 65536.0, None, op0=ALU.add)
                    samp_row = smp.tile([1, 8], FD, tag="samp_row")
                    nc.vector.tensor_reduce(
                        samp_row[:],
                        cand[:].rearrange("p (a b) -> p a b", b=8),
                        axis=AX.X, op=ALU.min)
                    # teleport select in row form
                    tel = smp.tile([1, 8], FD, tag="tel")
                    nc.vector.tensor_scalar(
                        tel[:], uni_t[i][:], float(np.float32(ALPHA)), None,
                        op0=ALU.is_lt)
                    dsrc = smp.tile([1, 8], FD, tag="dsrc")
                    nc.vector.tensor_tensor(
                        dsrc[:], src_row[:], samp_row[:], op=ALU.subtract)
                    td = smp.tile([1, 8], FD, tag="td")
                    nc.vector.tensor_tensor(
                        td[:], tel[:], dsrc[:], op=ALU.mult)
                    nxt_row = smp.tile([1, 8], FD, tag=f"nxtr{i}")
                    nc.vector.tensor_tensor(
                        nxt_row[:], td[:], samp_row[:], op=ALU.add)
                    nxt_col = row_to_col(nxt_row, f"nxtc{i}")
                    nc.vector.tensor_copy(path_f[:, i + 1:i + 2], nxt_col[:])
                    nxtrep = rep_walk_scalar(nxt_col, "nxtrep")
                    nxtadj = smp.tile([64, 1], FD, tag="nxtadj")
                    nc.vector.tensor_scalar(
                        nxtadj[:], nxtrep[:], base512[:, 0:1], None,
                        op0=ALU.subtract)
                    nc.vector.scalar_tensor_tensor(
                        vis01[:], iota512[:], nxtadj[:, 0:1], vis01[:],
                        op0=ALU.not_equal, op1=ALU.mult)
                    cur_col = nxt_col

            paths_i = smp.tile([K, L], I32, tag="paths_i")
            nc.vector.tensor_copy(paths_i[:], path_f[:])
            nc.sync.dma_start(d_paths[:], paths_i[:])

            # ---------------- GRU ----------------
            def gru_cell(h, xt, pre, hp):
                ps_i_rz = psp.tile([2 * H, K], FD, tag="mm")
                nc.tensor.matmul(ps_i_rz[:], gru[f"{pre}_ih_rzT"][:], xt[:],
                                 start=True, stop=True)
                ps_h_rz = psp.tile([2 * H, K], FD, tag="mm")
                nc.tensor.matmul(ps_h_rz[:], gru[f"{pre}_hh_rzT"][:], h[:],
                                 start=True, stop=True)
                gh_rz = smp.tile([2 * H, K], FD, tag="gh_rz")
                nc.vector.tensor_scalar(
                    gh_rz[:], ps_h_rz[:], gru[f"{pre}_bhh_rz"][:, 0:1], None,
                    op0=ALU.add)
                s_rz = smp.tile([2 * H, K], FD, tag="s_rz")
                nc.vector.scalar_tensor_tensor(
                    s_rz[:], ps_i_rz[:], gru[f"{pre}_bih_rz"][:, 0:1],
                    gh_rz[:], op0=ALU.add, op1=ALU.add)
                rz = smp.tile([2 * H, K], FD, tag="rz")
                nc.scalar.activation(rz[:], s_rz[:], ACT.Sigmoid)
                ps_i_n = psp.tile([H, K], FD, tag="mm")
                nc.tensor.matmul(ps_i_n[:], gru[f"{pre}_ih_nT"][:], xt[:],
                                 start=True, stop=True)
                ps_h_n = psp.tile([H, K], FD, tag="mm")
                nc.tensor.matmul(ps_h_n[:], gru[f"{pre}_hh_nT"][:], h[:],
                                 start=True, stop=True)
                inn = smp.tile([H, K], FD, tag="inn")
                nc.vector.tensor_scalar(
                    inn[:], ps_i_n[:], gru[f"{pre}_bih_n"][:, 0:1], None,
                    op0=ALU.add)
                hn = smp.tile([H, K], FD, tag="hn")
                nc.vector.tensor_scalar(
                    hn[:], ps_h_n[:], gru[f"{pre}_bhh_n"][:, 0:1], None,
                    op0=ALU.add)
                rh = smp.tile([H, K], FD, tag="rh")
                nc.vector.tensor_tensor(
                    rh[:], rz[0:H, :], hn[:], op=ALU.mult)
                npre = smp.tile([H, K], FD, tag="npre")
                nc.vector.tensor_tensor(npre[:], inn[:], rh[:], op=ALU.add)
                ng = smp.tile([H, K], FD, tag="ng")
                nc.scalar.activation(ng[:], npre[:], ACT.Tanh)
                zt = smp.tile([H, K], FD, tag="zt")
                nc.scalar.copy(zt[:], rz[H:2 * H, :])
                omz = smp.tile([H, K], FD, tag="omz")
                nc.vector.tensor_scalar(
                    omz[:], zt[:], -1.0, 1.0, op0=ALU.mult, op1=ALU.add)
                tb = smp.tile([H, K], FD, tag="tb")
                nc.vector.tensor_tensor(tb[:], omz[:], ng[:], op=ALU.mult)
                tcz = smp.tile([H, K], FD, tag="tc2")
                nc.vector.tensor_tensor(tcz[:], zt[:], h[:], op=ALU.mult)
                h2 = smp.tile([H, K], FD, tag=f"h_{pre}{hp}")
                nc.vector.tensor_tensor(h2[:], tb[:], tcz[:], op=ALU.add)
                return h2

            h_f = smp.tile([H, K], FD, tag="h_f0")
            nc.vector.memset(h_f[:], 0.0)
            for t in range(L):
                h_f = gru_cell(h_f, xT[t], "f", t + 1)
            h_b0 = smp.tile([H, K], FD, tag="h_b0")
            nc.vector.memset(h_b0[:], 0.0)
            h_b = gru_cell(h_b0, xT[L - 1], "b", 1)

            nc.vector.tensor_copy(pfT[0:H, :], h_f[:])
            nc.vector.tensor_copy(pfT[H:2 * H, :], h_b[:])
            nc.sync.dma_start(d_pfT[:], pfT[:])

            # ---------------- scorer ----------------
            ps1 = psp.tile([H, K], FD, tag="mm")
            nc.tensor.matmul(ps1[:], wps1t[:], pfT[:], start=True, stop=True)
            hs = smp.tile([H, K], FD, tag="hs")
            nc.scalar.activation(
                hs[:], ps1[:], ACT.Relu, bias=bps1[:, 0:1], scale=1.0)
            ps2 = psp.tile([1, K], FD, tag="mm")
            nc.tensor.matmul(ps2[:], wps2t[:], hs[:], start=True, stop=True)
            scores = smp.tile([1, K], FD, tag="scores")
            nc.scalar.activation(
                scores[:], ps2[:], ACT.Sigmoid, bias=bps2[:, 0:1], scale=1.0)
            nc.sync.dma_start(d_scores[:], scores[:])

    return nc


# --------------------------------------------------------------------------
# Host orchestration
# --------------------------------------------------------------------------

def _per_core_inputs(inputs, b, eg, uni):
    f32 = np.float32
    def c(x):
        return np.ascontiguousarray(np.asarray(x, dtype=f32))
    nf = c(inputs["node_features"][b])
    m = {
        "nf": nf,
        "feats": c(nf[T - 1]),
        "adj": c(inputs["adj_matrix"][b]),
        "w1t": c(inputs["W_sp1"].T),
        "b1": c(inputs["b_sp1"]).reshape(H, 1),
        "w2t": c(inputs["W_sp2"].T),
        "b2": c(inputs["b_sp2"]).reshape(1, 1),
        "wps1t": c(inputs["W_ps1"].T),
        "bps1": c(inputs["b_ps1"]).reshape(H, 1),
        "wps2t": c(inputs["W_ps2"].T),
        "bps2": c(inputs["b_ps2"]).reshape(1, 1),
        "eg": c(np.swapaxes(eg[b], 0, 1)).reshape(L - 1, 64, 512),
        "uni": c(np.swapaxes(uni[b], 0, 1))[:, None, :],  # (4, 1, K)
        "ident": np.eye(128, dtype=f32),
        "selmat": (np.arange(64)[None, :] % 8 == np.arange(8)[:, None]
                   ).astype(f32),
        "repmat": (np.arange(64)[None, :] // 8 == np.arange(8)[:, None]
                   ).astype(f32),
        "base512": ((np.arange(64) % 8) * 512).astype(f32).reshape(64, 1),
        "schunk": (np.arange(64) % 8).astype(f32).reshape(64, 1),
    }
    for pre, wih, whh, bih, bhh in (
        ("f", "Wih_f", "Whh_f", "bih_f", "bhh_f"),
        ("b", "Wih_b", "Whh_b", "bih_b", "bhh_b"),
    ):
        Wih, Whh = inputs[wih], inputs[whh]
        Bih, Bhh = inputs[bih], inputs[bhh]
        m[f"{pre}_ih_rzT"] = c(Wih[0:2 * H].T)
        m[f"{pre}_ih_nT"] = c(Wih[2 * H:3 * H].T)
        m[f"{pre}_hh_rzT"] = c(Whh[0:2 * H].T)
        m[f"{pre}_hh_nT"] = c(Whh[2 * H:3 * H].T)
        m[f"{pre}_bih_rz"] = c(Bih[0:2 * H]).reshape(2 * H, 1)
        m[f"{pre}_bih_n"] = c(Bih[2 * H:3 * H]).reshape(H, 1)
        m[f"{pre}_bhh_rz"] = c(Bhh[0:2 * H]).reshape(2 * H, 1)
        m[f"{pre}_bhh_n"] = c(Bhh[2 * H:3 * H]).reshape(H, 1)
    return m


_NC_CACHE = None
LAST_EXEC_NS = None


def _tunnel_ok(timeout=20.0):
    """Quick health probe of the axon relay before touching PJRT (a dead
    tunnel makes backend init hang indefinitely). Native-device setups
    (no relay env) skip the probe."""
    if not os.environ.get("TRN_TERMINAL_POOL_IPS"):
        return True
    import http.client
    try:
        conn = http.client.HTTPConnection("127.0.0.1", 8083, timeout=timeout)
        conn.request(
            "GET", "/init?rank=4294967295&topology=trn2.8x1&n_slices=1")
        resp = conn.getresponse()
        resp.read()
        conn.close()
        return resp.status == 200
    except Exception:
        return False


def _run_sim(nc, in_maps):
    """CoreSim fallback/debug path (KERNEL_BASS_SIM=1)."""
    from concourse import bass_interp
    outs = []
    for m in in_maps:
        sim = bass_interp.CoreSim(nc)
        for name, val in m.items():
            sim.tensor(name)[:] = val
        sim.simulate()
        outs.append({nm: np.array(sim.tensor(nm))
                     for nm in ("paths_o", "pfT_o", "scores_o")})
    return outs


def kernel(**inputs):
    global _NC_CACHE
    eg, uni = _host_rng()
    in_maps = [_per_core_inputs(inputs, b, eg, uni) for b in range(B)]
    global LAST_EXEC_NS
    use_sim = os.environ.get("KERNEL_BASS_SIM") == "1" or not _tunnel_ok()
    if not use_sim:
        try:
            if _NC_CACHE is None:
                _NC_CACHE = build_nc(num_devices=B)
            trace = os.environ.get("KERNEL_BASS_TRACE") == "1"
            res = run_bass_kernel_spmd(
                _NC_CACHE, in_maps, core_ids=list(range(B)), trace=trace)
            results = res.results
            if res.exec_time_ns is not None:
                LAST_EXEC_NS = res.exec_time_ns
        except Exception as e:
            import traceback
            traceback.print_exc()
            print(f"kernel: device path failed ({type(e).__name__}); "
                  "falling back to CoreSim", flush=True)
            use_sim = True
    if use_sim:
        nc1 = build_nc(num_devices=1)
        results = _run_sim(nc1, in_maps)
    paths = np.stack([results[b]["paths_o"] for b in range(B)])
    pf = np.stack([results[b]["pfT_o"].T for b in range(B)])
    scores = np.stack([results[b]["scores_o"][0] for b in range(B)])
    paths = paths.astype(np.int32)
    pf = pf.astype(np.float32)
    # weights = softmax over K of sigmoid scores
    sc = scores.astype(np.float32)[..., None]          # (B, K, 1)
    e = np.exp(sc - sc.max(axis=1, keepdims=True))
    weights = (e / e.sum(axis=1, keepdims=True)).astype(np.float32)
    return paths, weights, pf


# revision 35
# speedup vs baseline: 1.1251x; 1.0515x over previous
"""PathDiscovery Trainium2 Bass kernel.

Full pipeline on-device, data-parallel over batch (1 batch element per
NeuronCore, 8 cores):
  - fchange: mean |nf[t+1]-nf[t]| over (t, F)
  - source MLP -> z = logits * fchange -> top-8 (max8/max_index)
  - guided random walk with restart, 8 walks x 4 steps, vectorized over
    walks on partitions; the categorical sample is reproduced exactly as
    argmax_j probs_j * exp(gumbel_j) with host-precomputed (input
    independent) gumbel noise; adj/feature rows fetched by indirect DMA
  - bidirectional GRU encoding of gathered path features + path scorer

Host side: shard inputs over cores, precompute exp(gumbel)/uniform draws
from jax CPU threefry (deterministic, input-independent), pre-transpose
small weights, assemble outputs, softmax over K for weights.
"""

import os
import subprocess
import sys
import tempfile

import numpy as np

import concourse.bass as bass
import concourse.bacc as bacc
import concourse.mybir as mybir
from concourse import tile
from concourse.bass_utils import run_bass_kernel_spmd

B, T, N, F = 8, 12, 4096, 64
H = 64
K = 8
L = 5
ALPHA = 0.15
NT = N // 128          # 32 node tiles of 128
NCHUNK = N // 512      # 8 matmul chunks of 512
NHALF = N // 2
FD = mybir.dt.float32
I32 = mybir.dt.int32
U32 = mybir.dt.uint32
ALU = mybir.AluOpType
ACT = mybir.ActivationFunctionType
AX = mybir.AxisListType


# --------------------------------------------------------------------------
# Host-side RNG precompute (input independent; must match jax CPU threefry)
# --------------------------------------------------------------------------

_RNG_CACHE = None

_RNG_SCRIPT = r"""
import numpy as np
import jax, jax.numpy as jnp
B, K, L, N = 8, 8, 5, 4096
wkeys = jax.random.split(jax.random.key(42), B * K).reshape(B, K)
GUM = np.zeros((B, K, L - 1, N), np.float32)
UNI = np.zeros((B, K, L - 1), np.float32)
for b in range(B):
    for k in range(K):
        step_keys = jax.random.split(wkeys[b, k], L - 1)
        for i in range(L - 1):
            ku, kc = jax.random.split(step_keys[i])
            GUM[b, k, i] = np.asarray(jax.random.gumbel(kc, (N,), jnp.float32))
            UNI[b, k, i] = float(jax.random.uniform(ku))
np.savez(OUT_PATH, gum=GUM, uni=UNI)
"""


def _host_rng():
    """exp(gumbel) (B,K,4,N) f32 and uniforms (B,K,4) f32, via jax CPU."""
    global _RNG_CACHE
    if _RNG_CACHE is not None:
        return _RNG_CACHE
    # jax in this process may be bound to the axon backend; compute in a
    # clean subprocess pinned to the CPU backend so the threefry bits match
    # the reference implementation exactly.
    import jax  # just to locate the installed jax for the subprocess

    site_dir = os.path.dirname(os.path.dirname(jax.__file__))
    env = dict(os.environ)
    env.pop("TRN_TERMINAL_POOL_IPS", None)  # disables axon sitecustomize boot
    env["JAX_PLATFORMS"] = "cpu"
    env["PYTHONPATH"] = site_dir + os.pathsep + env.get("PYTHONPATH", "")
    with tempfile.TemporaryDirectory() as td:
        out_path = os.path.join(td, "rng.npz")
        script = f"OUT_PATH = {out_path!r}\n" + _RNG_SCRIPT
        subprocess.run(
            [sys.executable, "-c", script], env=env, check=True,
            capture_output=True,
        )
        dat = np.load(out_path)
        gum, uni = dat["gum"], dat["uni"]
    eg = np.exp(gum.astype(np.float64)).astype(np.float32)
    _RNG_CACHE = (eg, uni)
    return _RNG_CACHE


# --------------------------------------------------------------------------
# Bass kernel builder
# --------------------------------------------------------------------------

def build_nc(num_devices=1, debug=False):
    nc = bacc.Bacc(
        "TRN2",
        target_bir_lowering=False,
        debug=debug,
        num_devices=num_devices,
    )

    d_nf = nc.dram_tensor("nf", [T, N, F], FD, kind="ExternalInput")
    d_feats = nc.dram_tensor("feats", [N, F], FD, kind="ExternalInput")
    d_adj = nc.dram_tensor("adj", [N, N], FD, kind="ExternalInput")
    d_w1t = nc.dram_tensor("w1t", [F, H], FD, kind="ExternalInput")
    d_b1 = nc.dram_tensor("b1", [H, 1], FD, kind="ExternalInput")
    d_w2t = nc.dram_tensor("w2t", [H, 1], FD, kind="ExternalInput")
    d_b2 = nc.dram_tensor("b2", [1, 1], FD, kind="ExternalInput")
    gru_dram = {}
    for pre in ("f", "b"):
        for nm, shape in (
            ("ih_rzT", [F, 2 * H]), ("ih_nT", [F, H]),
            ("hh_rzT", [H, 2 * H]), ("hh_nT", [H, H]),
            ("bih_rz", [2 * H, 1]), ("bih_n", [H, 1]),
            ("bhh_rz", [2 * H, 1]), ("bhh_n", [H, 1]),
        ):
            gru_dram[f"{pre}_{nm}"] = nc.dram_tensor(
                f"{pre}_{nm}", shape, FD, kind="ExternalInput")
    d_wps1t = nc.dram_tensor("wps1t", [2 * H, H], FD, kind="ExternalInput")
    d_bps1 = nc.dram_tensor("bps1", [H, 1], FD, kind="ExternalInput")
    d_wps2t = nc.dram_tensor("wps2t", [H, 1], FD, kind="ExternalInput")
    d_bps2 = nc.dram_tensor("bps2", [1, 1], FD, kind="ExternalInput")
    d_eg = nc.dram_tensor("eg", [L - 1, 64, 512], FD, kind="ExternalInput")
    d_uni = nc.dram_tensor("uni", [L - 1, 1, K], FD, kind="ExternalInput")
    d_ident = nc.dram_tensor("ident", [128, 128], FD, kind="ExternalInput")
    d_selmat = nc.dram_tensor("selmat", [8, 64], FD, kind="ExternalInput")
    d_repmat = nc.dram_tensor("repmat", [8, 64], FD, kind="ExternalInput")
    d_base512 = nc.dram_tensor("base512", [64, 1], FD, kind="ExternalInput")
    d_schunk = nc.dram_tensor("schunk", [64, 1], FD, kind="ExternalInput")

    d_paths = nc.dram_tensor("paths_o", [K, L], I32, kind="ExternalOutput")
    d_pfT = nc.dram_tensor("pfT_o", [2 * H, K], FD, kind="ExternalOutput")
    d_scores = nc.dram_tensor("scores_o", [1, K], FD, kind="ExternalOutput")
    d_fch = nc.dram_tensor("fch_o", [1, N], FD, kind="ExternalOutput")
    d_z = nc.dram_tensor("z_o", [1, N], FD, kind="ExternalOutput")

    with tile.TileContext(nc) as tc:
        with (
            tc.tile_pool(name="const", bufs=1) as constp,
            tc.tile_pool(name="big", bufs=1) as bigp,
            tc.tile_pool(name="row", bufs=3) as rowp,
            tc.tile_pool(name="small", bufs=3) as smp,
            tc.tile_pool(name="ps", bufs=3, space="PSUM") as psp,
            tc.tile_pool(name="psh", bufs=4, space="PSUM") as psh,
        ):
            # ---------------- constants ----------------
            ident = constp.tile([128, 128], FD, tag="ident")
            nc.sync.dma_start(ident[:], d_ident[:])
            w1t = constp.tile([F, H], FD, tag="w1t")
            nc.sync.dma_start(w1t[:], d_w1t[:])
            b1 = constp.tile([H, 1], FD, tag="b1")
            nc.sync.dma_start(b1[:], d_b1[:])
            w2t = constp.tile([H, 1], FD, tag="w2t")
            nc.sync.dma_start(w2t[:], d_w2t[:])
            b2 = constp.tile([1, 1], FD, tag="b2")
            nc.sync.dma_start(b2[:], d_b2[:])
            gru = {}
            for nm, dtr in gru_dram.items():
                t_ = constp.tile(list(dtr.shape), FD, tag=nm)
                nc.sync.dma_start(t_[:], dtr[:])
                gru[nm] = t_
            wps1t = constp.tile([2 * H, H], FD, tag="wps1t")
            nc.sync.dma_start(wps1t[:], d_wps1t[:])
            bps1 = constp.tile([H, 1], FD, tag="bps1")
            nc.sync.dma_start(bps1[:], d_bps1[:])
            wps2t = constp.tile([H, 1], FD, tag="wps2t")
            nc.sync.dma_start(wps2t[:], d_wps2t[:])
            bps2 = constp.tile([1, 1], FD, tag="bps2")
            nc.sync.dma_start(bps2[:], d_bps2[:])

            ones_f8 = constp.tile([F, K], FD, tag="ones_f8")
            nc.vector.memset(ones_f8[:], 1.0)
            selmat = constp.tile([8, 64], FD, tag="selmat")
            nc.sync.dma_start(selmat[:], d_selmat[:])
            repmat = constp.tile([8, 64], FD, tag="repmat")
            nc.sync.dma_start(repmat[:], d_repmat[:])
            base512 = constp.tile([64, 1], FD, tag="base512")
            nc.sync.dma_start(base512[:], d_base512[:])
            schunk = constp.tile([64, 1], FD, tag="schunk")
            nc.sync.dma_start(schunk[:], d_schunk[:])

            iota512 = constp.tile([64, 512], FD, tag="iota512")
            nc.gpsimd.iota(
                iota512[:], pattern=[[1, 512]], base=0, channel_multiplier=0,
                allow_small_or_imprecise_dtypes=True,
            )

            uni_t = []
            for s in range(L - 1):
                t_ = constp.tile([1, K], FD, tag=f"uni{s}")
                nc.sync.dma_start(t_[:], d_uni[s])
                uni_t.append(t_)

            # persistent big tiles
            featsT = bigp.tile([F, N], FD, tag="featsT")
            sq64 = bigp.tile([64, 512], FD, tag="sq64")
            vis01 = bigp.tile([64, 512], FD, tag="vis01")
            src_col = bigp.tile([K, 1], FD, tag="src_col")
            path_f = bigp.tile([K, L], FD, tag="path_f")
            pfT = bigp.tile([2 * H, K], FD, tag="pfT")

            fch_row = rowp.tile([1, N], FD, tag="row")

            # ---------------- phase 1: fchange + featsT (scoped pools) ----
            with (
                tc.tile_pool(name="nfs", bufs=4) as nfp,
                tc.tile_pool(name="dts", bufs=3) as dtp,
            ):
                acc = nfp.tile([128, NT], FD, tag="acc")
                nc.vector.memset(acc[:], 0.0)
                nf_view = d_nf.ap().rearrange("t (a p) f -> t p a f", p=128)
                # nf[T-1] (= feats) is also a standalone input: load it
                # up front so featsT/sq/z matmuls overlap the 12.5MB stream
                feats2d = nfp.tile([128, NT * F], FD, tag="feats2d")
                nc.gpsimd.dma_start(
                    feats2d[:], d_feats.ap().rearrange(
                        "(a p) f -> p a f", p=128))
                prev = nfp.tile([128, NT * F], FD, tag="nft")
                nc.gpsimd.dma_start(prev[:], nf_view[0])
                for t in range(1, T):
                    curt = nfp.tile([128, NT * F], FD, tag="nft")
                    nc.gpsimd.dma_start(curt[:], nf_view[t])
                    dtile = dtp.tile([128, NT * F], FD, tag="dtile")
                    nc.vector.tensor_tensor(
                        dtile[:], curt[:], prev[:], op=ALU.subtract)
                    r = smp.tile([128, NT], FD, tag="red")
                    nc.vector.tensor_reduce(
                        r[:], dtile[:].rearrange("p (a f) -> p a f", f=F),
                        axis=AX.X, op=ALU.add, apply_absolute_value=True,
                    )
                    nc.vector.tensor_tensor(acc[:], acc[:], r[:], op=ALU.add)
                    prev = curt
                # fchange = acc / 704 -> transpose -> row
                fchm = smp.tile([128, NT], FD, tag="fchm")
                nc.vector.tensor_scalar(
                    fchm[:], acc[:],
                    float(np.float32(1.0) / np.float32((T - 1) * F)), None,
                    op0=ALU.mult)
                fch_ps = psp.tile([NT, 128], FD, tag="mm")
                nc.tensor.transpose(fch_ps[:], fchm[:], ident[:])
                fch_t = smp.tile([NT, 128], FD, tag="fch_t")
                nc.scalar.copy(fch_t[:], fch_ps[:])
                nc.sync.dma_start(fch_row[:], fch_t[:])
                nc.sync.dma_start(d_fch[:], fch_row[:])

                # featsT from transposed 128x64 tiles of feats
                for a in range(NT):
                    tp = psp.tile([F, 128], FD, tag="mm")
                    nc.tensor.transpose(
                        tp[:], feats2d[:, a * F:(a + 1) * F], ident[:])
                    nc.scalar.copy(featsT[:, a * 128:(a + 1) * 128], tp[:])

            # ---------------- sq -> sq64 (64, 512) ----------------
            sq_row = rowp.tile([1, N], FD, tag="row")
            for c in range(NCHUNK):
                sl = slice(c * 512, (c + 1) * 512)
                sqc = smp.tile([F, 512], FD, tag="sqc")
                nc.scalar.square(sqc[:], featsT[:, sl])
                sq_ps = psp.tile([1, 512], FD, tag="mm")
                nc.tensor.matmul(sq_ps[:], ones_f8[:, 0:1], sqc[:],
                                 start=True, stop=True)
                nc.scalar.copy(sq_row[:, sl], sq_ps[:])
            sq_rs = smp.tile([8, 512], FD, tag="rs8")
            nc.sync.dma_start(sq_rs[:], sq_row[:])
            sq64_ps = psp.tile([64, 512], FD, tag="mm")
            nc.tensor.matmul(sq64_ps[:], selmat[:], sq_rs[:],
                             start=True, stop=True)
            nc.scalar.copy(sq64[:], sq64_ps[:])

            # ---------------- acc_bias -> vis01 = (1 + accb) ----------
            S = smp.tile([1, 1], FD, tag="S")
            nc.vector.tensor_reduce(
                S[:], fch_row[:], axis=AX.X, op=ALU.add)
            Sp = smp.tile([1, 1], FD, tag="Sp")
            nc.vector.tensor_scalar(Sp[:], S[:], 1e-8, None, op0=ALU.add)
            rSp = smp.tile([1, 1], FD, tag="rSp")
            nc.vector.reciprocal(rSp[:], Sp[:])
            accb_row = rowp.tile([1, N], FD, tag="row")
            nc.vector.tensor_scalar(
                accb_row[:], fch_row[:], rSp[:, 0:1], None, op0=ALU.mult)
            ac_rs = smp.tile([8, 512], FD, tag="rs8")
            nc.sync.dma_start(ac_rs[:], accb_row[:])
            ac64_ps = psp.tile([64, 512], FD, tag="mm")
            nc.tensor.matmul(ac64_ps[:], selmat[:], ac_rs[:],
                             start=True, stop=True)
            # vis01 starts as (1 + acc_bias); visited nodes zeroed in place
            nc.vector.tensor_scalar(
                vis01[:], ac64_ps[:], 1.0, None, op0=ALU.add)

            # ---------------- source logits z ----------------
            # raw logits don't need fchange -> computed during the stream
            zraw_row = rowp.tile([1, N], FD, tag="row")
            for c in range(NCHUNK):
                sl = slice(c * 512, (c + 1) * 512)
                h_ps = psp.tile([H, 512], FD, tag="mm")
                nc.tensor.matmul(h_ps[:], w1t[:], featsT[:, sl],
                                 start=True, stop=True)
                h_sb = smp.tile([H, 512], FD, tag="h_sb")
                nc.scalar.activation(
                    h_sb[:], h_ps[:], ACT.Relu, bias=b1[:, 0:1], scale=1.0)
                z_ps = psp.tile([1, 512], FD, tag="mm")
                nc.tensor.matmul(z_ps[:], w2t[:], h_sb[:],
                                 start=True, stop=True)
                nc.scalar.copy(zraw_row[:, sl], z_ps[:])
            z_row = rowp.tile([1, N], FD, tag="row")
            for c in range(NCHUNK):
                sl = slice(c * 512, (c + 1) * 512)
                nc.vector.scalar_tensor_tensor(
                    z_row[:, sl], zraw_row[:, sl], b2[:, 0:1],
                    fch_row[:, sl], op0=ALU.add, op1=ALU.mult)
            nc.sync.dma_start(d_z[:], z_row[:])

            # ---------------- top-8 sources ----------------
            zmax = smp.tile([1, 8], FD, tag="zmax")
            nc.vector.max(zmax[:], z_row[:])
            zidx = smp.tile([1, 8], U32, tag="zidx")
            nc.vector.max_index(zidx[:], zmax[:], z_row[:])
            zidx_f = smp.tile([1, 8], FD, tag="zidx_f")
            nc.vector.tensor_copy(zidx_f[:], zidx[:])
            nc.sync.dma_start(src_col[:], zidx_f[:])

            # ---------------- walk ----------------
            # layout: partition p = k*8 + s (walk k, chunk s); node
            # j = s*512 + f.  adj viewed as (4096*8, 512) row-chunks.
            # per-walk reductions go through a PE transpose to a (1, 64)
            # row, reduced in 8-groups along the free dim.
            adj_ck = d_adj.ap().rearrange("r (s c) -> (r s) c", c=512)

            def rep_walk_scalar(col8, tag):
                """(8,1) col -> (64,1) per-walk replication (repmat matmul)."""
                ps = psp.tile([64, 1], FD, tag="mm")
                nc.tensor.matmul(ps[:], repmat[:], col8[:],
                                 start=True, stop=True)
                sb = smp.tile([64, 1], FD, tag=tag)
                nc.scalar.copy(sb[:], ps[:])
                return sb

            def col_to_row(col, tag, n=8):
                """(n,1) -> (1,n) via PE transpose."""
                ps = psp.tile([1, n], FD, tag="mm")
                nc.tensor.transpose(ps[:], col[:, 0:1], ident[0:n, 0:n])
                sb = smp.tile([1, n], FD, tag=tag)
                nc.scalar.copy(sb[:], ps[:])
                return sb

            def row_to_col(row, tag):
                """(1,8) -> (8,1) via PE transpose."""
                ps = psp.tile([8, 1], FD, tag="mm")
                nc.tensor.transpose(ps[:], row[0:1, :], ident[0:1, 0:1])
                sb = smp.tile([8, 1], FD, tag=tag)
                nc.scalar.copy(sb[:], ps[:])
                return sb

            src_row = zidx_f        # (1, 8) source node ids as f32
            nc.vector.tensor_copy(path_f[:, 0:1], src_col[:])
            srcrep = rep_walk_scalar(src_col, "srcrep")
            srcadj = smp.tile([64, 1], FD, tag="srcadj")
            nc.vector.tensor_scalar(
                srcadj[:], srcrep[:], base512[:, 0:1], None,
                op0=ALU.subtract)
            nc.vector.scalar_tensor_tensor(
                vis01[:], iota512[:], srcadj[:, 0:1], vis01[:],
                op0=ALU.not_equal, op1=ALU.mult)

            cur_col = src_col
            xT = []
            with tc.tile_pool(name="wkp", bufs=3) as wkp:
                for pos in range(L):
                    cur_i = smp.tile([K, 1], I32, tag="cur_i")
                    nc.vector.tensor_copy(cur_i[:], cur_col[:])
                    frows = smp.tile([K, F], FD, tag="frows")
                    nc.gpsimd.indirect_dma_start(
                        out=frows[:], out_offset=None, in_=d_feats.ap(),
                        in_offset=bass.IndirectOffsetOnAxis(
                            ap=cur_i[:, 0:1], axis=0))
                    xt_ps = psp.tile([F, K], FD, tag="mm")
                    nc.tensor.transpose(xt_ps[:], frows[:], ident[0:K, 0:K])
                    xt = bigp.tile([F, K], FD, tag=f"xT{pos}")
                    nc.scalar.copy(xt[:], xt_ps[:])
                    xT.append(xt)
                    if pos == L - 1:
                        break
                    i = pos
                    # adj row-chunk gather: idx64 = cur*8 + s
                    currep = rep_walk_scalar(cur_col, "currep")
                    idx64f = smp.tile([64, 1], FD, tag="idx64f")
                    nc.vector.tensor_scalar(
                        idx64f[:], currep[:], 8.0, schunk[:, 0:1],
                        op0=ALU.mult, op1=ALU.add)
                    idx64 = smp.tile([64, 1], I32, tag="idx64")
                    nc.vector.tensor_copy(idx64[:], idx64f[:])
                    adjrows = wkp.tile([64, 512], FD, tag="adjrows")
                    nc.gpsimd.indirect_dma_start(
                        out=adjrows[:], out_offset=None, in_=adj_ck,
                        in_offset=bass.IndirectOffsetOnAxis(
                            ap=idx64[:, 0:1], axis=0))
                    eg_s = wkp.tile([64, 512], FD, tag="eg")
                    nc.sync.dma_start(eg_s[:], d_eg[i])
                    fr2 = smp.tile([K, F], FD, tag="fr2")
                    nc.scalar.square(fr2[:], frows[:])
                    sqcur = smp.tile([K, 1], FD, tag="sqcur")
                    nc.vector.tensor_reduce(
                        sqcur[:], fr2[:], axis=AX.X, op=ALU.add)
                    sqcur_r = rep_walk_scalar(sqcur, "sqcur_r")
                    # dots: 8 chunk matmuls -> (8, 4096) sbuf -> DMA
                    # reshape into the (64, 512) walk layout
                    dots8 = wkp.tile([K, N], FD, tag="dots8")
                    for hf in range(8):
                        dots_ps = psh.tile([K, 512], FD, tag="mmh")
                        sl_f = slice(hf * 512, (hf + 1) * 512)
                        nc.tensor.matmul(
                            dots_ps[:], xT[pos][:], featsT[:, sl_f],
                            start=True, stop=True)
                        nc.scalar.copy(dots8[:, sl_f], dots_ps[:])
                    d2 = wkp.tile([64, 512], FD, tag="d2")
                    nc.sync.dma_start(
                        d2[:],
                        dots8[:].rearrange("k (s c) -> (k s) c", c=512))
                    nc.vector.scalar_tensor_tensor(
                        d2[:], d2[:], -2.0, sq64[:],
                        op0=ALU.mult, op1=ALU.add)
                    nc.vector.tensor_scalar(
                        d2[:], d2[:], sqcur_r[:, 0:1], 0.0,
                        op0=ALU.add, op1=ALU.max)
                    m2p = smp.tile([64, 1], FD, tag="m2p")
                    nc.vector.tensor_reduce(
                        m2p[:], d2[:], axis=AX.X, op=ALU.max)
                    # sqrt(d2) starts immediately; the per-walk max
                    # round-trip runs concurrently and folds in as rsqrt
                    m2row = col_to_row(m2p, "m2row", n=64)
                    m2w = smp.tile([1, 8], FD, tag="m2w")
                    nc.vector.tensor_reduce(
                        m2w[:], m2row[:].rearrange("p (a b) -> p a b", b=8),
                        axis=AX.X, op=ALU.max)
                    sm2w = smp.tile([1, 8], FD, tag="sm2w")
                    nc.scalar.sqrt(sm2w[:], m2w[:])
                    rm2w = smp.tile([1, 8], FD, tag="rm2w")
                    nc.vector.reciprocal(rm2w[:], sm2w[:])
                    rm2c = row_to_col(rm2w, "rm2c")
                    rsm_r = rep_walk_scalar(rm2c, "rsm_r")
                    nc.scalar.sqrt(d2[:], d2[:])
                    # d2 <- sqrt(d2)*rsqrt(m2) + 1, then * eg
                    nc.vector.tensor_scalar(
                        d2[:], d2[:], rsm_r[:, 0:1], 1.0,
                        op0=ALU.mult, op1=ALU.add)
                    nc.vector.tensor_tensor(
                        d2[:], d2[:], eg_s[:], op=ALU.mult)
                    # y (in adjrows): (adj * (vis*accb1)) * ((guid+1) * eg)
                    nc.vector.tensor_tensor(
                        adjrows[:], adjrows[:], vis01[:], op=ALU.mult)
                    nc.vector.tensor_tensor(
                        adjrows[:], adjrows[:], d2[:], op=ALU.mult)
                    # per-partition top-1 then global per-walk argmax with
                    # first-index tie-break via eq-mask + min-index
                    y8 = smp.tile([64, 8], FD, tag="y8")
                    nc.vector.max(y8[:], adjrows[:])
                    yidx = smp.tile([64, 8], U32, tag="yidx")
                    nc.vector.max_index(yidx[:], y8[:], adjrows[:])
                    jf = smp.tile([64, 1], FD, tag="jf")
                    nc.vector.tensor_copy(jf[:], yidx[:, 0:1])
                    nc.vector.tensor_scalar(
                        jf[:], jf[:], base512[:, 0:1], None, op0=ALU.add)
                    yvrow = col_to_row(y8[:, 0:1], "yvrow", n=64)
                    jfrow = col_to_row(jf, "jfrow", n=64)
                    maxv = smp.tile([1, 8], FD, tag="maxv")
                    nc.vector.tensor_reduce(
                        maxv[:],
                        yvrow[:].rearrange("p (a b) -> p a b", b=8),
                        axis=AX.X, op=ALU.max)
                    eqm = smp.tile([1, 64], FD, tag="eqm")
                    nc.vector.tensor_tensor(
                        eqm[:].rearrange("p (a b) -> p a b", b=8),
                        yvrow[:].rearrange("p (a b) -> p a b", b=8),
                        maxv[:].unsqueeze(2).to_broadcast([1, 8, 8]),
                        op=ALU.is_equal)
                    cand = smp.tile([1, 64], FD, tag="cand")
                    nc.vector.scalar_tensor_tensor(
                        cand[:], eqm[:], -65536.0, jfrow[:],
                        op0=ALU.mult, op1=ALU.add)
                    nc.vector.tensor_scalar(
                        cand[:], cand[:], 65536.0, None, op0=ALU.add)
                    samp_row = smp.tile([1, 8], FD, tag="samp_row")
                    nc.vector.tensor_reduce(
                        samp_row[:],
                        cand[:].rearrange("p (a b) -> p a b", b=8),
                        axis=AX.X, op=ALU.min)
                    # teleport flags are input-independent RNG ->
                    # precomputed on host (uni holds 0/1 flags)
                    tel = uni_t[i]
                    dsrc = smp.tile([1, 8], FD, tag="dsrc")
                    nc.vector.tensor_tensor(
                        dsrc[:], src_row[:], samp_row[:], op=ALU.subtract)
                    td = smp.tile([1, 8], FD, tag="td")
                    nc.vector.tensor_tensor(
                        td[:], tel[:], dsrc[:], op=ALU.mult)
                    nxt_row = smp.tile([1, 8], FD, tag=f"nxtr{i}")
                    nc.vector.tensor_tensor(
                        nxt_row[:], td[:], samp_row[:], op=ALU.add)
                    nxt_col = row_to_col(nxt_row, f"nxtc{i}")
                    nc.vector.tensor_copy(path_f[:, i + 1:i + 2], nxt_col[:])
                    nxtrep = rep_walk_scalar(nxt_col, "nxtrep")
                    nxtadj = smp.tile([64, 1], FD, tag="nxtadj")
                    nc.vector.tensor_scalar(
                        nxtadj[:], nxtrep[:], base512[:, 0:1], None,
                        op0=ALU.subtract)
                    nc.vector.scalar_tensor_tensor(
                        vis01[:], iota512[:], nxtadj[:, 0:1], vis01[:],
                        op0=ALU.not_equal, op1=ALU.mult)
                    cur_col = nxt_col

            paths_i = smp.tile([K, L], I32, tag="paths_i")
            nc.vector.tensor_copy(paths_i[:], path_f[:])
            nc.sync.dma_start(d_paths[:], paths_i[:])

            # ---------------- GRU ----------------
            def gru_cell(h, xt, pre, hp):
                ps_i_rz = psp.tile([2 * H, K], FD, tag="mm")
                nc.tensor.matmul(ps_i_rz[:], gru[f"{pre}_ih_rzT"][:], xt[:],
                                 start=True, stop=True)
                ps_h_rz = psp.tile([2 * H, K], FD, tag="mm")
                nc.tensor.matmul(ps_h_rz[:], gru[f"{pre}_hh_rzT"][:], h[:],
                                 start=True, stop=True)
                gh_rz = smp.tile([2 * H, K], FD, tag="gh_rz")
                nc.vector.tensor_scalar(
                    gh_rz[:], ps_h_rz[:], gru[f"{pre}_bhh_rz"][:, 0:1], None,
                    op0=ALU.add)
                s_rz = smp.tile([2 * H, K], FD, tag="s_rz")
                nc.vector.scalar_tensor_tensor(
                    s_rz[:], ps_i_rz[:], gru[f"{pre}_bih_rz"][:, 0:1],
                    gh_rz[:], op0=ALU.add, op1=ALU.add)
                rz = smp.tile([2 * H, K], FD, tag="rz")
                nc.scalar.activation(rz[:], s_rz[:], ACT.Sigmoid)
                ps_i_n = psp.tile([H, K], FD, tag="mm")
                nc.tensor.matmul(ps_i_n[:], gru[f"{pre}_ih_nT"][:], xt[:],
                                 start=True, stop=True)
                ps_h_n = psp.tile([H, K], FD, tag="mm")
                nc.tensor.matmul(ps_h_n[:], gru[f"{pre}_hh_nT"][:], h[:],
                                 start=True, stop=True)
                inn = smp.tile([H, K], FD, tag="inn")
                nc.vector.tensor_scalar(
                    inn[:], ps_i_n[:], gru[f"{pre}_bih_n"][:, 0:1], None,
                    op0=ALU.add)
                hn = smp.tile([H, K], FD, tag="hn")
                nc.vector.tensor_scalar(
                    hn[:], ps_h_n[:], gru[f"{pre}_bhh_n"][:, 0:1], None,
                    op0=ALU.add)
                rh = smp.tile([H, K], FD, tag="rh")
                nc.vector.tensor_tensor(
                    rh[:], rz[0:H, :], hn[:], op=ALU.mult)
                npre = smp.tile([H, K], FD, tag="npre")
                nc.vector.tensor_tensor(npre[:], inn[:], rh[:], op=ALU.add)
                ng = smp.tile([H, K], FD, tag="ng")
                nc.scalar.activation(ng[:], npre[:], ACT.Tanh)
                zt = smp.tile([H, K], FD, tag="zt")
                nc.scalar.copy(zt[:], rz[H:2 * H, :])
                omz = smp.tile([H, K], FD, tag="omz")
                nc.vector.tensor_scalar(
                    omz[:], zt[:], -1.0, 1.0, op0=ALU.mult, op1=ALU.add)
                tb = smp.tile([H, K], FD, tag="tb")
                nc.vector.tensor_tensor(tb[:], omz[:], ng[:], op=ALU.mult)
                tcz = smp.tile([H, K], FD, tag="tc2")
                nc.vector.tensor_tensor(tcz[:], zt[:], h[:], op=ALU.mult)
                h2 = smp.tile([H, K], FD, tag=f"h_{pre}{hp}")
                nc.vector.tensor_tensor(h2[:], tb[:], tcz[:], op=ALU.add)
                return h2

            h_f = smp.tile([H, K], FD, tag="h_f0")
            nc.vector.memset(h_f[:], 0.0)
            for t in range(L):
                h_f = gru_cell(h_f, xT[t], "f", t + 1)
            h_b0 = smp.tile([H, K], FD, tag="h_b0")
            nc.vector.memset(h_b0[:], 0.0)
            h_b = gru_cell(h_b0, xT[L - 1], "b", 1)

            nc.vector.tensor_copy(pfT[0:H, :], h_f[:])
            nc.vector.tensor_copy(pfT[H:2 * H, :], h_b[:])
            nc.sync.dma_start(d_pfT[:], pfT[:])

            # ---------------- scorer ----------------
            ps1 = psp.tile([H, K], FD, tag="mm")
            nc.tensor.matmul(ps1[:], wps1t[:], pfT[:], start=True, stop=True)
            hs = smp.tile([H, K], FD, tag="hs")
            nc.scalar.activation(
                hs[:], ps1[:], ACT.Relu, bias=bps1[:, 0:1], scale=1.0)
            ps2 = psp.tile([1, K], FD, tag="mm")
            nc.tensor.matmul(ps2[:], wps2t[:], hs[:], start=True, stop=True)
            scores = smp.tile([1, K], FD, tag="scores")
            nc.scalar.activation(
                scores[:], ps2[:], ACT.Sigmoid, bias=bps2[:, 0:1], scale=1.0)
            nc.sync.dma_start(d_scores[:], scores[:])

    return nc


# --------------------------------------------------------------------------
# Host orchestration
# --------------------------------------------------------------------------

def _per_core_inputs(inputs, b, eg, uni):
    f32 = np.float32
    def c(x):
        return np.ascontiguousarray(np.asarray(x, dtype=f32))
    nf = c(inputs["node_features"][b])
    m = {
        "nf": nf,
        "feats": c(nf[T - 1]),
        "adj": c(inputs["adj_matrix"][b]),
        "w1t": c(inputs["W_sp1"].T),
        "b1": c(inputs["b_sp1"]).reshape(H, 1),
        "w2t": c(inputs["W_sp2"].T),
        "b2": c(inputs["b_sp2"]).reshape(1, 1),
        "wps1t": c(inputs["W_ps1"].T),
        "bps1": c(inputs["b_ps1"]).reshape(H, 1),
        "wps2t": c(inputs["W_ps2"].T),
        "bps2": c(inputs["b_ps2"]).reshape(1, 1),
        "eg": c(np.swapaxes(eg[b], 0, 1)).reshape(L - 1, 64, 512),
        "uni": c(np.swapaxes(uni[b], 0, 1) < np.float32(ALPHA))[:, None, :],
        "ident": np.eye(128, dtype=f32),
        "selmat": (np.arange(64)[None, :] % 8 == np.arange(8)[:, None]
                   ).astype(f32),
        "repmat": (np.arange(64)[None, :] // 8 == np.arange(8)[:, None]
                   ).astype(f32),
        "base512": ((np.arange(64) % 8) * 512).astype(f32).reshape(64, 1),
        "schunk": (np.arange(64) % 8).astype(f32).reshape(64, 1),
    }
    for pre, wih, whh, bih, bhh in (
        ("f", "Wih_f", "Whh_f", "bih_f", "bhh_f"),
        ("b", "Wih_b", "Whh_b", "bih_b", "bhh_b"),
    ):
        Wih, Whh = inputs[wih], inputs[whh]
        Bih, Bhh = inputs[bih], inputs[bhh]
        m[f"{pre}_ih_rzT"] = c(Wih[0:2 * H].T)
        m[f"{pre}_ih_nT"] = c(Wih[2 * H:3 * H].T)
        m[f"{pre}_hh_rzT"] = c(Whh[0:2 * H].T)
        m[f"{pre}_hh_nT"] = c(Whh[2 * H:3 * H].T)
        m[f"{pre}_bih_rz"] = c(Bih[0:2 * H]).reshape(2 * H, 1)
        m[f"{pre}_bih_n"] = c(Bih[2 * H:3 * H]).reshape(H, 1)
        m[f"{pre}_bhh_rz"] = c(Bhh[0:2 * H]).reshape(2 * H, 1)
        m[f"{pre}_bhh_n"] = c(Bhh[2 * H:3 * H]).reshape(H, 1)
    return m


_NC_CACHE = None
LAST_EXEC_NS = None


def _tunnel_ok(timeout=20.0):
    """Quick health probe of the axon relay before touching PJRT (a dead
    tunnel makes backend init hang indefinitely). Native-device setups
    (no relay env) skip the probe."""
    if not os.environ.get("TRN_TERMINAL_POOL_IPS"):
        return True
    import http.client
    try:
        conn = http.client.HTTPConnection("127.0.0.1", 8083, timeout=timeout)
        conn.request(
            "GET", "/init?rank=4294967295&topology=trn2.8x1&n_slices=1")
        resp = conn.getresponse()
        resp.read()
        conn.close()
        return resp.status == 200
    except Exception:
        return False


def _run_sim(nc, in_maps):
    """CoreSim fallback/debug path (KERNEL_BASS_SIM=1)."""
    from concourse import bass_interp
    outs = []
    for m in in_maps:
        sim = bass_interp.CoreSim(nc)
        for name, val in m.items():
            sim.tensor(name)[:] = val
        sim.simulate()
        outs.append({nm: np.array(sim.tensor(nm))
                     for nm in ("paths_o", "pfT_o", "scores_o")})
    return outs


def kernel(**inputs):
    global _NC_CACHE
    eg, uni = _host_rng()
    in_maps = [_per_core_inputs(inputs, b, eg, uni) for b in range(B)]
    global LAST_EXEC_NS
    use_sim = os.environ.get("KERNEL_BASS_SIM") == "1" or not _tunnel_ok()
    if not use_sim:
        try:
            if _NC_CACHE is None:
                _NC_CACHE = build_nc(num_devices=B)
            trace = os.environ.get("KERNEL_BASS_TRACE") == "1"
            res = run_bass_kernel_spmd(
                _NC_CACHE, in_maps, core_ids=list(range(B)), trace=trace)
            results = res.results
            if res.exec_time_ns is not None:
                LAST_EXEC_NS = res.exec_time_ns
        except Exception as e:
            import traceback
            traceback.print_exc()
            print(f"kernel: device path failed ({type(e).__name__}); "
                  "falling back to CoreSim", flush=True)
            use_sim = True
    if use_sim:
        nc1 = build_nc(num_devices=1)
        results = _run_sim(nc1, in_maps)
    paths = np.stack([results[b]["paths_o"] for b in range(B)])
    pf = np.stack([results[b]["pfT_o"].T for b in range(B)])
    scores = np.stack([results[b]["scores_o"][0] for b in range(B)])
    paths = paths.astype(np.int32)
    pf = pf.astype(np.float32)
    # weights = softmax over K of sigmoid scores
    sc = scores.astype(np.float32)[..., None]          # (B, K, 1)
    e = np.exp(sc - sc.max(axis=1, keepdims=True))
    weights = (e / e.sum(axis=1, keepdims=True)).astype(np.float32)
    return paths, weights, pf


# revision 38
# speedup vs baseline: 1.2772x; 1.1352x over previous
"""PathDiscovery Trainium2 Bass kernel.

Full pipeline on-device, data-parallel over batch (1 batch element per
NeuronCore, 8 cores):
  - fchange: mean |nf[t+1]-nf[t]| over (t, F)
  - source MLP -> z = logits * fchange -> top-8 (max8/max_index)
  - guided random walk with restart, 8 walks x 4 steps, vectorized over
    walks on partitions; the categorical sample is reproduced exactly as
    argmax_j probs_j * exp(gumbel_j) with host-precomputed (input
    independent) gumbel noise; adj/feature rows fetched by indirect DMA
  - bidirectional GRU encoding of gathered path features + path scorer

Host side: shard inputs over cores, precompute exp(gumbel)/uniform draws
from jax CPU threefry (deterministic, input-independent), pre-transpose
small weights, assemble outputs, softmax over K for weights.
"""

import os
import subprocess
import sys
import tempfile

import numpy as np

import concourse.bass as bass
import concourse.bacc as bacc
import concourse.mybir as mybir
from concourse import tile
from concourse.bass_utils import run_bass_kernel_spmd

B, T, N, F = 8, 12, 4096, 64
H = 64
K = 8
L = 5
ALPHA = 0.15
NT = N // 128          # 32 node tiles of 128
NCHUNK = N // 512      # 8 matmul chunks of 512
NHALF = N // 2
FD = mybir.dt.float32
I32 = mybir.dt.int32
U32 = mybir.dt.uint32
ALU = mybir.AluOpType
ACT = mybir.ActivationFunctionType
AX = mybir.AxisListType


# --------------------------------------------------------------------------
# Host-side RNG precompute (input independent; must match jax CPU threefry)
# --------------------------------------------------------------------------

_RNG_CACHE = None

_RNG_SCRIPT = r"""
import numpy as np
import jax, jax.numpy as jnp
B, K, L, N = 8, 8, 5, 4096
wkeys = jax.random.split(jax.random.key(42), B * K).reshape(B, K)
GUM = np.zeros((B, K, L - 1, N), np.float32)
UNI = np.zeros((B, K, L - 1), np.float32)
for b in range(B):
    for k in range(K):
        step_keys = jax.random.split(wkeys[b, k], L - 1)
        for i in range(L - 1):
            ku, kc = jax.random.split(step_keys[i])
            GUM[b, k, i] = np.asarray(jax.random.gumbel(kc, (N,), jnp.float32))
            UNI[b, k, i] = float(jax.random.uniform(ku))
np.savez(OUT_PATH, gum=GUM, uni=UNI)
"""


def _host_rng():
    """exp(gumbel) (B,K,4,N) f32 and uniforms (B,K,4) f32, via jax CPU."""
    global _RNG_CACHE
    if _RNG_CACHE is not None:
        return _RNG_CACHE
    # jax in this process may be bound to the axon backend; compute in a
    # clean subprocess pinned to the CPU backend so the threefry bits match
    # the reference implementation exactly.
    import jax  # just to locate the installed jax for the subprocess

    site_dir = os.path.dirname(os.path.dirname(jax.__file__))
    env = dict(os.environ)
    env.pop("TRN_TERMINAL_POOL_IPS", None)  # disables axon sitecustomize boot
    env["JAX_PLATFORMS"] = "cpu"
    env["PYTHONPATH"] = site_dir + os.pathsep + env.get("PYTHONPATH", "")
    with tempfile.TemporaryDirectory() as td:
        out_path = os.path.join(td, "rng.npz")
        script = f"OUT_PATH = {out_path!r}\n" + _RNG_SCRIPT
        subprocess.run(
            [sys.executable, "-c", script], env=env, check=True,
            capture_output=True,
        )
        dat = np.load(out_path)
        gum, uni = dat["gum"], dat["uni"]
    eg = np.exp(gum.astype(np.float64)).astype(np.float32)
    _RNG_CACHE = (eg, uni)
    return _RNG_CACHE


# --------------------------------------------------------------------------
# Bass kernel builder
# --------------------------------------------------------------------------

def build_nc(num_devices=1, debug=False):
    nc = bacc.Bacc(
        "TRN2",
        target_bir_lowering=False,
        debug=debug,
        num_devices=num_devices,
    )

    d_nf = nc.dram_tensor("nf", [T, N, F], FD, kind="ExternalInput")
    d_feats = nc.dram_tensor("feats", [N, F], FD, kind="ExternalInput")
    d_adj = nc.dram_tensor("adj", [N, N], FD, kind="ExternalInput")
    d_w1t = nc.dram_tensor("w1t", [F, H], FD, kind="ExternalInput")
    d_b1 = nc.dram_tensor("b1", [H, 1], FD, kind="ExternalInput")
    d_w2t = nc.dram_tensor("w2t", [H, 1], FD, kind="ExternalInput")
    d_b2 = nc.dram_tensor("b2", [1, 1], FD, kind="ExternalInput")
    gru_dram = {}
    for pre in ("f", "b"):
        for nm, shape in (
            ("ih_rzT", [F, 2 * H]), ("ih_nT", [F, H]),
            ("hh_rzT", [H, 2 * H]), ("hh_nT", [H, H]),
            ("bih_rz", [2 * H, 1]), ("bih_n", [H, 1]),
            ("bhh_rz", [2 * H, 1]), ("bhh_n", [H, 1]),
        ):
            gru_dram[f"{pre}_{nm}"] = nc.dram_tensor(
                f"{pre}_{nm}", shape, FD, kind="ExternalInput")
    d_wps1t = nc.dram_tensor("wps1t", [2 * H, H], FD, kind="ExternalInput")
    d_bps1 = nc.dram_tensor("bps1", [H, 1], FD, kind="ExternalInput")
    d_wps2t = nc.dram_tensor("wps2t", [H, 1], FD, kind="ExternalInput")
    d_bps2 = nc.dram_tensor("bps2", [1, 1], FD, kind="ExternalInput")
    d_eg = nc.dram_tensor("eg", [L - 1, 64, 512], FD, kind="ExternalInput")
    d_uni = nc.dram_tensor("uni", [L - 1, 1, K], FD, kind="ExternalInput")
    d_ident = nc.dram_tensor("ident", [128, 128], FD, kind="ExternalInput")
    d_selmat = nc.dram_tensor("selmat", [8, 64], FD, kind="ExternalInput")
    d_repmat = nc.dram_tensor("repmat", [8, 64], FD, kind="ExternalInput")
    d_base512 = nc.dram_tensor("base512", [64, 1], FD, kind="ExternalInput")
    d_schunk = nc.dram_tensor("schunk", [64, 1], FD, kind="ExternalInput")

    d_paths = nc.dram_tensor("paths_o", [K, L], I32, kind="ExternalOutput")
    d_pfT = nc.dram_tensor("pfT_o", [2 * H, K], FD, kind="ExternalOutput")
    d_scores = nc.dram_tensor("scores_o", [1, K], FD, kind="ExternalOutput")
    d_fch = nc.dram_tensor("fch_o", [1, N], FD, kind="ExternalOutput")
    d_z = nc.dram_tensor("z_o", [1, N], FD, kind="ExternalOutput")

    with tile.TileContext(nc) as tc:
        with (
            tc.tile_pool(name="const", bufs=1) as constp,
            tc.tile_pool(name="big", bufs=1) as bigp,
            tc.tile_pool(name="row", bufs=3) as rowp,
            tc.tile_pool(name="small", bufs=3) as smp,
            tc.tile_pool(name="ps", bufs=3, space="PSUM") as psp,
            tc.tile_pool(name="psh", bufs=4, space="PSUM") as psh,
        ):
            # ---------------- constants ----------------
            ident = constp.tile([128, 128], FD, tag="ident")
            nc.sync.dma_start(ident[:], d_ident[:])
            w1t = constp.tile([F, H], FD, tag="w1t")
            nc.sync.dma_start(w1t[:], d_w1t[:])
            b1 = constp.tile([H, 1], FD, tag="b1")
            nc.sync.dma_start(b1[:], d_b1[:])
            w2t = constp.tile([H, 1], FD, tag="w2t")
            nc.sync.dma_start(w2t[:], d_w2t[:])
            b2 = constp.tile([1, 1], FD, tag="b2")
            nc.sync.dma_start(b2[:], d_b2[:])
            gru = {}
            for nm, dtr in gru_dram.items():
                t_ = constp.tile(list(dtr.shape), FD, tag=nm)
                nc.sync.dma_start(t_[:], dtr[:])
                gru[nm] = t_
            wps1t = constp.tile([2 * H, H], FD, tag="wps1t")
            nc.sync.dma_start(wps1t[:], d_wps1t[:])
            bps1 = constp.tile([H, 1], FD, tag="bps1")
            nc.sync.dma_start(bps1[:], d_bps1[:])
            wps2t = constp.tile([H, 1], FD, tag="wps2t")
            nc.sync.dma_start(wps2t[:], d_wps2t[:])
            bps2 = constp.tile([1, 1], FD, tag="bps2")
            nc.sync.dma_start(bps2[:], d_bps2[:])

            ones_f8 = constp.tile([F, K], FD, tag="ones_f8")
            nc.vector.memset(ones_f8[:], 1.0)
            selmat = constp.tile([8, 64], FD, tag="selmat")
            nc.sync.dma_start(selmat[:], d_selmat[:])
            repmat = constp.tile([8, 64], FD, tag="repmat")
            nc.sync.dma_start(repmat[:], d_repmat[:])
            base512 = constp.tile([64, 1], FD, tag="base512")
            nc.sync.dma_start(base512[:], d_base512[:])
            schunk = constp.tile([64, 1], FD, tag="schunk")
            nc.sync.dma_start(schunk[:], d_schunk[:])

            iota512 = constp.tile([64, 512], FD, tag="iota512")
            nc.gpsimd.iota(
                iota512[:], pattern=[[1, 512]], base=0, channel_multiplier=0,
                allow_small_or_imprecise_dtypes=True,
            )

            uni_t = []
            for s in range(L - 1):
                t_ = constp.tile([1, K], FD, tag=f"uni{s}")
                nc.sync.dma_start(t_[:], d_uni[s])
                uni_t.append(t_)

            # persistent big tiles
            featsT = bigp.tile([F, N], FD, tag="featsT")
            sq64 = bigp.tile([64, 512], FD, tag="sq64")
            vis01 = bigp.tile([64, 512], FD, tag="vis01")
            src_col = bigp.tile([K, 1], FD, tag="src_col")
            path_f = bigp.tile([K, L], FD, tag="path_f")
            pfT = bigp.tile([2 * H, K], FD, tag="pfT")

            fch_row = rowp.tile([1, N], FD, tag="row")

            # ---------------- phase 1: fchange + featsT (scoped pools) ----
            with (
                tc.tile_pool(name="nfs", bufs=4) as nfp,
                tc.tile_pool(name="dts", bufs=3) as dtp,
            ):
                acc = nfp.tile([128, NT], FD, tag="acc")
                nc.vector.memset(acc[:], 0.0)
                # node j = p*32 + a: 8KB contiguous per partition per DMA
                nf_view = d_nf.ap().rearrange("t (p a) f -> t p a f", p=128)
                # nf[T-1] (= feats) is also a standalone input: load it
                # up front so featsT/sq/z matmuls overlap the 12.5MB stream
                feats2d = nfp.tile([128, NT * F], FD, tag="feats2d")
                nc.gpsimd.dma_start(
                    feats2d[:], d_feats.ap().rearrange(
                        "(a p) f -> p a f", p=128))
                prev = nfp.tile([128, NT * F], FD, tag="nft")
                nc.gpsimd.dma_start(prev[:], nf_view[0])
                for t in range(1, T):
                    curt = nfp.tile([128, NT * F], FD, tag="nft")
                    nc.gpsimd.dma_start(curt[:], nf_view[t])
                    dtile = dtp.tile([128, NT * F], FD, tag="dtile")
                    nc.vector.tensor_tensor(
                        dtile[:], curt[:], prev[:], op=ALU.subtract)
                    r = smp.tile([128, NT], FD, tag="red")
                    nc.vector.tensor_reduce(
                        r[:], dtile[:].rearrange("p (a f) -> p a f", f=F),
                        axis=AX.X, op=ALU.add, apply_absolute_value=True,
                    )
                    nc.vector.tensor_tensor(acc[:], acc[:], r[:], op=ALU.add)
                    prev = curt
                # fchange = acc / 704 -> transpose -> row
                fchm = smp.tile([128, NT], FD, tag="fchm")
                nc.vector.tensor_scalar(
                    fchm[:], acc[:],
                    float(np.float32(1.0) / np.float32((T - 1) * F)), None,
                    op0=ALU.mult)
                nc.sync.dma_start(fch_row[:], fchm[:])
                nc.sync.dma_start(d_fch[:], fch_row[:])

                # featsT from transposed 128x64 tiles of feats
                for a in range(NT):
                    tp = psp.tile([F, 128], FD, tag="mm")
                    nc.tensor.transpose(
                        tp[:], feats2d[:, a * F:(a + 1) * F], ident[:])
                    nc.scalar.copy(featsT[:, a * 128:(a + 1) * 128], tp[:])

            # ---------------- sq -> sq64 (64, 512) ----------------
            sq_row = rowp.tile([1, N], FD, tag="row")
            for c in range(NCHUNK):
                sl = slice(c * 512, (c + 1) * 512)
                sqc = smp.tile([F, 512], FD, tag="sqc")
                nc.scalar.square(sqc[:], featsT[:, sl])
                sq_ps = psp.tile([1, 512], FD, tag="mm")
                nc.tensor.matmul(sq_ps[:], ones_f8[:, 0:1], sqc[:],
                                 start=True, stop=True)
                nc.scalar.copy(sq_row[:, sl], sq_ps[:])
            sq_rs = smp.tile([8, 512], FD, tag="rs8")
            nc.sync.dma_start(sq_rs[:], sq_row[:])
            sq64_ps = psp.tile([64, 512], FD, tag="mm")
            nc.tensor.matmul(sq64_ps[:], selmat[:], sq_rs[:],
                             start=True, stop=True)
            nc.scalar.copy(sq64[:], sq64_ps[:])

            # ---------------- acc_bias -> vis01 = (1 + accb) ----------
            S = smp.tile([1, 1], FD, tag="S")
            nc.vector.tensor_reduce(
                S[:], fch_row[:], axis=AX.X, op=ALU.add)
            Sp = smp.tile([1, 1], FD, tag="Sp")
            nc.vector.tensor_scalar(Sp[:], S[:], 1e-8, None, op0=ALU.add)
            rSp = smp.tile([1, 1], FD, tag="rSp")
            nc.vector.reciprocal(rSp[:], Sp[:])
            accb_row = rowp.tile([1, N], FD, tag="row")
            nc.vector.tensor_scalar(
                accb_row[:], fch_row[:], rSp[:, 0:1], None, op0=ALU.mult)
            ac_rs = smp.tile([8, 512], FD, tag="rs8")
            nc.sync.dma_start(ac_rs[:], accb_row[:])
            ac64_ps = psp.tile([64, 512], FD, tag="mm")
            nc.tensor.matmul(ac64_ps[:], selmat[:], ac_rs[:],
                             start=True, stop=True)
            # vis01 starts as (1 + acc_bias); visited nodes zeroed in place
            nc.vector.tensor_scalar(
                vis01[:], ac64_ps[:], 1.0, None, op0=ALU.add)

            # ---------------- source logits z ----------------
            # raw logits don't need fchange -> computed during the stream
            zraw_row = rowp.tile([1, N], FD, tag="row")
            for c in range(NCHUNK):
                sl = slice(c * 512, (c + 1) * 512)
                h_ps = psp.tile([H, 512], FD, tag="mm")
                nc.tensor.matmul(h_ps[:], w1t[:], featsT[:, sl],
                                 start=True, stop=True)
                h_sb = smp.tile([H, 512], FD, tag="h_sb")
                nc.scalar.activation(
                    h_sb[:], h_ps[:], ACT.Relu, bias=b1[:, 0:1], scale=1.0)
                z_ps = psp.tile([1, 512], FD, tag="mm")
                nc.tensor.matmul(z_ps[:], w2t[:], h_sb[:],
                                 start=True, stop=True)
                nc.scalar.copy(zraw_row[:, sl], z_ps[:])
            z_row = rowp.tile([1, N], FD, tag="row")
            for c in range(NCHUNK):
                sl = slice(c * 512, (c + 1) * 512)
                nc.vector.scalar_tensor_tensor(
                    z_row[:, sl], zraw_row[:, sl], b2[:, 0:1],
                    fch_row[:, sl], op0=ALU.add, op1=ALU.mult)
            nc.sync.dma_start(d_z[:], z_row[:])

            # ---------------- top-8 sources ----------------
            zmax = smp.tile([1, 8], FD, tag="zmax")
            nc.vector.max(zmax[:], z_row[:])
            zidx = smp.tile([1, 8], U32, tag="zidx")
            nc.vector.max_index(zidx[:], zmax[:], z_row[:])
            zidx_f = smp.tile([1, 8], FD, tag="zidx_f")
            nc.vector.tensor_copy(zidx_f[:], zidx[:])
            nc.sync.dma_start(src_col[:], zidx_f[:])

            # ---------------- walk ----------------
            # layout: partition p = k*8 + s (walk k, chunk s); node
            # j = s*512 + f.  adj viewed as (4096*8, 512) row-chunks.
            # per-walk reductions go through a PE transpose to a (1, 64)
            # row, reduced in 8-groups along the free dim.
            adj_ck = d_adj.ap().rearrange("r (s c) -> (r s) c", c=512)

            def rep_walk_scalar(col8, tag):
                """(8,1) col -> (64,1) per-walk replication (repmat matmul)."""
                ps = psp.tile([64, 1], FD, tag="mm")
                nc.tensor.matmul(ps[:], repmat[:], col8[:],
                                 start=True, stop=True)
                sb = smp.tile([64, 1], FD, tag=tag)
                nc.scalar.copy(sb[:], ps[:])
                return sb

            def col_to_row(col, tag, n=8):
                """(n,1) -> (1,n) via PE transpose."""
                ps = psp.tile([1, n], FD, tag="mm")
                nc.tensor.transpose(ps[:], col[:, 0:1], ident[0:n, 0:n])
                sb = smp.tile([1, n], FD, tag=tag)
                nc.scalar.copy(sb[:], ps[:])
                return sb

            def row_to_col(row, tag):
                """(1,8) -> (8,1) via PE transpose."""
                ps = psp.tile([8, 1], FD, tag="mm")
                nc.tensor.transpose(ps[:], row[0:1, :], ident[0:1, 0:1])
                sb = smp.tile([8, 1], FD, tag=tag)
                nc.scalar.copy(sb[:], ps[:])
                return sb

            src_row = zidx_f        # (1, 8) source node ids as f32
            nc.vector.tensor_copy(path_f[:, 0:1], src_col[:])
            srcrep = rep_walk_scalar(src_col, "srcrep")
            srcadj = smp.tile([64, 1], FD, tag="srcadj")
            nc.vector.tensor_scalar(
                srcadj[:], srcrep[:], base512[:, 0:1], None,
                op0=ALU.subtract)
            nc.vector.scalar_tensor_tensor(
                vis01[:], iota512[:], srcadj[:, 0:1], vis01[:],
                op0=ALU.not_equal, op1=ALU.mult)

            cur_col = src_col
            xT = []
            with tc.tile_pool(name="wkp", bufs=3) as wkp:
                for pos in range(L):
                    cur_i = smp.tile([K, 1], I32, tag="cur_i")
                    nc.vector.tensor_copy(cur_i[:], cur_col[:])
                    frows = smp.tile([K, F], FD, tag="frows")
                    nc.gpsimd.indirect_dma_start(
                        out=frows[:], out_offset=None, in_=d_feats.ap(),
                        in_offset=bass.IndirectOffsetOnAxis(
                            ap=cur_i[:, 0:1], axis=0))
                    xt_ps = psp.tile([F, K], FD, tag="mm")
                    nc.tensor.transpose(xt_ps[:], frows[:], ident[0:K, 0:K])
                    xt = bigp.tile([F, K], FD, tag=f"xT{pos}")
                    nc.scalar.copy(xt[:], xt_ps[:])
                    xT.append(xt)
                    if pos == L - 1:
                        break
                    i = pos
                    # adj row-chunk gather: idx64 = cur*8 + s
                    currep = rep_walk_scalar(cur_col, "currep")
                    idx64f = smp.tile([64, 1], FD, tag="idx64f")
                    nc.vector.tensor_scalar(
                        idx64f[:], currep[:], 8.0, schunk[:, 0:1],
                        op0=ALU.mult, op1=ALU.add)
                    idx64 = smp.tile([64, 1], I32, tag="idx64")
                    nc.vector.tensor_copy(idx64[:], idx64f[:])
                    adjrows = wkp.tile([64, 512], FD, tag="adjrows")
                    nc.gpsimd.indirect_dma_start(
                        out=adjrows[:], out_offset=None, in_=adj_ck,
                        in_offset=bass.IndirectOffsetOnAxis(
                            ap=idx64[:, 0:1], axis=0))
                    eg_s = wkp.tile([64, 512], FD, tag="eg")
                    nc.sync.dma_start(eg_s[:], d_eg[i])
                    fr2 = smp.tile([K, F], FD, tag="fr2")
                    nc.scalar.square(fr2[:], frows[:])
                    sqcur = smp.tile([K, 1], FD, tag="sqcur")
                    nc.vector.tensor_reduce(
                        sqcur[:], fr2[:], axis=AX.X, op=ALU.add)
                    sqcur_r = rep_walk_scalar(sqcur, "sqcur_r")
                    # dots: 8 chunk matmuls -> (8, 4096) sbuf -> DMA
                    # reshape into the (64, 512) walk layout
                    dots8 = wkp.tile([K, N], FD, tag="dots8")
                    for hf in range(8):
                        dots_ps = psh.tile([K, 512], FD, tag="mmh")
                        sl_f = slice(hf * 512, (hf + 1) * 512)
                        nc.tensor.matmul(
                            dots_ps[:], xT[pos][:], featsT[:, sl_f],
                            start=True, stop=True)
                        nc.scalar.copy(dots8[:, sl_f], dots_ps[:])
                    d2 = wkp.tile([64, 512], FD, tag="d2")
                    nc.sync.dma_start(
                        d2[:],
                        dots8[:].rearrange("k (s c) -> (k s) c", c=512))
                    nc.vector.scalar_tensor_tensor(
                        d2[:], d2[:], -2.0, sq64[:],
                        op0=ALU.mult, op1=ALU.add)
                    nc.vector.tensor_scalar(
                        d2[:], d2[:], sqcur_r[:, 0:1], 0.0,
                        op0=ALU.add, op1=ALU.max)
                    m2p = smp.tile([64, 1], FD, tag="m2p")
                    nc.vector.tensor_reduce(
                        m2p[:], d2[:], axis=AX.X, op=ALU.max)
                    # sqrt(d2) starts immediately; the per-walk max
                    # round-trip runs concurrently and folds in as rsqrt
                    m2row = col_to_row(m2p, "m2row", n=64)
                    m2w = smp.tile([1, 8], FD, tag="m2w")
                    nc.vector.tensor_reduce(
                        m2w[:], m2row[:].rearrange("p (a b) -> p a b", b=8),
                        axis=AX.X, op=ALU.max)
                    sm2w = smp.tile([1, 8], FD, tag="sm2w")
                    nc.scalar.sqrt(sm2w[:], m2w[:])
                    rm2w = smp.tile([1, 8], FD, tag="rm2w")
                    nc.vector.reciprocal(rm2w[:], sm2w[:])
                    rm2c = row_to_col(rm2w, "rm2c")
                    rsm_r = rep_walk_scalar(rm2c, "rsm_r")
                    nc.scalar.sqrt(d2[:], d2[:])
                    # d2 <- sqrt(d2)*rsqrt(m2) + 1, then * eg
                    nc.vector.tensor_scalar(
                        d2[:], d2[:], rsm_r[:, 0:1], 1.0,
                        op0=ALU.mult, op1=ALU.add)
                    nc.vector.tensor_tensor(
                        d2[:], d2[:], eg_s[:], op=ALU.mult)
                    # y (in adjrows): (adj * (vis*accb1)) * ((guid+1) * eg)
                    nc.vector.tensor_tensor(
                        adjrows[:], adjrows[:], vis01[:], op=ALU.mult)
                    nc.vector.tensor_tensor(
                        adjrows[:], adjrows[:], d2[:], op=ALU.mult)
                    # per-partition top-1 then global per-walk argmax with
                    # first-index tie-break via eq-mask + min-index
                    y8 = smp.tile([64, 8], FD, tag="y8")
                    nc.vector.max(y8[:], adjrows[:])
                    yidx = smp.tile([64, 8], U32, tag="yidx")
                    nc.vector.max_index(yidx[:], y8[:], adjrows[:])
                    jf = smp.tile([64, 1], FD, tag="jf")
                    nc.vector.tensor_copy(jf[:], yidx[:, 0:1])
                    nc.vector.tensor_scalar(
                        jf[:], jf[:], base512[:, 0:1], None, op0=ALU.add)
                    yvrow = col_to_row(y8[:, 0:1], "yvrow", n=64)
                    jfrow = col_to_row(jf, "jfrow", n=64)
                    maxv = smp.tile([1, 8], FD, tag="maxv")
                    nc.vector.tensor_reduce(
                        maxv[:],
                        yvrow[:].rearrange("p (a b) -> p a b", b=8),
                        axis=AX.X, op=ALU.max)
                    eqm = smp.tile([1, 64], FD, tag="eqm")
                    nc.vector.tensor_tensor(
                        eqm[:].rearrange("p (a b) -> p a b", b=8),
                        yvrow[:].rearrange("p (a b) -> p a b", b=8),
                        maxv[:].unsqueeze(2).to_broadcast([1, 8, 8]),
                        op=ALU.is_equal)
                    cand = smp.tile([1, 64], FD, tag="cand")
                    nc.vector.scalar_tensor_tensor(
                        cand[:], eqm[:], -65536.0, jfrow[:],
                        op0=ALU.mult, op1=ALU.add)
                    nc.vector.tensor_scalar(
                        cand[:], cand[:], 65536.0, None, op0=ALU.add)
                    samp_row = smp.tile([1, 8], FD, tag="samp_row")
                    nc.vector.tensor_reduce(
                        samp_row[:],
                        cand[:].rearrange("p (a b) -> p a b", b=8),
                        axis=AX.X, op=ALU.min)
                    # teleport flags are input-independent RNG ->
                    # precomputed on host (uni holds 0/1 flags)
                    tel = uni_t[i]
                    dsrc = smp.tile([1, 8], FD, tag="dsrc")
                    nc.vector.tensor_tensor(
                        dsrc[:], src_row[:], samp_row[:], op=ALU.subtract)
                    td = smp.tile([1, 8], FD, tag="td")
                    nc.vector.tensor_tensor(
                        td[:], tel[:], dsrc[:], op=ALU.mult)
                    nxt_row = smp.tile([1, 8], FD, tag=f"nxtr{i}")
                    nc.vector.tensor_tensor(
                        nxt_row[:], td[:], samp_row[:], op=ALU.add)
                    nxt_col = row_to_col(nxt_row, f"nxtc{i}")
                    nc.vector.tensor_copy(path_f[:, i + 1:i + 2], nxt_col[:])
                    nxtrep = rep_walk_scalar(nxt_col, "nxtrep")
                    nxtadj = smp.tile([64, 1], FD, tag="nxtadj")
                    nc.vector.tensor_scalar(
                        nxtadj[:], nxtrep[:], base512[:, 0:1], None,
                        op0=ALU.subtract)
                    nc.vector.scalar_tensor_tensor(
                        vis01[:], iota512[:], nxtadj[:, 0:1], vis01[:],
                        op0=ALU.not_equal, op1=ALU.mult)
                    cur_col = nxt_col

            paths_i = smp.tile([K, L], I32, tag="paths_i")
            nc.vector.tensor_copy(paths_i[:], path_f[:])
            nc.sync.dma_start(d_paths[:], paths_i[:])

            # ---------------- GRU ----------------
            def gru_cell(h, xt, pre, hp):
                ps_i_rz = psp.tile([2 * H, K], FD, tag="mm")
                nc.tensor.matmul(ps_i_rz[:], gru[f"{pre}_ih_rzT"][:], xt[:],
                                 start=True, stop=True)
                ps_h_rz = psp.tile([2 * H, K], FD, tag="mm")
                nc.tensor.matmul(ps_h_rz[:], gru[f"{pre}_hh_rzT"][:], h[:],
                                 start=True, stop=True)
                gh_rz = smp.tile([2 * H, K], FD, tag="gh_rz")
                nc.vector.tensor_scalar(
                    gh_rz[:], ps_h_rz[:], gru[f"{pre}_bhh_rz"][:, 0:1], None,
                    op0=ALU.add)
                s_rz = smp.tile([2 * H, K], FD, tag="s_rz")
                nc.vector.scalar_tensor_tensor(
                    s_rz[:], ps_i_rz[:], gru[f"{pre}_bih_rz"][:, 0:1],
                    gh_rz[:], op0=ALU.add, op1=ALU.add)
                rz = smp.tile([2 * H, K], FD, tag="rz")
                nc.scalar.activation(rz[:], s_rz[:], ACT.Sigmoid)
                ps_i_n = psp.tile([H, K], FD, tag="mm")
                nc.tensor.matmul(ps_i_n[:], gru[f"{pre}_ih_nT"][:], xt[:],
                                 start=True, stop=True)
                ps_h_n = psp.tile([H, K], FD, tag="mm")
                nc.tensor.matmul(ps_h_n[:], gru[f"{pre}_hh_nT"][:], h[:],
                                 start=True, stop=True)
                inn = smp.tile([H, K], FD, tag="inn")
                nc.vector.tensor_scalar(
                    inn[:], ps_i_n[:], gru[f"{pre}_bih_n"][:, 0:1], None,
                    op0=ALU.add)
                hn = smp.tile([H, K], FD, tag="hn")
                nc.vector.tensor_scalar(
                    hn[:], ps_h_n[:], gru[f"{pre}_bhh_n"][:, 0:1], None,
                    op0=ALU.add)
                rh = smp.tile([H, K], FD, tag="rh")
                nc.vector.tensor_tensor(
                    rh[:], rz[0:H, :], hn[:], op=ALU.mult)
                npre = smp.tile([H, K], FD, tag="npre")
                nc.vector.tensor_tensor(npre[:], inn[:], rh[:], op=ALU.add)
                ng = smp.tile([H, K], FD, tag="ng")
                nc.scalar.activation(ng[:], npre[:], ACT.Tanh)
                zt = smp.tile([H, K], FD, tag="zt")
                nc.scalar.copy(zt[:], rz[H:2 * H, :])
                omz = smp.tile([H, K], FD, tag="omz")
                nc.vector.tensor_scalar(
                    omz[:], zt[:], -1.0, 1.0, op0=ALU.mult, op1=ALU.add)
                tb = smp.tile([H, K], FD, tag="tb")
                nc.vector.tensor_tensor(tb[:], omz[:], ng[:], op=ALU.mult)
                tcz = smp.tile([H, K], FD, tag="tc2")
                nc.vector.tensor_tensor(tcz[:], zt[:], h[:], op=ALU.mult)
                h2 = smp.tile([H, K], FD, tag=f"h_{pre}{hp}")
                nc.vector.tensor_tensor(h2[:], tb[:], tcz[:], op=ALU.add)
                return h2

            h_f = smp.tile([H, K], FD, tag="h_f0")
            nc.vector.memset(h_f[:], 0.0)
            for t in range(L):
                h_f = gru_cell(h_f, xT[t], "f", t + 1)
            h_b0 = smp.tile([H, K], FD, tag="h_b0")
            nc.vector.memset(h_b0[:], 0.0)
            h_b = gru_cell(h_b0, xT[L - 1], "b", 1)

            nc.vector.tensor_copy(pfT[0:H, :], h_f[:])
            nc.vector.tensor_copy(pfT[H:2 * H, :], h_b[:])
            nc.sync.dma_start(d_pfT[:], pfT[:])

            # ---------------- scorer ----------------
            ps1 = psp.tile([H, K], FD, tag="mm")
            nc.tensor.matmul(ps1[:], wps1t[:], pfT[:], start=True, stop=True)
            hs = smp.tile([H, K], FD, tag="hs")
            nc.scalar.activation(
                hs[:], ps1[:], ACT.Relu, bias=bps1[:, 0:1], scale=1.0)
            ps2 = psp.tile([1, K], FD, tag="mm")
            nc.tensor.matmul(ps2[:], wps2t[:], hs[:], start=True, stop=True)
            scores = smp.tile([1, K], FD, tag="scores")
            nc.scalar.activation(
                scores[:], ps2[:], ACT.Sigmoid, bias=bps2[:, 0:1], scale=1.0)
            nc.sync.dma_start(d_scores[:], scores[:])

    return nc


# --------------------------------------------------------------------------
# Host orchestration
# --------------------------------------------------------------------------

def _per_core_inputs(inputs, b, eg, uni):
    f32 = np.float32
    def c(x):
        return np.ascontiguousarray(np.asarray(x, dtype=f32))
    nf = c(inputs["node_features"][b])
    m = {
        "nf": nf,
        "feats": c(nf[T - 1]),
        "adj": c(inputs["adj_matrix"][b]),
        "w1t": c(inputs["W_sp1"].T),
        "b1": c(inputs["b_sp1"]).reshape(H, 1),
        "w2t": c(inputs["W_sp2"].T),
        "b2": c(inputs["b_sp2"]).reshape(1, 1),
        "wps1t": c(inputs["W_ps1"].T),
        "bps1": c(inputs["b_ps1"]).reshape(H, 1),
        "wps2t": c(inputs["W_ps2"].T),
        "bps2": c(inputs["b_ps2"]).reshape(1, 1),
        "eg": c(np.swapaxes(eg[b], 0, 1)).reshape(L - 1, 64, 512),
        "uni": c(np.swapaxes(uni[b], 0, 1) < np.float32(ALPHA))[:, None, :],
        "ident": np.eye(128, dtype=f32),
        "selmat": (np.arange(64)[None, :] % 8 == np.arange(8)[:, None]
                   ).astype(f32),
        "repmat": (np.arange(64)[None, :] // 8 == np.arange(8)[:, None]
                   ).astype(f32),
        "base512": ((np.arange(64) % 8) * 512).astype(f32).reshape(64, 1),
        "schunk": (np.arange(64) % 8).astype(f32).reshape(64, 1),
    }
    for pre, wih, whh, bih, bhh in (
        ("f", "Wih_f", "Whh_f", "bih_f", "bhh_f"),
        ("b", "Wih_b", "Whh_b", "bih_b", "bhh_b"),
    ):
        Wih, Whh = inputs[wih], inputs[whh]
        Bih, Bhh = inputs[bih], inputs[bhh]
        m[f"{pre}_ih_rzT"] = c(Wih[0:2 * H].T)
        m[f"{pre}_ih_nT"] = c(Wih[2 * H:3 * H].T)
        m[f"{pre}_hh_rzT"] = c(Whh[0:2 * H].T)
        m[f"{pre}_hh_nT"] = c(Whh[2 * H:3 * H].T)
        m[f"{pre}_bih_rz"] = c(Bih[0:2 * H]).reshape(2 * H, 1)
        m[f"{pre}_bih_n"] = c(Bih[2 * H:3 * H]).reshape(H, 1)
        m[f"{pre}_bhh_rz"] = c(Bhh[0:2 * H]).reshape(2 * H, 1)
        m[f"{pre}_bhh_n"] = c(Bhh[2 * H:3 * H]).reshape(H, 1)
    return m


_NC_CACHE = None
LAST_EXEC_NS = None


def _tunnel_ok(timeout=20.0):
    """Quick health probe of the axon relay before touching PJRT (a dead
    tunnel makes backend init hang indefinitely). Native-device setups
    (no relay env) skip the probe."""
    if not os.environ.get("TRN_TERMINAL_POOL_IPS"):
        return True
    import http.client
    try:
        conn = http.client.HTTPConnection("127.0.0.1", 8083, timeout=timeout)
        conn.request(
            "GET", "/init?rank=4294967295&topology=trn2.8x1&n_slices=1")
        resp = conn.getresponse()
        resp.read()
        conn.close()
        return resp.status == 200
    except Exception:
        return False


def _run_sim(nc, in_maps):
    """CoreSim fallback/debug path (KERNEL_BASS_SIM=1)."""
    from concourse import bass_interp
    outs = []
    for m in in_maps:
        sim = bass_interp.CoreSim(nc)
        for name, val in m.items():
            sim.tensor(name)[:] = val
        sim.simulate()
        outs.append({nm: np.array(sim.tensor(nm))
                     for nm in ("paths_o", "pfT_o", "scores_o")})
    return outs


def kernel(**inputs):
    global _NC_CACHE
    eg, uni = _host_rng()
    in_maps = [_per_core_inputs(inputs, b, eg, uni) for b in range(B)]
    global LAST_EXEC_NS
    use_sim = os.environ.get("KERNEL_BASS_SIM") == "1" or not _tunnel_ok()
    if not use_sim:
        try:
            if _NC_CACHE is None:
                _NC_CACHE = build_nc(num_devices=B)
            trace = os.environ.get("KERNEL_BASS_TRACE") == "1"
            res = run_bass_kernel_spmd(
                _NC_CACHE, in_maps, core_ids=list(range(B)), trace=trace)
            results = res.results
            if res.exec_time_ns is not None:
                LAST_EXEC_NS = res.exec_time_ns
        except Exception as e:
            import traceback
            traceback.print_exc()
            print(f"kernel: device path failed ({type(e).__name__}); "
                  "falling back to CoreSim", flush=True)
            use_sim = True
    if use_sim:
        nc1 = build_nc(num_devices=1)
        results = _run_sim(nc1, in_maps)
    paths = np.stack([results[b]["paths_o"] for b in range(B)])
    pf = np.stack([results[b]["pfT_o"].T for b in range(B)])
    scores = np.stack([results[b]["scores_o"][0] for b in range(B)])
    paths = paths.astype(np.int32)
    pf = pf.astype(np.float32)
    # weights = softmax over K of sigmoid scores
    sc = scores.astype(np.float32)[..., None]          # (B, K, 1)
    e = np.exp(sc - sc.max(axis=1, keepdims=True))
    weights = (e / e.sum(axis=1, keepdims=True)).astype(np.float32)
    return paths, weights, pf


# revision 41
# speedup vs baseline: 1.2967x; 1.0153x over previous
"""PathDiscovery Trainium2 Bass kernel.

Full pipeline on-device, data-parallel over batch (1 batch element per
NeuronCore, 8 cores):
  - fchange: mean |nf[t+1]-nf[t]| over (t, F)
  - source MLP -> z = logits * fchange -> top-8 (max8/max_index)
  - guided random walk with restart, 8 walks x 4 steps, vectorized over
    walks on partitions; the categorical sample is reproduced exactly as
    argmax_j probs_j * exp(gumbel_j) with host-precomputed (input
    independent) gumbel noise; adj/feature rows fetched by indirect DMA
  - bidirectional GRU encoding of gathered path features + path scorer

Host side: shard inputs over cores, precompute exp(gumbel)/uniform draws
from jax CPU threefry (deterministic, input-independent), pre-transpose
small weights, assemble outputs, softmax over K for weights.
"""

import os
import subprocess
import sys
import tempfile

import numpy as np

import concourse.bass as bass
import concourse.bacc as bacc
import concourse.mybir as mybir
from concourse import tile
from concourse.bass_utils import run_bass_kernel_spmd

B, T, N, F = 8, 12, 4096, 64
H = 64
K = 8
L = 5
ALPHA = 0.15
NT = N // 128          # 32 node tiles of 128
NCHUNK = N // 512      # 8 matmul chunks of 512
NHALF = N // 2
FD = mybir.dt.float32
I32 = mybir.dt.int32
U32 = mybir.dt.uint32
ALU = mybir.AluOpType
ACT = mybir.ActivationFunctionType
AX = mybir.AxisListType


# --------------------------------------------------------------------------
# Host-side RNG precompute (input independent; must match jax CPU threefry)
# --------------------------------------------------------------------------

_RNG_CACHE = None

_RNG_SCRIPT = r"""
import numpy as np
import jax, jax.numpy as jnp
B, K, L, N = 8, 8, 5, 4096
wkeys = jax.random.split(jax.random.key(42), B * K).reshape(B, K)
GUM = np.zeros((B, K, L - 1, N), np.float32)
UNI = np.zeros((B, K, L - 1), np.float32)
for b in range(B):
    for k in range(K):
        step_keys = jax.random.split(wkeys[b, k], L - 1)
        for i in range(L - 1):
            ku, kc = jax.random.split(step_keys[i])
            GUM[b, k, i] = np.asarray(jax.random.gumbel(kc, (N,), jnp.float32))
            UNI[b, k, i] = float(jax.random.uniform(ku))
np.savez(OUT_PATH, gum=GUM, uni=UNI)
"""


def _host_rng():
    """exp(gumbel) (B,K,4,N) f32 and uniforms (B,K,4) f32, via jax CPU."""
    global _RNG_CACHE
    if _RNG_CACHE is not None:
        return _RNG_CACHE
    # jax in this process may be bound to the axon backend; compute in a
    # clean subprocess pinned to the CPU backend so the threefry bits match
    # the reference implementation exactly.
    import jax  # just to locate the installed jax for the subprocess

    site_dir = os.path.dirname(os.path.dirname(jax.__file__))
    env = dict(os.environ)
    env.pop("TRN_TERMINAL_POOL_IPS", None)  # disables axon sitecustomize boot
    env["JAX_PLATFORMS"] = "cpu"
    env["PYTHONPATH"] = site_dir + os.pathsep + env.get("PYTHONPATH", "")
    with tempfile.TemporaryDirectory() as td:
        out_path = os.path.join(td, "rng.npz")
        script = f"OUT_PATH = {out_path!r}\n" + _RNG_SCRIPT
        subprocess.run(
            [sys.executable, "-c", script], env=env, check=True,
            capture_output=True,
        )
        dat = np.load(out_path)
        gum, uni = dat["gum"], dat["uni"]
    eg = np.exp(gum.astype(np.float64)).astype(np.float32)
    _RNG_CACHE = (eg, uni)
    return _RNG_CACHE


# --------------------------------------------------------------------------
# Bass kernel builder
# --------------------------------------------------------------------------

def build_nc(num_devices=1, debug=False):
    nc = bacc.Bacc(
        "TRN2",
        target_bir_lowering=False,
        debug=debug,
        num_devices=num_devices,
    )

    d_nf = nc.dram_tensor("nf", [T, N, F], FD, kind="ExternalInput")
    d_feats = nc.dram_tensor("feats", [N, F], FD, kind="ExternalInput")
    d_adj = nc.dram_tensor("adj", [N, N], FD, kind="ExternalInput")
    d_w1t = nc.dram_tensor("w1t", [F, H], FD, kind="ExternalInput")
    d_b1 = nc.dram_tensor("b1", [H, 1], FD, kind="ExternalInput")
    d_w2t = nc.dram_tensor("w2t", [H, 1], FD, kind="ExternalInput")
    d_b2 = nc.dram_tensor("b2", [1, 1], FD, kind="ExternalInput")
    gru_dram = {}
    for pre in ("f", "b"):
        for nm, shape in (
            ("ih_rzT", [F, 2 * H]), ("ih_nT", [F, H]),
            ("hh_rzT", [H, 2 * H]), ("hh_nT", [H, H]),
            ("bih_rz", [2 * H, 1]), ("bih_n", [H, 1]),
            ("bhh_rz", [2 * H, 1]), ("bhh_n", [H, 1]),
        ):
            gru_dram[f"{pre}_{nm}"] = nc.dram_tensor(
                f"{pre}_{nm}", shape, FD, kind="ExternalInput")
    d_wps1t = nc.dram_tensor("wps1t", [2 * H, H], FD, kind="ExternalInput")
    d_bps1 = nc.dram_tensor("bps1", [H, 1], FD, kind="ExternalInput")
    d_wps2t = nc.dram_tensor("wps2t", [H, 1], FD, kind="ExternalInput")
    d_bps2 = nc.dram_tensor("bps2", [1, 1], FD, kind="ExternalInput")
    d_eg = nc.dram_tensor("eg", [L - 1, 64, 512], FD, kind="ExternalInput")
    d_uni = nc.dram_tensor("uni", [L - 1, 1, K], FD, kind="ExternalInput")
    d_ident = nc.dram_tensor("ident", [128, 128], FD, kind="ExternalInput")
    d_selmat = nc.dram_tensor("selmat", [8, 64], FD, kind="ExternalInput")
    d_repmat = nc.dram_tensor("repmat", [8, 64], FD, kind="ExternalInput")
    d_base512 = nc.dram_tensor("base512", [64, 1], FD, kind="ExternalInput")
    d_schunk = nc.dram_tensor("schunk", [64, 1], FD, kind="ExternalInput")

    d_paths = nc.dram_tensor("paths_o", [K, L], I32, kind="ExternalOutput")
    d_pfT = nc.dram_tensor("pfT_o", [2 * H, K], FD, kind="ExternalOutput")
    d_scores = nc.dram_tensor("scores_o", [1, K], FD, kind="ExternalOutput")
    d_fch = nc.dram_tensor("fch_o", [1, N], FD, kind="ExternalOutput")
    d_z = nc.dram_tensor("z_o", [1, N], FD, kind="ExternalOutput")

    with tile.TileContext(nc) as tc:
        with (
            tc.tile_pool(name="const", bufs=1) as constp,
            tc.tile_pool(name="big", bufs=1) as bigp,
            tc.tile_pool(name="row", bufs=3) as rowp,
            tc.tile_pool(name="small", bufs=3) as smp,
            tc.tile_pool(name="ps", bufs=4, space="PSUM") as psp,
            tc.tile_pool(name="psh", bufs=4, space="PSUM") as psh,
        ):
            # ---------------- constants ----------------
            ident = constp.tile([128, 128], FD, tag="ident")
            nc.sync.dma_start(ident[:], d_ident[:])
            w1t = constp.tile([F, H], FD, tag="w1t")
            nc.sync.dma_start(w1t[:], d_w1t[:])
            b1 = constp.tile([H, 1], FD, tag="b1")
            nc.sync.dma_start(b1[:], d_b1[:])
            w2t = constp.tile([H, 1], FD, tag="w2t")
            nc.sync.dma_start(w2t[:], d_w2t[:])
            b2 = constp.tile([1, 1], FD, tag="b2")
            nc.sync.dma_start(b2[:], d_b2[:])
            gru = {}
            for nm, dtr in gru_dram.items():
                t_ = constp.tile(list(dtr.shape), FD, tag=nm)
                nc.sync.dma_start(t_[:], dtr[:])
                gru[nm] = t_
            wps1t = constp.tile([2 * H, H], FD, tag="wps1t")
            nc.sync.dma_start(wps1t[:], d_wps1t[:])
            bps1 = constp.tile([H, 1], FD, tag="bps1")
            nc.sync.dma_start(bps1[:], d_bps1[:])
            wps2t = constp.tile([H, 1], FD, tag="wps2t")
            nc.sync.dma_start(wps2t[:], d_wps2t[:])
            bps2 = constp.tile([1, 1], FD, tag="bps2")
            nc.sync.dma_start(bps2[:], d_bps2[:])

            eg_t = []
            for s_ in range(L - 1):
                egt = constp.tile([64, 512], FD, tag=f"egt{s_}")
                nc.sync.dma_start(egt[:], d_eg[s_])
                eg_t.append(egt)
            ones_f8 = constp.tile([F, K], FD, tag="ones_f8")
            nc.vector.memset(ones_f8[:], 1.0)
            selmat = constp.tile([8, 64], FD, tag="selmat")
            nc.sync.dma_start(selmat[:], d_selmat[:])
            repmat = constp.tile([8, 64], FD, tag="repmat")
            nc.sync.dma_start(repmat[:], d_repmat[:])
            base512 = constp.tile([64, 1], FD, tag="base512")
            nc.sync.dma_start(base512[:], d_base512[:])
            schunk = constp.tile([64, 1], FD, tag="schunk")
            nc.sync.dma_start(schunk[:], d_schunk[:])

            iota512 = constp.tile([64, 512], FD, tag="iota512")
            nc.gpsimd.iota(
                iota512[:], pattern=[[1, 512]], base=0, channel_multiplier=0,
                allow_small_or_imprecise_dtypes=True,
            )

            uni_t = []
            for s in range(L - 1):
                t_ = constp.tile([1, K], FD, tag=f"uni{s}")
                nc.sync.dma_start(t_[:], d_uni[s])
                uni_t.append(t_)

            # persistent big tiles
            featsT = bigp.tile([F, N], FD, tag="featsT")
            sq64 = bigp.tile([64, 512], FD, tag="sq64")
            vis01 = bigp.tile([64, 512], FD, tag="vis01")
            src_col = bigp.tile([K, 1], FD, tag="src_col")
            path_f = bigp.tile([K, L], FD, tag="path_f")
            pfT = bigp.tile([2 * H, K], FD, tag="pfT")

            fch_row = rowp.tile([1, N], FD, tag="row")

            # ---------------- phase 1: fchange + featsT (scoped pools) ----
            with (
                tc.tile_pool(name="nfs", bufs=4) as nfp,
                tc.tile_pool(name="dts", bufs=3) as dtp,
            ):
                acc = nfp.tile([128, NT], FD, tag="acc")
                nc.vector.memset(acc[:], 0.0)
                # node j = p*32 + a: 8KB contiguous per partition per DMA
                nf_view = d_nf.ap().rearrange("t (p a) f -> t p a f", p=128)
                # nf[T-1] (= feats) is also a standalone input: load it
                # up front so featsT/sq/z matmuls overlap the 12.5MB stream
                feats2d = nfp.tile([128, NT * F], FD, tag="feats2d")
                nc.gpsimd.dma_start(
                    feats2d[:], d_feats.ap().rearrange(
                        "(a p) f -> p a f", p=128))
                prev = nfp.tile([128, NT * F], FD, tag="nft")
                nc.gpsimd.dma_start(prev[:], nf_view[0])
                for t in range(1, T):
                    curt = nfp.tile([128, NT * F], FD, tag="nft")
                    nc.gpsimd.dma_start(curt[:], nf_view[t])
                    dtile = dtp.tile([128, NT * F], FD, tag="dtile")
                    nc.vector.tensor_tensor(
                        dtile[:], curt[:], prev[:], op=ALU.subtract)
                    r = smp.tile([128, NT], FD, tag="red")
                    nc.vector.tensor_reduce(
                        r[:], dtile[:].rearrange("p (a f) -> p a f", f=F),
                        axis=AX.X, op=ALU.add, apply_absolute_value=True,
                    )
                    nc.vector.tensor_tensor(acc[:], acc[:], r[:], op=ALU.add)
                    prev = curt
                # fchange = acc / 704 -> transpose -> row
                fchm = smp.tile([128, NT], FD, tag="fchm")
                nc.vector.tensor_scalar(
                    fchm[:], acc[:],
                    float(np.float32(1.0) / np.float32((T - 1) * F)), None,
                    op0=ALU.mult)
                nc.sync.dma_start(fch_row[:], fchm[:])
                nc.sync.dma_start(d_fch[:], fch_row[:])

                # featsT from transposed 128x64 tiles of feats
                for a in range(NT):
                    tp = psp.tile([F, 128], FD, tag="mm")
                    nc.tensor.transpose(
                        tp[:], feats2d[:, a * F:(a + 1) * F], ident[:])
                    nc.scalar.copy(featsT[:, a * 128:(a + 1) * 128], tp[:])

            # ---------------- sq -> sq64 (64, 512) ----------------
            sq_row = rowp.tile([1, N], FD, tag="row")
            for c in range(NCHUNK):
                sl = slice(c * 512, (c + 1) * 512)
                sqc = smp.tile([F, 512], FD, tag="sqc")
                nc.scalar.square(sqc[:], featsT[:, sl])
                sq_ps = psp.tile([1, 512], FD, tag="mm")
                nc.tensor.matmul(sq_ps[:], ones_f8[:, 0:1], sqc[:],
                                 start=True, stop=True)
                nc.scalar.copy(sq_row[:, sl], sq_ps[:])
            sq_rs = smp.tile([8, 512], FD, tag="rs8")
            nc.sync.dma_start(sq_rs[:], sq_row[:])
            sq64_ps = psp.tile([64, 512], FD, tag="mm")
            nc.tensor.matmul(sq64_ps[:], selmat[:], sq_rs[:],
                             start=True, stop=True)
            nc.scalar.copy(sq64[:], sq64_ps[:])

            # ---------------- acc_bias -> vis01 = (1 + accb) ----------
            S = smp.tile([1, 1], FD, tag="S")
            nc.vector.tensor_reduce(
                S[:], fch_row[:], axis=AX.X, op=ALU.add)
            Sp = smp.tile([1, 1], FD, tag="Sp")
            nc.vector.tensor_scalar(Sp[:], S[:], 1e-8, None, op0=ALU.add)
            rSp = smp.tile([1, 1], FD, tag="rSp")
            nc.vector.reciprocal(rSp[:], Sp[:])
            accb_row = rowp.tile([1, N], FD, tag="row")
            nc.vector.tensor_scalar(
                accb_row[:], fch_row[:], rSp[:, 0:1], None, op0=ALU.mult)
            ac_rs = smp.tile([8, 512], FD, tag="rs8")
            nc.sync.dma_start(ac_rs[:], accb_row[:])
            ac64_ps = psp.tile([64, 512], FD, tag="mm")
            nc.tensor.matmul(ac64_ps[:], selmat[:], ac_rs[:],
                             start=True, stop=True)
            # vis01 starts as (1 + acc_bias); visited nodes zeroed in place
            nc.vector.tensor_scalar(
                vis01[:], ac64_ps[:], 1.0, None, op0=ALU.add)

            # ---------------- source logits z ----------------
            # raw logits don't need fchange -> computed during the stream
            zraw_row = rowp.tile([1, N], FD, tag="row")
            for c in range(NCHUNK):
                sl = slice(c * 512, (c + 1) * 512)
                h_ps = psp.tile([H, 512], FD, tag="mm")
                nc.tensor.matmul(h_ps[:], w1t[:], featsT[:, sl],
                                 start=True, stop=True)
                h_sb = smp.tile([H, 512], FD, tag="h_sb")
                nc.scalar.activation(
                    h_sb[:], h_ps[:], ACT.Relu, bias=b1[:, 0:1], scale=1.0)
                z_ps = psp.tile([1, 512], FD, tag="mm")
                nc.tensor.matmul(z_ps[:], w2t[:], h_sb[:],
                                 start=True, stop=True)
                nc.scalar.copy(zraw_row[:, sl], z_ps[:])
            z_row = rowp.tile([1, N], FD, tag="row")
            for c in range(NCHUNK):
                sl = slice(c * 512, (c + 1) * 512)
                nc.vector.scalar_tensor_tensor(
                    z_row[:, sl], zraw_row[:, sl], b2[:, 0:1],
                    fch_row[:, sl], op0=ALU.add, op1=ALU.mult)
            nc.sync.dma_start(d_z[:], z_row[:])

            # ---------------- top-8 sources ----------------
            zmax = smp.tile([1, 8], FD, tag="zmax")
            nc.vector.max(zmax[:], z_row[:])
            zidx = smp.tile([1, 8], U32, tag="zidx")
            nc.vector.max_index(zidx[:], zmax[:], z_row[:])
            zidx_f = smp.tile([1, 8], FD, tag="zidx_f")
            nc.vector.tensor_copy(zidx_f[:], zidx[:])
            nc.sync.dma_start(src_col[:], zidx_f[:])

            # ---------------- walk ----------------
            # layout: partition p = k*8 + s (walk k, chunk s); node
            # j = s*512 + f.  adj viewed as (4096*8, 512) row-chunks.
            # per-walk reductions go through a PE transpose to a (1, 64)
            # row, reduced in 8-groups along the free dim.
            adj_ck = d_adj.ap().rearrange("r (s c) -> (r s) c", c=512)

            def rep_walk_scalar(col8, tag):
                """(8,1) col -> (64,1) per-walk replication (repmat matmul)."""
                ps = psp.tile([64, 1], FD, tag="mm")
                nc.tensor.matmul(ps[:], repmat[:], col8[:],
                                 start=True, stop=True)
                sb = smp.tile([64, 1], FD, tag=tag)
                nc.scalar.copy(sb[:], ps[:])
                return sb

            def col_to_row(col, tag, n=8):
                """(n,1) -> (1,n) via PE transpose."""
                ps = psp.tile([1, n], FD, tag="mm")
                nc.tensor.transpose(ps[:], col[:, 0:1], ident[0:n, 0:n])
                sb = smp.tile([1, n], FD, tag=tag)
                nc.scalar.copy(sb[:], ps[:])
                return sb

            def row_to_col(row, tag):
                """(1,8) -> (8,1) via PE transpose."""
                ps = psp.tile([8, 1], FD, tag="mm")
                nc.tensor.transpose(ps[:], row[0:1, :], ident[0:1, 0:1])
                sb = smp.tile([8, 1], FD, tag=tag)
                nc.scalar.copy(sb[:], ps[:])
                return sb

            src_row = zidx_f        # (1, 8) source node ids as f32
            nc.vector.tensor_copy(path_f[:, 0:1], src_col[:])
            srcrep = rep_walk_scalar(src_col, "srcrep")
            srcadj = smp.tile([64, 1], FD, tag="srcadj")
            nc.vector.tensor_scalar(
                srcadj[:], srcrep[:], base512[:, 0:1], None,
                op0=ALU.subtract)
            nc.vector.scalar_tensor_tensor(
                vis01[:], iota512[:], srcadj[:, 0:1], vis01[:],
                op0=ALU.not_equal, op1=ALU.mult)

            cur_col = src_col
            xT = []
            with tc.tile_pool(name="wkp", bufs=3) as wkp:
                for pos in range(L):
                    cur_i = smp.tile([K, 1], I32, tag="cur_i")
                    nc.vector.tensor_copy(cur_i[:], cur_col[:])
                    frows = smp.tile([K, F], FD, tag="frows")
                    nc.gpsimd.indirect_dma_start(
                        out=frows[:], out_offset=None, in_=d_feats.ap(),
                        in_offset=bass.IndirectOffsetOnAxis(
                            ap=cur_i[:, 0:1], axis=0))
                    xt_ps = psp.tile([F, K], FD, tag="mm")
                    nc.tensor.transpose(xt_ps[:], frows[:], ident[0:K, 0:K])
                    xt = bigp.tile([F, K], FD, tag=f"xT{pos}")
                    nc.scalar.copy(xt[:], xt_ps[:])
                    xT.append(xt)
                    if pos == L - 1:
                        break
                    i = pos
                    # adj row-chunk gather: idx64 = cur*8 + s
                    currep = rep_walk_scalar(cur_col, "currep")
                    idx64f = smp.tile([64, 1], FD, tag="idx64f")
                    nc.vector.tensor_scalar(
                        idx64f[:], currep[:], 8.0, schunk[:, 0:1],
                        op0=ALU.mult, op1=ALU.add)
                    idx64 = smp.tile([64, 1], I32, tag="idx64")
                    nc.vector.tensor_copy(idx64[:], idx64f[:])
                    adjrows = wkp.tile([64, 512], FD, tag="adjrows")
                    nc.gpsimd.indirect_dma_start(
                        out=adjrows[:], out_offset=None, in_=adj_ck,
                        in_offset=bass.IndirectOffsetOnAxis(
                            ap=idx64[:, 0:1], axis=0))
                    eg_s = eg_t[i]
                    fr2 = smp.tile([K, F], FD, tag="fr2")
                    nc.scalar.square(fr2[:], frows[:])
                    sqcur = smp.tile([K, 1], FD, tag="sqcur")
                    nc.vector.tensor_reduce(
                        sqcur[:], fr2[:], axis=AX.X, op=ALU.add)
                    sqcur_r = rep_walk_scalar(sqcur, "sqcur_r")
                    # dots: 8 chunk matmuls -> (8, 4096) sbuf -> DMA
                    # reshape into the (64, 512) walk layout
                    dots8 = wkp.tile([K, N], FD, tag="dots8")
                    for hf in range(8):
                        dots_ps = psh.tile([K, 512], FD, tag="mmh")
                        sl_f = slice(hf * 512, (hf + 1) * 512)
                        nc.tensor.matmul(
                            dots_ps[:], xT[pos][:], featsT[:, sl_f],
                            start=True, stop=True)
                        nc.scalar.copy(dots8[:, sl_f], dots_ps[:])
                    d2 = wkp.tile([64, 512], FD, tag="d2")
                    nc.sync.dma_start(
                        d2[:],
                        dots8[:].rearrange("k (s c) -> (k s) c", c=512))
                    nc.vector.scalar_tensor_tensor(
                        d2[:], d2[:], -2.0, sq64[:],
                        op0=ALU.mult, op1=ALU.add)
                    nc.vector.tensor_scalar(
                        d2[:], d2[:], sqcur_r[:, 0:1], 0.0,
                        op0=ALU.add, op1=ALU.max)
                    m2p = smp.tile([64, 1], FD, tag="m2p")
                    nc.vector.tensor_reduce(
                        m2p[:], d2[:], axis=AX.X, op=ALU.max)
                    # sqrt(d2) starts immediately; the per-walk max
                    # round-trip runs concurrently and folds in as rsqrt
                    m2row = col_to_row(m2p, "m2row", n=64)
                    m2w = smp.tile([1, 8], FD, tag="m2w")
                    nc.vector.tensor_reduce(
                        m2w[:], m2row[:].rearrange("p (a b) -> p a b", b=8),
                        axis=AX.X, op=ALU.max)
                    sm2w = smp.tile([1, 8], FD, tag="sm2w")
                    nc.scalar.sqrt(sm2w[:], m2w[:])
                    rm2w = smp.tile([1, 8], FD, tag="rm2w")
                    nc.vector.reciprocal(rm2w[:], sm2w[:])
                    rm2c = row_to_col(rm2w, "rm2c")
                    rsm_r = rep_walk_scalar(rm2c, "rsm_r")
                    nc.scalar.sqrt(d2[:], d2[:])
                    # d2 <- sqrt(d2)*rsqrt(m2) + 1, then * eg
                    nc.vector.tensor_scalar(
                        d2[:], d2[:], rsm_r[:, 0:1], 1.0,
                        op0=ALU.mult, op1=ALU.add)
                    nc.vector.tensor_tensor(
                        d2[:], d2[:], eg_s[:], op=ALU.mult)
                    # y (in adjrows): (adj * (vis*accb1)) * ((guid+1) * eg)
                    nc.vector.tensor_tensor(
                        adjrows[:], adjrows[:], vis01[:], op=ALU.mult)
                    nc.vector.tensor_tensor(
                        adjrows[:], adjrows[:], d2[:], op=ALU.mult)
                    # per-partition top-1 then global per-walk argmax with
                    # first-index tie-break via eq-mask + min-index
                    y8 = smp.tile([64, 8], FD, tag="y8")
                    nc.vector.max(y8[:], adjrows[:])
                    yidx = smp.tile([64, 8], U32, tag="yidx")
                    nc.vector.max_index(yidx[:], y8[:], adjrows[:])
                    jf = smp.tile([64, 1], FD, tag="jf")
                    nc.vector.tensor_copy(jf[:], yidx[:, 0:1])
                    nc.vector.tensor_scalar(
                        jf[:], jf[:], base512[:, 0:1], None, op0=ALU.add)
                    yvrow = col_to_row(y8[:, 0:1], "yvrow", n=64)
                    jfrow = col_to_row(jf, "jfrow", n=64)
                    maxv = smp.tile([1, 8], FD, tag="maxv")
                    nc.vector.tensor_reduce(
                        maxv[:],
                        yvrow[:].rearrange("p (a b) -> p a b", b=8),
                        axis=AX.X, op=ALU.max)
                    eqm = smp.tile([1, 64], FD, tag="eqm")
                    nc.vector.tensor_tensor(
                        eqm[:].rearrange("p (a b) -> p a b", b=8),
                        yvrow[:].rearrange("p (a b) -> p a b", b=8),
                        maxv[:].unsqueeze(2).to_broadcast([1, 8, 8]),
                        op=ALU.is_equal)
                    cand = smp.tile([1, 64], FD, tag="cand")
                    nc.vector.scalar_tensor_tensor(
                        cand[:], eqm[:], -65536.0, jfrow[:],
                        op0=ALU.mult, op1=ALU.add)
                    nc.vector.tensor_scalar(
                        cand[:], cand[:], 65536.0, None, op0=ALU.add)
                    samp_row = smp.tile([1, 8], FD, tag="samp_row")
                    nc.vector.tensor_reduce(
                        samp_row[:],
                        cand[:].rearrange("p (a b) -> p a b", b=8),
                        axis=AX.X, op=ALU.min)
                    # teleport flags are input-independent RNG ->
                    # precomputed on host (uni holds 0/1 flags)
                    tel = uni_t[i]
                    dsrc = smp.tile([1, 8], FD, tag="dsrc")
                    nc.vector.tensor_tensor(
                        dsrc[:], src_row[:], samp_row[:], op=ALU.subtract)
                    td = smp.tile([1, 8], FD, tag="td")
                    nc.vector.tensor_tensor(
                        td[:], tel[:], dsrc[:], op=ALU.mult)
                    nxt_row = smp.tile([1, 8], FD, tag=f"nxtr{i}")
                    nc.vector.tensor_tensor(
                        nxt_row[:], td[:], samp_row[:], op=ALU.add)
                    nxt_col = row_to_col(nxt_row, f"nxtc{i}")
                    nc.vector.tensor_copy(path_f[:, i + 1:i + 2], nxt_col[:])
                    nxtrep = rep_walk_scalar(nxt_col, "nxtrep")
                    nxtadj = smp.tile([64, 1], FD, tag="nxtadj")
                    nc.vector.tensor_scalar(
                        nxtadj[:], nxtrep[:], base512[:, 0:1], None,
                        op0=ALU.subtract)
                    nc.vector.scalar_tensor_tensor(
                        vis01[:], iota512[:], nxtadj[:, 0:1], vis01[:],
                        op0=ALU.not_equal, op1=ALU.mult)
                    cur_col = nxt_col

            paths_i = smp.tile([K, L], I32, tag="paths_i")
            nc.vector.tensor_copy(paths_i[:], path_f[:])
            nc.sync.dma_start(d_paths[:], paths_i[:])

            # ---------------- GRU ----------------
            def gru_cell(h, xt, pre, hp):
                ps_i_rz = psp.tile([2 * H, K], FD, tag="mm")
                nc.tensor.matmul(ps_i_rz[:], gru[f"{pre}_ih_rzT"][:], xt[:],
                                 start=True, stop=True)
                ps_h_rz = psp.tile([2 * H, K], FD, tag="mm")
                nc.tensor.matmul(ps_h_rz[:], gru[f"{pre}_hh_rzT"][:], h[:],
                                 start=True, stop=True)
                gh_rz = smp.tile([2 * H, K], FD, tag="gh_rz")
                nc.vector.tensor_scalar(
                    gh_rz[:], ps_h_rz[:], gru[f"{pre}_bhh_rz"][:, 0:1], None,
                    op0=ALU.add)
                s_rz = smp.tile([2 * H, K], FD, tag="s_rz")
                nc.vector.scalar_tensor_tensor(
                    s_rz[:], ps_i_rz[:], gru[f"{pre}_bih_rz"][:, 0:1],
                    gh_rz[:], op0=ALU.add, op1=ALU.add)
                rz = smp.tile([2 * H, K], FD, tag="rz")
                nc.scalar.activation(rz[:], s_rz[:], ACT.Sigmoid)
                ps_i_n = psp.tile([H, K], FD, tag="mm")
                nc.tensor.matmul(ps_i_n[:], gru[f"{pre}_ih_nT"][:], xt[:],
                                 start=True, stop=True)
                ps_h_n = psp.tile([H, K], FD, tag="mm")
                nc.tensor.matmul(ps_h_n[:], gru[f"{pre}_hh_nT"][:], h[:],
                                 start=True, stop=True)
                inn = smp.tile([H, K], FD, tag="inn")
                nc.vector.tensor_scalar(
                    inn[:], ps_i_n[:], gru[f"{pre}_bih_n"][:, 0:1], None,
                    op0=ALU.add)
                hn = smp.tile([H, K], FD, tag="hn")
                nc.vector.tensor_scalar(
                    hn[:], ps_h_n[:], gru[f"{pre}_bhh_n"][:, 0:1], None,
                    op0=ALU.add)
                rh = smp.tile([H, K], FD, tag="rh")
                nc.vector.tensor_tensor(
                    rh[:], rz[0:H, :], hn[:], op=ALU.mult)
                npre = smp.tile([H, K], FD, tag="npre")
                nc.vector.tensor_tensor(npre[:], inn[:], rh[:], op=ALU.add)
                ng = smp.tile([H, K], FD, tag="ng")
                nc.scalar.activation(ng[:], npre[:], ACT.Tanh)
                zt = smp.tile([H, K], FD, tag="zt")
                nc.scalar.copy(zt[:], rz[H:2 * H, :])
                omz = smp.tile([H, K], FD, tag="omz")
                nc.vector.tensor_scalar(
                    omz[:], zt[:], -1.0, 1.0, op0=ALU.mult, op1=ALU.add)
                tb = smp.tile([H, K], FD, tag="tb")
                nc.vector.tensor_tensor(tb[:], omz[:], ng[:], op=ALU.mult)
                tcz = smp.tile([H, K], FD, tag="tc2")
                nc.vector.tensor_tensor(tcz[:], zt[:], h[:], op=ALU.mult)
                h2 = smp.tile([H, K], FD, tag=f"h_{pre}{hp}")
                nc.vector.tensor_tensor(h2[:], tb[:], tcz[:], op=ALU.add)
                return h2

            h_f = smp.tile([H, K], FD, tag="h_f0")
            nc.vector.memset(h_f[:], 0.0)
            for t in range(L):
                h_f = gru_cell(h_f, xT[t], "f", t + 1)
            h_b0 = smp.tile([H, K], FD, tag="h_b0")
            nc.vector.memset(h_b0[:], 0.0)
            h_b = gru_cell(h_b0, xT[L - 1], "b", 1)

            nc.vector.tensor_copy(pfT[0:H, :], h_f[:])
            nc.vector.tensor_copy(pfT[H:2 * H, :], h_b[:])
            nc.sync.dma_start(d_pfT[:], pfT[:])

            # ---------------- scorer ----------------
            ps1 = psp.tile([H, K], FD, tag="mm")
            nc.tensor.matmul(ps1[:], wps1t[:], pfT[:], start=True, stop=True)
            hs = smp.tile([H, K], FD, tag="hs")
            nc.scalar.activation(
                hs[:], ps1[:], ACT.Relu, bias=bps1[:, 0:1], scale=1.0)
            ps2 = psp.tile([1, K], FD, tag="mm")
            nc.tensor.matmul(ps2[:], wps2t[:], hs[:], start=True, stop=True)
            scores = smp.tile([1, K], FD, tag="scores")
            nc.scalar.activation(
                scores[:], ps2[:], ACT.Sigmoid, bias=bps2[:, 0:1], scale=1.0)
            nc.sync.dma_start(d_scores[:], scores[:])

    return nc


# --------------------------------------------------------------------------
# Host orchestration
# --------------------------------------------------------------------------

def _per_core_inputs(inputs, b, eg, uni):
    f32 = np.float32
    def c(x):
        return np.ascontiguousarray(np.asarray(x, dtype=f32))
    nf = c(inputs["node_features"][b])
    m = {
        "nf": nf,
        "feats": c(nf[T - 1]),
        "adj": c(inputs["adj_matrix"][b]),
        "w1t": c(inputs["W_sp1"].T),
        "b1": c(inputs["b_sp1"]).reshape(H, 1),
        "w2t": c(inputs["W_sp2"].T),
        "b2": c(inputs["b_sp2"]).reshape(1, 1),
        "wps1t": c(inputs["W_ps1"].T),
        "bps1": c(inputs["b_ps1"]).reshape(H, 1),
        "wps2t": c(inputs["W_ps2"].T),
        "bps2": c(inputs["b_ps2"]).reshape(1, 1),
        "eg": c(np.swapaxes(eg[b], 0, 1)).reshape(L - 1, 64, 512),
        "uni": c(np.swapaxes(uni[b], 0, 1) < np.float32(ALPHA))[:, None, :],
        "ident": np.eye(128, dtype=f32),
        "selmat": (np.arange(64)[None, :] % 8 == np.arange(8)[:, None]
                   ).astype(f32),
        "repmat": (np.arange(64)[None, :] // 8 == np.arange(8)[:, None]
                   ).astype(f32),
        "base512": ((np.arange(64) % 8) * 512).astype(f32).reshape(64, 1),
        "schunk": (np.arange(64) % 8).astype(f32).reshape(64, 1),
    }
    for pre, wih, whh, bih, bhh in (
        ("f", "Wih_f", "Whh_f", "bih_f", "bhh_f"),
        ("b", "Wih_b", "Whh_b", "bih_b", "bhh_b"),
    ):
        Wih, Whh = inputs[wih], inputs[whh]
        Bih, Bhh = inputs[bih], inputs[bhh]
        m[f"{pre}_ih_rzT"] = c(Wih[0:2 * H].T)
        m[f"{pre}_ih_nT"] = c(Wih[2 * H:3 * H].T)
        m[f"{pre}_hh_rzT"] = c(Whh[0:2 * H].T)
        m[f"{pre}_hh_nT"] = c(Whh[2 * H:3 * H].T)
        m[f"{pre}_bih_rz"] = c(Bih[0:2 * H]).reshape(2 * H, 1)
        m[f"{pre}_bih_n"] = c(Bih[2 * H:3 * H]).reshape(H, 1)
        m[f"{pre}_bhh_rz"] = c(Bhh[0:2 * H]).reshape(2 * H, 1)
        m[f"{pre}_bhh_n"] = c(Bhh[2 * H:3 * H]).reshape(H, 1)
    return m


_NC_CACHE = None
LAST_EXEC_NS = None


def _tunnel_ok(timeout=20.0):
    """Quick health probe of the axon relay before touching PJRT (a dead
    tunnel makes backend init hang indefinitely). Native-device setups
    (no relay env) skip the probe."""
    if not os.environ.get("TRN_TERMINAL_POOL_IPS"):
        return True
    import http.client
    try:
        conn = http.client.HTTPConnection("127.0.0.1", 8083, timeout=timeout)
        conn.request(
            "GET", "/init?rank=4294967295&topology=trn2.8x1&n_slices=1")
        resp = conn.getresponse()
        resp.read()
        conn.close()
        return resp.status == 200
    except Exception:
        return False


def _run_sim(nc, in_maps):
    """CoreSim fallback/debug path (KERNEL_BASS_SIM=1)."""
    from concourse import bass_interp
    outs = []
    for m in in_maps:
        sim = bass_interp.CoreSim(nc)
        for name, val in m.items():
            sim.tensor(name)[:] = val
        sim.simulate()
        outs.append({nm: np.array(sim.tensor(nm))
                     for nm in ("paths_o", "pfT_o", "scores_o")})
    return outs


def kernel(**inputs):
    global _NC_CACHE
    eg, uni = _host_rng()
    in_maps = [_per_core_inputs(inputs, b, eg, uni) for b in range(B)]
    global LAST_EXEC_NS
    use_sim = os.environ.get("KERNEL_BASS_SIM") == "1" or not _tunnel_ok()
    if not use_sim:
        try:
            if _NC_CACHE is None:
                _NC_CACHE = build_nc(num_devices=B)
            trace = os.environ.get("KERNEL_BASS_TRACE") == "1"
            res = run_bass_kernel_spmd(
                _NC_CACHE, in_maps, core_ids=list(range(B)), trace=trace)
            results = res.results
            if res.exec_time_ns is not None:
                LAST_EXEC_NS = res.exec_time_ns
        except Exception as e:
            import traceback
            traceback.print_exc()
            print(f"kernel: device path failed ({type(e).__name__}); "
                  "falling back to CoreSim", flush=True)
            use_sim = True
    if use_sim:
        nc1 = build_nc(num_devices=1)
        results = _run_sim(nc1, in_maps)
    paths = np.stack([results[b]["paths_o"] for b in range(B)])
    pf = np.stack([results[b]["pfT_o"].T for b in range(B)])
    scores = np.stack([results[b]["scores_o"][0] for b in range(B)])
    paths = paths.astype(np.int32)
    pf = pf.astype(np.float32)
    # weights = softmax over K of sigmoid scores
    sc = scores.astype(np.float32)[..., None]          # (B, K, 1)
    e = np.exp(sc - sc.max(axis=1, keepdims=True))
    weights = (e / e.sum(axis=1, keepdims=True)).astype(np.float32)
    return paths, weights, pf
